# revision 1
# baseline (speedup 1.0000x reference)
"""Trainium2 Bass kernel for nn_Net_17532056502451.

5 "think" iterations: shift-window cosine selector (159 shifts) + softmax
attention + scatter-back + conv-style encoder/decoder with energy argmax
(81 shifts), masked-MSE losses averaged.  Data-parallel: 1024 tokens over
8 cores, 128 tokens/core (one per SBUF partition), token-major fp32.

Mappings per core:
- dot correlation: 80 fused scalar_tensor_tensor MACs (DVE).
- sliding norms: Square + prefix-scan + strided diff.
- argmaxes: nc.vector.max / max_index (first-occurrence ties = jnp.argmax).
- per-token dynamic windows: GPSIMD ap_gather (idx[p,j] = off_p + j wrap
  trick, 16 candidate lanes) + 16-way predicated-copy diagonal extract.
- energy: quadratic Gram form. z[t,(d,i)] = ye[t,i]*ye[t,i+d] in ONE DVE op
  (overlapping APs), contracted with host-precomputed A via PE
  transpose -> PSUM->SBUF DMA -> accumulating matmuls.
- encoder/decoder: shared-weight matmuls on yhat (y_att embedded at d*),
  biases folded into PSUM->SBUF activation copies.
"""
import numpy as np

IDIM = 80
ODIM = 80
HDIM = 512
THINK_ITER = 5
TEMPER = 0.7
B, T = 4, 256
NTOK = B * T
P = 128
NCORES = 8
S1 = 159
S2 = 81
NFEAT = 80 * 80
NCHUNK = NFEAT // 128   # 50

_cache = {}


def _build_consts(W_enc, b_enc, W_src, b_src):
    W_enc = np.asarray(W_enc, np.float32)
    b_enc = np.asarray(b_enc, np.float32)
    W_src = np.asarray(W_src, np.float32)
    b_src = np.asarray(b_src, np.float32)
    C = (W_enc.T @ W_enc).astype(np.float32)
    q = (W_enc.T @ b_enc).astype(np.float32)
    bb = np.float32(b_enc @ b_enc)
    # E[t,s] = sum_{d,i} Az[s, 80d+i] * ye_i ye_{i+d} + sum_i 2 q[dd+i] ye_i + bb,
    # dd = 80 - s
    Az = np.zeros((S2, NFEAT), np.float32)
    Al = np.zeros((S2, 81), np.float32)
    for s in range(S2):
        dd = 80 - s
        blk = C[dd:dd + 80, dd:dd + 80]
        for d in range(80):
            diag = np.diagonal(blk, offset=d).copy()
            Az[s, d * 80: d * 80 + (80 - d)] = (2.0 if d > 0 else 1.0) * diag
        Al[s, :80] = 2.0 * q[dd:dd + 80]
        Al[s, 80] = bb
    Az_cat = np.ascontiguousarray(Az.T)               # (6400, 81): pi-major
    Atail = np.ascontiguousarray(Al.T)                # (81, 81)
    W_encT = np.ascontiguousarray(W_enc.T)            # (160, 512)
    W_srcT = np.ascontiguousarray(W_src.T)            # (512, 160)
    M16 = np.zeros((P, 16), np.uint8)
    for p in range(P):
        M16[p, p % 16] = 1
    iota80 = np.broadcast_to(np.arange(80, dtype=np.float32), (P, 80)).copy()
    iota160 = np.broadcast_to(np.arange(160, dtype=np.float32), (P, 160)).copy()
    ident = np.eye(128, dtype=np.float32)
    benc4 = b_enc.reshape(4, 128).T.copy()            # (128, 4)
    bsrc2 = np.zeros((128, 2), np.float32)
    bsrc2[:, 0] = b_src[0:128]
    bsrc2[0:32, 1] = b_src[128:160]
    return dict(Az=Az_cat, Atail=Atail, WencT=W_encT, WsrcT=W_srcT,
                benc=benc4, bsrc=bsrc2, M16=M16, iota80=iota80,
                iota160=iota160, ident=ident,
                ones1=np.ones((1, 128), np.float32))


def _build_nc():
    import concourse.bass as bass
    import concourse.bacc as bacc
    import concourse.mybir as mybir
    from concourse.tile import TileContext

    F32 = mybir.dt.float32
    I16 = mybir.dt.int16
    U32 = mybir.dt.uint32
    Op = mybir.AluOpType
    AF = mybir.ActivationFunctionType

    nc = bacc.Bacc()
    d_x = nc.declare_dram_parameter("x", [P, 80], F32, isOutput=False)
    d_y = nc.declare_dram_parameter("y", [P, 80], F32, isOutput=False)
    d_A = nc.declare_dram_parameter("Az", [NFEAT, 81], F32, isOutput=False)
    d_At = nc.declare_dram_parameter("Atail", [81, 81], F32, isOutput=False)
    d_We = nc.declare_dram_parameter("WencT", [160, 512], F32, isOutput=False)
    d_Ws = nc.declare_dram_parameter("WsrcT", [512, 160], F32, isOutput=False)
    d_be = nc.declare_dram_parameter("benc", [128, 4], F32, isOutput=False)
    d_bs = nc.declare_dram_parameter("bsrc", [128, 2], F32, isOutput=False)
    d_M = nc.declare_dram_parameter("M16", [P, 16], mybir.dt.uint8, isOutput=False)
    d_i80 = nc.declare_dram_parameter("iota80", [P, 80], F32, isOutput=False)
    d_i160 = nc.declare_dram_parameter("iota160", [P, 160], F32, isOutput=False)
    d_id = nc.declare_dram_parameter("ident", [128, 128], F32, isOutput=False)
    d_on = nc.declare_dram_parameter("ones1", [1, 128], F32, isOutput=False)
    d_out = nc.declare_dram_parameter("losspart", [P, 8], F32, isOutput=True)

    with TileContext(nc) as tc:
        with (
            tc.tile_pool(name="const", bufs=1) as cpool,
            tc.tile_pool(name="work", bufs=1) as pool,
            tc.tile_pool(name="zrot", bufs=3) as zpool,
            tc.tile_pool(name="ps_rot", bufs=3, space="PSUM") as pp,
            tc.tile_pool(name="ps_acc", bufs=1, space="PSUM") as ppe,
        ):
            # ---- constants ----
            A_t = cpool.tile([P, NCHUNK * 81], F32, tag="A")
            for k in range(NCHUNK):
                nc.sync.dma_start(A_t[:, k * 81:(k + 1) * 81],
                                  d_A[k * 128:(k + 1) * 128, :])
            At_t = cpool.tile([81, 81], F32, tag="At")
            nc.sync.dma_start(At_t[:], d_At[:])
            We_t = cpool.tile([P, 2 * 512], F32, tag="We")
            nc.sync.dma_start(We_t[:, 0:512], d_We[0:128, :])
            nc.sync.dma_start(We_t[0:32, 512:1024], d_We[128:160, :])
            Ws_t = cpool.tile([P, 4 * 160], F32, tag="Ws")
            for k in range(4):
                nc.sync.dma_start(Ws_t[:, k * 160:(k + 1) * 160],
                                  d_Ws[k * 128:(k + 1) * 128, :])
            be_t = cpool.tile([128, 4], F32, tag="be")
            nc.sync.dma_start(be_t[:], d_be[:])
            bs_t = cpool.tile([128, 2], F32, tag="bs")
            nc.sync.dma_start(bs_t[:], d_bs[:])
            M_t = cpool.tile([P, 16], mybir.dt.uint8, tag="M")
            nc.sync.dma_start(M_t[:], d_M[:])
            i80_t = cpool.tile([P, 80], F32, tag="i80")
            nc.sync.dma_start(i80_t[:], d_i80[:])
            i160_t = cpool.tile([P, 160], F32, tag="i160")
            nc.sync.dma_start(i160_t[:], d_i160[:])
            id_t = cpool.tile([128, 128], F32, tag="id")
            nc.sync.dma_start(id_t[:], d_id[:])

            # ---- state ----
            xpad = pool.tile([P, 238], F32, tag="xpad")
            yres = pool.tile([P, 80], F32, tag="yres")
            keep = pool.tile([P, 80], F32, tag="keep")
            yap = pool.tile([P, 240], F32, tag="yap")
            lossp = pool.tile([P, 8], F32, tag="lossp")
            nc.vector.memset(xpad[:], 0.0)
            nc.vector.memset(yap[:], 0.0)
            nc.vector.memset(lossp[:], 0.0)
            nc.sync.dma_start(xpad[:, 79:159], d_x[:])
            nc.sync.dma_start(yres[:], d_y[:])
            nc.vector.tensor_scalar(keep[:], yres[:], 0.0, None, Op.not_equal)

            sqx = pool.tile([P, 239], F32, tag="sqx")
            nc.vector.memset(sqx[:, 0:1], 0.0)
            cs = pool.tile([P, 239], F32, tag="cs")
            nsq = pool.tile([P, S1], F32, tag="nsq")
            dot = pool.tile([P, S1], F32, tag="dot")
            adot = pool.tile([P, S1], F32, tag="adot")
            gsel = pool.tile([P, S1], F32, tag="gsel")
            rnsq = pool.tile([P, S1], F32, tag="rnsq")
            mx8 = pool.tile([P, 8], F32, tag="mx8")
            mi8 = pool.tile([P, 8], U32, tag="mi8")
            thf = pool.tile([P, 1], F32, tag="thf")
            idxf = pool.tile([P, 160], F32, tag="idxf")
            idxi = pool.tile([P, 160], I16, tag="idxi")
            g1280 = pool.tile([P, 1280], F32, tag="g1280")
            g2560 = pool.tile([P, 2560], F32, tag="g2560")
            yal = pool.tile([P, 80], F32, tag="yal")
            zt = pool.tile([P, 80], F32, tag="zt")
            et = pool.tile([P, 80], F32, tag="et")
            ssum = pool.tile([P, 1], F32, tag="ssum")
            rsum = pool.tile([P, 1], F32, tag="rsum")
            nzm = pool.tile([P, 1], F32, tag="nzm")
            zero1 = pool.tile([P, 1], F32, tag="zero1")
            nc.vector.memset(zero1[:], 0.0)
            xele = pool.tile([P, 80], F32, tag="xele")
            zfeat = pool.tile([P, NFEAT], F32, tag="zfeat")
            e81 = pool.tile([81, 128], F32, tag="e81")
            etail = pool.tile([81, 128], F32, tag="etail")
            nc.sync.dma_start(etail[80:81, :], d_on[:])
            Etok = pool.tile([P, S2], F32, tag="Etok")
            sf = pool.tile([P, 1], F32, tag="sf")
            df = pool.tile([P, 1], F32, tag="df")
            yhat = pool.tile([P, 160], F32, tag="yhat")
            yhT0 = pool.tile([128, 128], F32, tag="yhT0")
            yhT1 = pool.tile([32, 128], F32, tag="yhT1")
            hsT = pool.tile([128, 4 * 128], F32, tag="hsT")
            xeT0 = pool.tile([128, 128], F32, tag="xeT0")
            xeT1 = pool.tile([32, 128], F32, tag="xeT1")
            xext = pool.tile([P, 160], F32, tag="xext")
            yele = pool.tile([P, 80], F32, tag="yele")
            dtmp = pool.tile([P, 80], F32, tag="dtmp")

            ye_view = yap[:, 80:240]

            def gather_extract(src_ap, src_elems, width, out_tile, gbuf):
                """out[p, j] = src[p, idxf[p, j]], j in [0,width)."""
                nc.vector.tensor_copy(idxi[:, 0:width], idxf[:, 0:width])
                nc.gpsimd.ap_gather(gbuf[:, 0:16 * width], src_ap,
                                    idxi[:, 0:width], channels=128,
                                    num_elems=src_elems, d=1,
                                    num_idxs=16 * width)
                gv = gbuf[:, 0:16 * width].rearrange("p (j k) -> p j k", k=16)
                for k in range(16):
                    nc.vector.copy_predicated(
                        out_tile[:, 0:width],
                        M_t[:, k:k + 1].to_broadcast((P, width)),
                        gv[:, :, k])

            for it in range(THINK_ITER):
                # --- sliding norms ---
                nc.scalar.activation(sqx[:, 1:239], xpad[:], AF.Square)
                nc.vector.tensor_tensor_scan(cs[:], sqx[:],
                                             zero1[:].to_broadcast((P, 239)),
                                             0.0, Op.add, Op.bypass)
                nc.vector.tensor_tensor(nsq[:], cs[:, 80:239], cs[:, 0:159],
                                        Op.subtract)
                # --- dot: 80 MACs ---
                nc.vector.tensor_scalar_mul(dot[:], xpad[:, 0:S1], yres[:, 0:1])
                for c in range(1, 80):
                    nc.vector.scalar_tensor_tensor(dot[:], xpad[:, c:c + S1],
                                                   yres[:, c:c + 1], dot[:],
                                                   Op.mult, Op.add)
                # --- theta = argmax dot*|dot|/nsq ---
                nc.scalar.activation(adot[:], dot[:], AF.Abs)
                nc.vector.tensor_scalar_max(rnsq[:], nsq[:], 1e-30)
                nc.vector.reciprocal(rnsq[:], rnsq[:])
                nc.vector.tensor_tensor(gsel[:], dot[:], adot[:], Op.mult)
                nc.vector.tensor_tensor(gsel[:], gsel[:], rnsq[:], Op.mult)
                nc.vector.max(mx8[:], gsel[:])
                nc.vector.max_index(mi8[:], mx8[:], gsel[:])
                nc.vector.tensor_copy(thf[:], mi8[:, 0:1])
                # --- y_align gather ---
                nc.vector.scalar_tensor_tensor(idxf[:, 0:80], i80_t[:],
                                               thf[:, 0:1], i80_t[:],
                                               Op.add, Op.bypass)
                gather_extract(xpad[:], 238, 80, yal, g1280)
                # --- softmax attention -> y_att in yap[:, 80:160] ---
                nc.vector.tensor_tensor(zt[:], yal[:], yres[:], Op.mult)
                nc.vector.max(mx8[:], zt[:])
                nc.vector.tensor_scalar_mul(nzm[:], mx8[:, 0:1], -1.0 / TEMPER)
                nc.scalar.activation(et[:], zt[:], AF.Exp, bias=nzm[:, 0:1],
                                     scale=1.0 / TEMPER)
                nc.vector.tensor_reduce(ssum[:], et[:], mybir.AxisListType.X, Op.add)
                nc.vector.reciprocal(rsum[:], ssum[:])
                nc.vector.tensor_tensor(et[:], et[:], yal[:], Op.mult)
                nc.vector.tensor_scalar_mul(yap[:, 80:160], et[:], rsum[:, 0:1])
                # --- z features: z[p, 80d+i] = ye[i] * ye[i+d] ---
                in0 = ye_view[:, 0:80].unsqueeze(1).to_broadcast((P, 80, 80))
                in1 = bass.AP(ye_view.tensor, ye_view.offset,
                              [list(ye_view.ap[0]), [1, 80], [1, 80]])
                zv = zfeat[:].rearrange("p (d i) -> p d i", i=80)
                nc.vector.tensor_tensor(zv, in0, in1, Op.mult)
                # --- x_ele gather: idx = iota80 + (159 - theta) ---
                nc.vector.tensor_scalar_mul(thf[:], thf[:], -1.0)
                nc.vector.tensor_scalar_add(thf[:], thf[:], 159.0)
                nc.vector.scalar_tensor_tensor(idxf[:, 0:80], i80_t[:],
                                               thf[:, 0:1], i80_t[:],
                                               Op.add, Op.bypass)
                gather_extract(yap[:], 240, 80, xele, g1280)
                nc.vector.tensor_tensor(xpad[:, 79:159], xpad[:, 79:159],
                                        xele[:], Op.subtract)
                # --- E accumulation: pipelined T -> DMA -> MM ---
                Eps = ppe.tile([81, 128], F32, tag="Eps")
                zsb = [None] * NCHUNK
                for k in range(NCHUNK + 2):
                    if k < NCHUNK:
                        zTp = pp.tile([128, 128], F32, tag="zTp")
                        nc.tensor.transpose(zTp[:],
                                            zfeat[:, k * 128:(k + 1) * 128],
                                            id_t[:])
                        zsb_k = zpool.tile([128, 128], F32, tag="zT")
                        zsb[k] = zsb_k
                        nc.scalar.copy(zsb[k][:], zTp[:])
                    j = k - 2
                    if 0 <= j < NCHUNK:
                        nc.tensor.matmul(Eps[:], A_t[:, j * 81:(j + 1) * 81],
                                         zsb[j][:], start=(j == 0), stop=False)
                # tail: feats [ya(80); 1]
                yaTp = pp.tile([128, 128], F32, tag="zTp")
                nc.tensor.transpose(yaTp[0:80, :], yap[:, 80:160], id_t[:])
                nc.scalar.copy(etail[0:80, :], yaTp[0:80, :])
                nc.tensor.matmul(Eps[:], At_t[:], etail[:], start=False,
                                 stop=True)
                # E back to token-major
                nc.scalar.copy(e81[:], Eps[:])
                Etp = pp.tile([128, 128], F32, tag="zTp")
                nc.tensor.transpose(Etp[:, 0:81], e81[:], id_t[0:81, 0:81])
                nc.scalar.copy(Etok[:], Etp[:, 0:81])
                # --- s* argmax, d* = 80 - s* ---
                nc.vector.max(mx8[:], Etok[:])
                nc.vector.max_index(mi8[:], mx8[:], Etok[:])
                nc.vector.tensor_copy(sf[:], mi8[:, 0:1])
                nc.vector.tensor_scalar_mul(df[:], sf[:], -1.0)
                nc.vector.tensor_scalar_add(df[:], df[:], 80.0)
                # --- yhat embed: idx = iota160 + s* ---
                nc.vector.scalar_tensor_tensor(idxf[:, 0:160], i160_t[:],
                                               sf[:, 0:1], i160_t[:],
                                               Op.add, Op.bypass)
                gather_extract(yap[:], 240, 80, yhat, g1280)
                nc.vector.tensor_copy(idxi[:, 0:80], idxf[:, 80:160])
                nc.gpsimd.ap_gather(g1280[:], yap[:], idxi[:, 0:80],
                                    channels=128, num_elems=240, d=1,
                                    num_idxs=1280)
                gv2 = g1280[:].rearrange("p (j k) -> p j k", k=16)
                for k2 in range(16):
                    nc.vector.copy_predicated(
                        yhat[:, 80:160],
                        M_t[:, k2:k2 + 1].to_broadcast((P, 80)),
                        gv2[:, :, k2])
                # --- h_selT = W_enc @ yhat^T (+ b_enc) ---
                yhTp = pp.tile([128, 128], F32, tag="zTp")
                nc.tensor.transpose(yhTp[:], yhat[:, 0:128], id_t[:])
                nc.scalar.copy(yhT0[:], yhTp[:])
                yhTp2 = pp.tile([128, 128], F32, tag="zTp")
                nc.tensor.transpose(yhTp2[0:32, :], yhat[:, 128:160], id_t[:])
                nc.scalar.copy(yhT1[:], yhTp2[0:32, :])
                for hc in range(4):
                    Hp = pp.tile([128, 128], F32, tag="Hp")
                    nc.tensor.matmul(Hp[:], We_t[:, hc * 128:(hc + 1) * 128],
                                     yhT0[:], start=True, stop=False)
                    nc.tensor.matmul(Hp[:],
                                     We_t[0:32, 512 + hc * 128:512 + (hc + 1) * 128],
                                     yhT1[:], start=False, stop=True)
                    nc.scalar.copy(hsT[:, hc * 128:(hc + 1) * 128], Hp[:])
                    nc.vector.tensor_scalar_add(hsT[:, hc * 128:(hc + 1) * 128],
                                                hsT[:, hc * 128:(hc + 1) * 128],
                                                be_t[:, hc:hc + 1])
                # --- x_extT = W_src @ h_selT (+ b_src) ---
                for oc in range(2):
                    ow = 128 if oc == 0 else 32
                    Xp = pp.tile([128, 128], F32, tag="Hp")
                    for hc in range(4):
                        nc.tensor.matmul(
                            Xp[0:ow, :],
                            Ws_t[:, hc * 160 + oc * 128: hc * 160 + oc * 128 + ow],
                            hsT[:, hc * 128:(hc + 1) * 128],
                            start=(hc == 0), stop=(hc == 3))
                    dst = xeT0 if oc == 0 else xeT1
                    nc.scalar.copy(dst[:], Xp[0:ow, :])
                    nc.vector.tensor_scalar_add(dst[:], dst[:],
                                                bs_t[0:ow, oc:oc + 1])
                Xtp = pp.tile([128, 128], F32, tag="Hp")
                nc.tensor.transpose(Xtp[:], xeT0[:], id_t[:])
                nc.scalar.copy(xext[:, 0:128], Xtp[:])
                Xtp2 = pp.tile([128, 128], F32, tag="Hp")
                nc.tensor.transpose(Xtp2[:, 0:32], xeT1[:], id_t[0:32, 0:32])
                nc.scalar.copy(xext[:, 128:160], Xtp2[:, 0:32])
                # --- y_ele gather: idx = iota80 + d* ---
                nc.vector.scalar_tensor_tensor(idxf[:, 0:80], i80_t[:],
                                               df[:, 0:1], i80_t[:],
                                               Op.add, Op.bypass)
                gather_extract(xext[:], 160, 80, yele, g1280)
                # --- loss partial + state updates ---
                nc.vector.tensor_tensor(dtmp[:], yele[:], yres[:], Op.subtract)
                nc.vector.tensor_tensor(dtmp[:], dtmp[:], keep[:], Op.mult)
                nc.vector.tensor_tensor(et[:], dtmp[:], dtmp[:], Op.mult)
                nc.vector.tensor_reduce(lossp[:, it:it + 1], et[:],
                                        mybir.AxisListType.X, Op.add)
                nc.vector.tensor_tensor(yres[:], yres[:], yele[:], Op.subtract)

            nc.sync.dma_start(d_out[:], lossp[:])
    return nc


def kernel(x, y, W_enc, b_enc, W_src, b_src):
    import sys
    if '/opt/trn_rl_repo' not in sys.path:
        sys.path.insert(0, '/opt/trn_rl_repo')
    x = np.asarray(x, np.float32)
    y = np.asarray(y, np.float32)
    consts = _build_consts(W_enc, b_enc, W_src, b_src)

    if "nc" not in _cache:
        _cache["nc"] = _build_nc()
        _cache["nc"].finalize()
    nc = _cache["nc"]

    xt = x.reshape(NTOK, IDIM)
    yt = y.reshape(NTOK, ODIM)
    in_maps = []
    for c in range(NCORES):
        m = dict(consts)
        m["x"] = np.ascontiguousarray(xt[c * P:(c + 1) * P])
        m["y"] = np.ascontiguousarray(yt[c * P:(c + 1) * P])
        in_maps.append(m)

    from concourse.bass_utils import run_bass_kernel_spmd
    res = run_bass_kernel_spmd(nc, in_maps, list(range(NCORES)))
    parts = np.stack([r["losspart"] for r in res.results])
    keep_cnt = max(int((y != 0.0).sum()), 1)
    nums = parts[:, :, :THINK_ITER].sum(axis=(0, 1), dtype=np.float64)
    losses = (nums / keep_cnt).astype(np.float32)
    return np.float32(np.mean(losses))



# revision 3
# speedup vs baseline: 4.4183x; 4.4183x over previous
"""Trainium2 Bass kernel for nn_Net_17532056502451.

5 "think" iterations: shift-window cosine selector (159 shifts) + softmax
attention + scatter-back + conv-style encoder/decoder with energy argmax
(81 shifts), masked-MSE losses averaged.  Data-parallel: 1024 tokens over
8 cores, 128 tokens/core (one per SBUF partition), token-major.

v2 design notes (perf rework of the working v1 baseline):
- dot correlation: ONE broadcast-product DVE op (fp16 out) + ONE strided
  tensor_reduce (fp16 2x mode) instead of 80 scalar_tensor_tensor MACs.
- energy Gram features packed 6400 -> 4096 (d<32: i<80, d<48: i<48,
  d<64: i<32, d<80: i<16) = 32 chunks of 128 instead of 50.
- whole PE path in fp16: 1-pass matmuls + FWL instead of 4-pass fp32.
  Transposes via regular matmul against a fp16 identity.
- E-matmul operands swapped (stationary = zT chunk, moving = Az chunk) so
  E accumulates TOKEN-major in PSUM: argmax reads PSUM directly, no
  transpose-back.
- per-token dynamic window gathers via gpsimd local_scatter (per-partition
  indices: idx[p,j] = j - start_p, negatives ignored) -- replaces the v1
  16-lane ap_gather + 16 copy_predicated extraction entirely.
- loss: (d*keep) then ACT Square with accum_out (one op for square+sum).
"""
import numpy as np

IDIM = 80
ODIM = 80
HDIM = 512
THINK_ITER = 5
TEMPER = 0.7
B, T = 4, 256
NTOK = B * T
P = 128
NCORES = 8
S1 = 159
S2 = 81
# packed z-feature blocks: (d0, d1, imax)
ZBLOCKS = [(0, 32, 80), (32, 48, 48), (48, 64, 32), (64, 80, 16)]
NFEAT = sum((d1 - d0) * im for d0, d1, im in ZBLOCKS)   # 4096
NCHUNK = NFEAT // 128   # 32

_cache = {}


def _feat_list():
    feats = []
    for d0, d1, im in ZBLOCKS:
        for d in range(d0, d1):
            for i in range(im):
                feats.append((d, i))
    return feats


def _build_consts(W_enc, b_enc, W_src, b_src):
    W_enc = np.asarray(W_enc, np.float32)
    b_enc = np.asarray(b_enc, np.float32)
    W_src = np.asarray(W_src, np.float32)
    b_src = np.asarray(b_src, np.float32)
    C = (W_enc.T @ W_enc).astype(np.float32)
    q = (W_enc.T @ b_enc).astype(np.float32)
    bb = np.float32(b_enc @ b_enc)
    # E[t,s] = sum_f Az[s, f] * z[t, f] + sum_i 2 q[dd+i] ya_i + bb, dd = 80-s
    # z[t, f] = ya_i * ya_{i+d} for f -> (d, i) per ZBLOCKS packing.
    feats = _feat_list()
    Az = np.zeros((S2, NFEAT), np.float32)
    Al = np.zeros((S2, 81), np.float32)
    for s in range(S2):
        dd = 80 - s
        for f, (d, i) in enumerate(feats):
            if i < 80 - d:
                Az[s, f] = (2.0 if d > 0 else 1.0) * C[dd + i, dd + i + d]
        Al[s, :80] = 2.0 * q[dd:dd + 80]
        Al[s, 80] = bb
    Az_cat = np.ascontiguousarray(Az.T).astype(np.float16)   # (4096, 81)
    Atail = np.ascontiguousarray(Al.T).astype(np.float16)    # (81, 81)
    W_encT = np.ascontiguousarray(W_enc.T).astype(np.float16)  # (160, 512)
    W_srcT = np.ascontiguousarray(W_src.T).astype(np.float16)  # (512, 160)
    iota256 = np.broadcast_to(np.arange(256, dtype=np.float32), (P, 256)).copy()
    ident = np.eye(128, dtype=np.float16)
    benc4 = b_enc.reshape(4, 128).T.copy()            # (128, 4)
    bsrc2 = np.zeros((128, 2), np.float32)
    bsrc2[:, 0] = b_src[0:128]
    bsrc2[0:32, 1] = b_src[128:160]
    return dict(Az=Az_cat, Atail=Atail, WencT=W_encT, WsrcT=W_srcT,
                benc=benc4, bsrc=bsrc2, iota256=iota256,
                ident=ident, ones1=np.ones((1, 128), np.float16))


def _build_nc():
    import concourse.bass as bass
    import concourse.bacc as bacc
    import concourse.mybir as mybir
    from concourse.tile import TileContext

    F32 = mybir.dt.float32
    F16 = mybir.dt.float16
    I16 = mybir.dt.int16
    U32 = mybir.dt.uint32
    Op = mybir.AluOpType
    AF = mybir.ActivationFunctionType

    nc = bacc.Bacc()
    d_x = nc.declare_dram_parameter("x", [P, 80], F32, isOutput=False)
    d_y = nc.declare_dram_parameter("y", [P, 80], F32, isOutput=False)
    d_A = nc.declare_dram_parameter("Az", [NFEAT, 81], F16, isOutput=False)
    d_At = nc.declare_dram_parameter("Atail", [81, 81], F16, isOutput=False)
    d_We = nc.declare_dram_parameter("WencT", [160, 512], F16, isOutput=False)
    d_Ws = nc.declare_dram_parameter("WsrcT", [512, 160], F16, isOutput=False)
    d_be = nc.declare_dram_parameter("benc", [128, 4], F32, isOutput=False)
    d_bs = nc.declare_dram_parameter("bsrc", [128, 2], F32, isOutput=False)
    d_io = nc.declare_dram_parameter("iota256", [P, 256], F32, isOutput=False)
    d_id = nc.declare_dram_parameter("ident", [128, 128], F16, isOutput=False)
    d_on = nc.declare_dram_parameter("ones1", [1, 128], F16, isOutput=False)
    d_out = nc.declare_dram_parameter("losspart", [P, 8], F32, isOutput=True)

    with TileContext(nc) as tc:
        with (
            tc.tile_pool(name="const", bufs=1) as cpool,
            tc.tile_pool(name="work", bufs=1) as pool,
            tc.tile_pool(name="zrot", bufs=3) as zpool,
            tc.tile_pool(name="ps_rot", bufs=3, space="PSUM") as pp,
            tc.tile_pool(name="ps_acc", bufs=1, space="PSUM") as ppe,
        ):
            # ---- constants ----
            A_t = cpool.tile([P, NCHUNK * 81], F16, tag="A")
            for k in range(NCHUNK):
                nc.sync.dma_start(A_t[:, k * 81:(k + 1) * 81],
                                  d_A[k * 128:(k + 1) * 128, :])
            At_t = cpool.tile([81, 81], F16, tag="At")
            nc.sync.dma_start(At_t[:], d_At[:])
            We_t = cpool.tile([P, 2 * 512], F16, tag="We")
            nc.sync.dma_start(We_t[:, 0:512], d_We[0:128, :])
            nc.sync.dma_start(We_t[0:32, 512:1024], d_We[128:160, :])
            Ws_t = cpool.tile([P, 4 * 160], F16, tag="Ws")
            for k in range(4):
                nc.sync.dma_start(Ws_t[:, k * 160:(k + 1) * 160],
                                  d_Ws[k * 128:(k + 1) * 128, :])
            be_t = cpool.tile([128, 4], F32, tag="be")
            nc.sync.dma_start(be_t[:], d_be[:])
            bs_t = cpool.tile([128, 2], F32, tag="bs")
            nc.sync.dma_start(bs_t[:], d_bs[:])
            io_t = cpool.tile([P, 256], F32, tag="io")
            nc.sync.dma_start(io_t[:], d_io[:])
            id_t = cpool.tile([128, 128], F16, tag="id")
            nc.sync.dma_start(id_t[:], d_id[:])

            # ---- state ----
            xpad = pool.tile([P, 238], F32, tag="xpad")
            xpad16 = pool.tile([P, 238], F16, tag="xpad16")
            yres = pool.tile([P, 80], F32, tag="yres")
            keep = pool.tile([P, 80], F32, tag="keep")
            yap16 = pool.tile([P, 240], F16, tag="yap16")
            lossp = pool.tile([P, 8], F32, tag="lossp")
            nc.vector.memset(xpad[:], 0.0)
            nc.vector.memset(yap16[:], 0.0)
            nc.vector.memset(lossp[:], 0.0)
            nc.sync.dma_start(xpad[:, 79:159], d_x[:])
            nc.sync.dma_start(yres[:], d_y[:])
            nc.vector.tensor_scalar(keep[:], yres[:], 0.0, None, Op.not_equal)

            sqx = pool.tile([P, 239], F32, tag="sqx")
            nc.vector.memset(sqx[:, 0:1], 0.0)
            cs = pool.tile([P, 239], F32, tag="cs")
            nsq = pool.tile([P, S1], F32, tag="nsq")
            rnsq = pool.tile([P, S1], F32, tag="rnsq")
            w2 = pool.tile([P, S1 * 80], F16, tag="w2")
            dot16 = pool.tile([P, S1], F16, tag="dot16")
            adot = pool.tile([P, S1], F16, tag="adot")
            gsel = pool.tile([P, S1], F32, tag="gsel")
            mx8 = pool.tile([P, 8], F32, tag="mx8")
            mi8 = pool.tile([P, 8], U32, tag="mi8")
            thf = pool.tile([P, 1], F32, tag="thf")
            th2 = pool.tile([P, 1], F32, tag="th2")
            sf = pool.tile([P, 1], F32, tag="sf")
            df = pool.tile([P, 1], F32, tag="df")
            ixf = pool.tile([P, 256], F32, tag="ixf")
            ix1 = pool.tile([P, 256], I16, tag="ix1")
            ix2 = pool.tile([P, 256], I16, tag="ix2")
            ix3 = pool.tile([P, 256], I16, tag="ix3")
            ix4 = pool.tile([P, 256], I16, tag="ix4")
            yal = pool.tile([P, 256], F16, tag="yal")
            xele = pool.tile([P, 256], F16, tag="xele")
            yhat = pool.tile([P, 256], F16, tag="yhat")
            yele = pool.tile([P, 160], F16, tag="yele")
            zt = pool.tile([P, 80], F32, tag="zt")
            et = pool.tile([P, 80], F32, tag="et")
            ssum = pool.tile([P, 1], F32, tag="ssum")
            rsum = pool.tile([P, 1], F32, tag="rsum")
            nzm = pool.tile([P, 1], F32, tag="nzm")
            zero1 = pool.tile([P, 1], F32, tag="zero1")
            nc.vector.memset(zero1[:], 0.0)
            zf16 = pool.tile([P, NFEAT], F16, tag="zf16")
            etail = pool.tile([81, 128], F16, tag="etail")
            nc.sync.dma_start(etail[80:81, :], d_on[:])
            yhT0 = pool.tile([128, 128], F16, tag="yhT0")
            yhT1 = pool.tile([32, 128], F16, tag="yhT1")
            hsT = pool.tile([128, 4 * 128], F16, tag="hsT")
            xeT0 = pool.tile([128, 128], F16, tag="xeT0")
            xeT1 = pool.tile([32, 128], F16, tag="xeT1")
            xext16 = pool.tile([P, 160], F16, tag="xext16")
            dtmp = pool.tile([P, 80], F32, tag="dtmp")
            dsq = pool.tile([P, 80], F32, tag="dsq")

            for it in range(THINK_ITER):
                # --- sliding norms ---
                nc.scalar.activation(sqx[:, 1:239], xpad[:], AF.Square)
                nc.vector.tensor_tensor_scan(cs[:], sqx[:],
                                             zero1[:].to_broadcast((P, 239)),
                                             0.0, Op.add, Op.bypass)
                nc.vector.tensor_tensor(nsq[:], cs[:, 80:239], cs[:, 0:159],
                                        Op.subtract)
                nc.vector.tensor_scalar_max(rnsq[:], nsq[:], 1e-30)
                nc.vector.reciprocal(rnsq[:], rnsq[:])
                # --- dot: broadcast product + strided reduce ---
                xp = xpad[:]
                in0 = bass.AP(xp.tensor, xp.offset,
                              [list(xp.ap[0]), [1, S1], [1, 80]])
                in1 = yres[:].unsqueeze(1).to_broadcast((P, S1, 80))
                wv = w2[:].rearrange("p (s c) -> p s c", c=80)
                nc.vector.tensor_tensor(wv, in0, in1, Op.mult)
                with nc.allow_low_precision("argmax-only dot"):
                    nc.vector.tensor_reduce(dot16[:], wv,
                                            mybir.AxisListType.X, Op.add)
                # --- theta = argmax dot*|dot|/nsq ---
                nc.scalar.activation(adot[:], dot16[:], AF.Abs)
                nc.vector.tensor_tensor(gsel[:], dot16[:], adot[:], Op.mult)
                nc.vector.tensor_tensor(gsel[:], gsel[:], rnsq[:], Op.mult)
                nc.vector.max(mx8[:], gsel[:])
                nc.vector.max_index(mi8[:], mx8[:], gsel[:])
                nc.vector.tensor_copy(thf[:], mi8[:, 0:1])
                # --- y_align: scatter xpad16[theta+j] -> yal[j] ---
                nc.scalar.copy(xpad16[:], xpad[:])
                nc.vector.scalar_tensor_tensor(ixf[:, 0:238], io_t[:, 0:238],
                                               thf[:, 0:1], io_t[:, 0:238],
                                               Op.subtract, Op.bypass)
                nc.vector.tensor_copy(ix1[:, 0:238], ixf[:, 0:238])
                nc.gpsimd.local_scatter(yal[:], xpad16[:], ix1[:, 0:238],
                                        channels=128, num_elems=256,
                                        num_idxs=238)
                # --- softmax attention -> y_att in yap16[:, 80:160] ---
                nc.vector.tensor_tensor(zt[:], yal[:, 0:80], yres[:], Op.mult)
                nc.vector.max(mx8[:], zt[:])
                nc.vector.tensor_scalar_mul(nzm[:], mx8[:, 0:1], -1.0 / TEMPER)
                nc.scalar.activation(et[:], zt[:], AF.Exp, bias=nzm[:, 0:1],
                                     scale=1.0 / TEMPER)
                nc.vector.tensor_reduce(ssum[:], et[:], mybir.AxisListType.X,
                                        Op.add)
                nc.vector.reciprocal(rsum[:], ssum[:])
                nc.vector.tensor_tensor(et[:], et[:], yal[:, 0:80], Op.mult)
                nc.vector.tensor_scalar_mul(yap16[:, 80:160], et[:],
                                            rsum[:, 0:1])
                # --- x_ele: scatter yap16[(159-theta)+j] -> xele[j] ---
                nc.vector.tensor_scalar(th2[:], thf[:], -1.0, 159.0,
                                        Op.mult, Op.add)
                nc.vector.scalar_tensor_tensor(ixf[:, 0:240], io_t[:, 0:240],
                                               th2[:, 0:1], io_t[:, 0:240],
                                               Op.subtract, Op.bypass)
                nc.vector.tensor_copy(ix2[:, 0:240], ixf[:, 0:240])
                nc.gpsimd.local_scatter(xele[:], yap16[:], ix2[:, 0:240],
                                        channels=128, num_elems=256,
                                        num_idxs=240)
                nc.vector.tensor_tensor(xpad[:, 79:159], xpad[:, 79:159],
                                        xele[:, 0:80], Op.subtract)
                # --- z features (fp16, packed 4096) ---
                foff = 0
                yb = yap16[:, 80:240]
                for d0, d1, im in ZBLOCKS:
                    nblk = (d1 - d0) * im
                    ov = bass.AP(zf16[:].tensor, zf16[:].offset + foff,
                                 [list(zf16[:].ap[0]), [im, d1 - d0], [1, im]])
                    b0 = bass.AP(yb.tensor, yb.offset,
                                 [list(yb.ap[0]), [0, d1 - d0], [1, im]])
                    b1 = bass.AP(yb.tensor, yb.offset + d0,
                                 [list(yb.ap[0]), [1, d1 - d0], [1, im]])
                    nc.vector.tensor_tensor(ov, b0, b1, Op.mult)
                    foff += nblk
                # --- E accumulation: transpose chunk (matmul w/ id) ->
                #     copy to SBUF -> accumulate token-major in PSUM ---
                Eps = ppe.tile([128, 81], F32, tag="Eps")
                zsb = [None] * NCHUNK
                for k in range(NCHUNK + 2):
                    if k < NCHUNK:
                        zTp = pp.tile([128, 128], F32, tag="zTp")
                        nc.tensor.matmul(zTp[:],
                                         zf16[:, k * 128:(k + 1) * 128],
                                         id_t[:], start=True, stop=True)
                        zsb_k = zpool.tile([128, 128], F16, tag="zT")
                        zsb[k] = zsb_k
                        nc.scalar.copy(zsb[k][:], zTp[:])
                    j = k - 2
                    if 0 <= j < NCHUNK:
                        nc.tensor.matmul(Eps[:], zsb[j][:],
                                         A_t[:, j * 81:(j + 1) * 81],
                                         start=(j == 0), stop=False)
                # tail: feats [yaT(80); ones]
                yaTp = pp.tile([128, 128], F32, tag="zTp")
                nc.tensor.matmul(yaTp[0:80, :], yap16[:, 80:160], id_t[:],
                                 start=True, stop=True)
                nc.scalar.copy(etail[0:80, :], yaTp[0:80, :])
                nc.tensor.matmul(Eps[:], etail[:], At_t[:], start=False,
                                 stop=True)
                # --- s* argmax directly on PSUM, d* = 80 - s* ---
                nc.vector.max(mx8[:], Eps[:])
                nc.vector.max_index(mi8[:], mx8[:], Eps[:])
                nc.vector.tensor_copy(sf[:], mi8[:, 0:1])
                nc.vector.tensor_scalar(df[:], sf[:], -1.0, 80.0,
                                        Op.mult, Op.add)
                # --- yhat: scatter yap16[s*+j] -> yhat[j] (160 wide) ---
                nc.vector.scalar_tensor_tensor(ixf[:, 0:240], io_t[:, 0:240],
                                               sf[:, 0:1], io_t[:, 0:240],
                                               Op.subtract, Op.bypass)
                nc.vector.tensor_copy(ix3[:, 0:240], ixf[:, 0:240])
                nc.gpsimd.local_scatter(yhat[:], yap16[:], ix3[:, 0:240],
                                        channels=128, num_elems=256,
                                        num_idxs=240)
                # --- h_selT = W_enc @ yhat^T (+ b_enc) ---
                yhTp = pp.tile([128, 128], F32, tag="zTp")
                nc.tensor.matmul(yhTp[:], yhat[:, 0:128], id_t[:],
                                 start=True, stop=True)
                nc.scalar.copy(yhT0[:], yhTp[:])
                yhTp2 = pp.tile([128, 128], F32, tag="zTp")
                nc.tensor.matmul(yhTp2[0:32, :], yhat[:, 128:160], id_t[:],
                                 start=True, stop=True)
                nc.scalar.copy(yhT1[:], yhTp2[0:32, :])
                for hc in range(4):
                    Hp = pp.tile([128, 128], F32, tag="Hp")
                    nc.tensor.matmul(Hp[:], We_t[:, hc * 128:(hc + 1) * 128],
                                     yhT0[:], start=True, stop=False)
                    nc.tensor.matmul(Hp[:],
                                     We_t[0:32, 512 + hc * 128:512 + (hc + 1) * 128],
                                     yhT1[:], start=False, stop=True)
                    nc.scalar.activation(hsT[:, hc * 128:(hc + 1) * 128],
                                         Hp[:], AF.Identity,
                                         bias=be_t[:, hc:hc + 1])
                # --- x_extT = W_src @ h_selT (+ b_src) ---
                for oc in range(2):
                    ow = 128 if oc == 0 else 32
                    Xp = pp.tile([128, 128], F32, tag="Hp")
                    for hc in range(4):
                        nc.tensor.matmul(
                            Xp[0:ow, :],
                            Ws_t[:, hc * 160 + oc * 128: hc * 160 + oc * 128 + ow],
                            hsT[:, hc * 128:(hc + 1) * 128],
                            start=(hc == 0), stop=(hc == 3))
                    dst = xeT0 if oc == 0 else xeT1
                    nc.scalar.activation(dst[:], Xp[0:ow, :], AF.Identity,
                                         bias=bs_t[0:ow, oc:oc + 1])
                Xtp = pp.tile([128, 128], F32, tag="Hp")
                nc.tensor.matmul(Xtp[:], xeT0[:], id_t[:], start=True,
                                 stop=True)
                nc.scalar.copy(xext16[:, 0:128], Xtp[:])
                Xtp2 = pp.tile([128, 128], F32, tag="Hp")
                nc.tensor.matmul(Xtp2[:, 0:32], xeT1[:], id_t[0:32, 0:32],
                                 start=True, stop=True)
                nc.scalar.copy(xext16[:, 128:160], Xtp2[:, 0:32])
                # --- y_ele: scatter xext16[d*+j] -> yele[j] ---
                nc.vector.scalar_tensor_tensor(ixf[:, 0:160], io_t[:, 0:160],
                                               df[:, 0:1], io_t[:, 0:160],
                                               Op.subtract, Op.bypass)
                nc.vector.tensor_copy(ix4[:, 0:160], ixf[:, 0:160])
                nc.gpsimd.local_scatter(yele[:], xext16[:], ix4[:, 0:160],
                                        channels=128, num_elems=160,
                                        num_idxs=160)
                # --- loss partial + state updates ---
                nc.vector.tensor_tensor(dtmp[:], yele[:, 0:80], yres[:],
                                        Op.subtract)
                nc.vector.tensor_tensor(dtmp[:], dtmp[:], keep[:], Op.mult)
                nc.scalar.activation(dsq[:], dtmp[:], AF.Square,
                                     accum_out=lossp[:, it:it + 1])
                nc.vector.tensor_tensor(yres[:], yres[:], yele[:, 0:80],
                                        Op.subtract)

            nc.sync.dma_start(d_out[:], lossp[:])
    return nc


def kernel(x, y, W_enc, b_enc, W_src, b_src):
    import sys
    if '/opt/trn_rl_repo' not in sys.path:
        sys.path.insert(0, '/opt/trn_rl_repo')
    x = np.asarray(x, np.float32)
    y = np.asarray(y, np.float32)
    consts = _build_consts(W_enc, b_enc, W_src, b_src)

    if "nc" not in _cache:
        _cache["nc"] = _build_nc()
        _cache["nc"].finalize()
    nc = _cache["nc"]

    xt = x.reshape(NTOK, IDIM)
    yt = y.reshape(NTOK, ODIM)
    in_maps = []
    for c in range(NCORES):
        m = dict(consts)
        m["x"] = np.ascontiguousarray(xt[c * P:(c + 1) * P])
        m["y"] = np.ascontiguousarray(yt[c * P:(c + 1) * P])
        in_maps.append(m)

    from concourse.bass_utils import run_bass_kernel_spmd
    res = run_bass_kernel_spmd(nc, in_maps, list(range(NCORES)))
    parts = np.stack([r["losspart"] for r in res.results])
    keep_cnt = max(int((y != 0.0).sum()), 1)
    nums = parts[:, :, :THINK_ITER].sum(axis=(0, 1), dtype=np.float64)
    losses = (nums / keep_cnt).astype(np.float32)
    return np.float32(np.mean(losses))


# revision 9
# speedup vs baseline: 5.4215x; 1.2271x over previous
"""Trainium2 Bass kernel for nn_Net_17532056502451.

5 "think" iterations: shift-window cosine selector (159 shifts) + softmax
attention + scatter-back + conv-style encoder/decoder with energy argmax
(81 shifts), masked-MSE losses averaged.  Data-parallel: 1024 tokens over
8 cores, 128 tokens/core (one per SBUF partition), token-major.

v2 design notes (perf rework of the working v1 baseline):
- dot correlation: ONE broadcast-product DVE op (fp16 out) + ONE strided
  tensor_reduce (fp16 2x mode) instead of 80 scalar_tensor_tensor MACs.
- energy Gram features packed 6400 -> 4096 (d<32: i<80, d<48: i<48,
  d<64: i<32, d<80: i<16) = 32 chunks of 128 instead of 50.
- whole PE path in fp16: 1-pass matmuls + FWL instead of 4-pass fp32.
  Transposes via regular matmul against a fp16 identity.
- E-matmul operands swapped (stationary = zT chunk, moving = Az chunk) so
  E accumulates TOKEN-major in PSUM: argmax reads PSUM directly, no
  transpose-back.
- per-token dynamic window gathers via gpsimd local_scatter (per-partition
  indices: idx[p,j] = j - start_p, negatives ignored) -- replaces the v1
  16-lane ap_gather + 16 copy_predicated extraction entirely.
- loss: (d*keep) then ACT Square with accum_out (one op for square+sum).
"""
import numpy as np

IDIM = 80
ODIM = 80
HDIM = 512
THINK_ITER = 5
TEMPER = 0.7
B, T = 4, 256
NTOK = B * T
P = 128
NCORES = 8
S1 = 159
S2 = 81
# packed z-feature blocks: (d0, d1, imax)
ZBLOCKS = [(0, 32, 80), (32, 48, 48), (48, 64, 32), (64, 80, 16)]
NFEAT = sum((d1 - d0) * im for d0, d1, im in ZBLOCKS)   # 4096
NCHUNK = NFEAT // 128   # 32

_cache = {}


def _feat_list():
    feats = []
    for d0, d1, im in ZBLOCKS:
        for d in range(d0, d1):
            for i in range(im):
                feats.append((d, i))
    return feats


def _build_consts(W_enc, b_enc, W_src, b_src):
    W_enc = np.asarray(W_enc, np.float32)
    b_enc = np.asarray(b_enc, np.float32)
    W_src = np.asarray(W_src, np.float32)
    b_src = np.asarray(b_src, np.float32)
    C = (W_enc.T @ W_enc).astype(np.float32)
    q = (W_enc.T @ b_enc).astype(np.float32)
    bb = np.float32(b_enc @ b_enc)
    # E[t,s] = sum_f Az[s, f] * z[t, f] + sum_i 2 q[dd+i] ya_i + bb, dd = 80-s
    # z[t, f] = ya_i * ya_{i+d} for f -> (d, i) per ZBLOCKS packing.
    feats = _feat_list()
    Az = np.zeros((S2, NFEAT), np.float32)
    Al = np.zeros((S2, 81), np.float32)
    for s in range(S2):
        dd = 80 - s
        for f, (d, i) in enumerate(feats):
            if i < 80 - d:
                Az[s, f] = (2.0 if d > 0 else 1.0) * C[dd + i, dd + i + d]
        Al[s, :80] = 2.0 * q[dd:dd + 80]
        Al[s, 80] = bb
    Az_cat = np.ascontiguousarray(Az.T).astype(np.float16)   # (4096, 81)
    Atail = np.ascontiguousarray(Al.T).astype(np.float16)    # (81, 81)
    W_encT = np.ascontiguousarray(W_enc.T).astype(np.float16)  # (160, 512)
    W_srcT = np.ascontiguousarray(W_src.T).astype(np.float16)  # (512, 160)
    iota256 = np.broadcast_to(np.arange(256, dtype=np.float32), (P, 256)).copy()
    ident = np.eye(128, dtype=np.float16)
    benc4 = b_enc.reshape(4, 128).T.copy()            # (128, 4)
    bsrc2 = np.zeros((128, 2), np.float32)
    bsrc2[:, 0] = b_src[0:128]
    bsrc2[0:32, 1] = b_src[128:160]
    return dict(Az=Az_cat, Atail=Atail, WencT=W_encT, WsrcT=W_srcT,
                benc=benc4, bsrc=bsrc2, iota256=iota256,
                ident=ident, ones1=np.ones((1, 128), np.float16))


def _build_nc():
    import concourse.bass as bass
    import concourse.bacc as bacc
    import concourse.mybir as mybir
    from concourse.tile import TileContext

    F32 = mybir.dt.float32
    F16 = mybir.dt.float16
    I16 = mybir.dt.int16
    U32 = mybir.dt.uint32
    Op = mybir.AluOpType
    AF = mybir.ActivationFunctionType

    nc = bacc.Bacc()
    d_x = nc.declare_dram_parameter("x", [P, 80], F32, isOutput=False)
    d_y = nc.declare_dram_parameter("y", [P, 80], F32, isOutput=False)
    d_A = nc.declare_dram_parameter("Az", [NFEAT, 81], F16, isOutput=False)
    d_At = nc.declare_dram_parameter("Atail", [81, 81], F16, isOutput=False)
    d_We = nc.declare_dram_parameter("WencT", [160, 512], F16, isOutput=False)
    d_Ws = nc.declare_dram_parameter("WsrcT", [512, 160], F16, isOutput=False)
    d_be = nc.declare_dram_parameter("benc", [128, 4], F32, isOutput=False)
    d_bs = nc.declare_dram_parameter("bsrc", [128, 2], F32, isOutput=False)
    d_io = nc.declare_dram_parameter("iota256", [P, 256], F32, isOutput=False)
    d_id = nc.declare_dram_parameter("ident", [128, 128], F16, isOutput=False)
    d_on = nc.declare_dram_parameter("ones1", [1, 128], F16, isOutput=False)
    d_out = nc.declare_dram_parameter("losspart", [P, 8], F32, isOutput=True)

    with TileContext(nc) as tc:
        with (
            tc.tile_pool(name="const", bufs=1) as cpool,
            tc.tile_pool(name="work", bufs=1) as pool,
            tc.tile_pool(name="zrot", bufs=6) as zpool,
            tc.tile_pool(name="ps_rot", bufs=4, space="PSUM") as pp,
            tc.tile_pool(name="ps_h", bufs=2, space="PSUM") as pph,
            tc.tile_pool(name="ps_acc", bufs=1, space="PSUM") as ppe,
            tc.tile_pool(name="ps_warm", bufs=1, space="PSUM") as ppw,
        ):
            # ---- constants ----
            A_t = cpool.tile([P, NCHUNK * 81], F16, tag="A")
            for k in range(NCHUNK):
                nc.sync.dma_start(A_t[:, k * 81:(k + 1) * 81],
                                  d_A[k * 128:(k + 1) * 128, :])
            At_t = cpool.tile([81, 81], F16, tag="At")
            nc.sync.dma_start(At_t[:], d_At[:])
            We_t = cpool.tile([P, 2 * 512], F16, tag="We")
            nc.sync.dma_start(We_t[:, 0:512], d_We[0:128, :])
            nc.sync.dma_start(We_t[0:32, 512:1024], d_We[128:160, :])
            Ws_t = cpool.tile([P, 4 * 160], F16, tag="Ws")
            for k in range(4):
                nc.sync.dma_start(Ws_t[:, k * 160:(k + 1) * 160],
                                  d_Ws[k * 128:(k + 1) * 128, :])
            be_t = cpool.tile([128, 4], F32, tag="be")
            nc.sync.dma_start(be_t[:], d_be[:])
            bs_t = cpool.tile([128, 2], F32, tag="bs")
            nc.sync.dma_start(bs_t[:], d_bs[:])
            io_t = cpool.tile([P, 256], F32, tag="io")
            nc.sync.dma_start(io_t[:], d_io[:])
            id_t = cpool.tile([128, 128], F16, tag="id")
            nc.sync.dma_start(id_t[:], d_id[:])

            # ---- state ----
            xpad = pool.tile([P, 238], F32, tag="xpad")
            xpad16 = pool.tile([P, 238], F16, tag="xpad16")
            yres = pool.tile([P, 80], F32, tag="yres")
            keep = pool.tile([P, 80], F32, tag="keep")
            yap16 = pool.tile([P, 240], F16, tag="yap16")
            lossp = pool.tile([P, 8], F32, tag="lossp")
            nc.vector.memset(xpad[:], 0.0)
            nc.vector.memset(yap16[:], 0.0)
            nc.vector.memset(lossp[:], 0.0)
            nc.sync.dma_start(xpad[:, 79:159], d_x[:])
            nc.sync.dma_start(yres[:], d_y[:])
            nc.vector.tensor_scalar(keep[:], yres[:], 0.0, None, Op.not_equal)

            sqx = pool.tile([P, 239], F32, tag="sqx")
            nc.vector.memset(sqx[:, 0:1], 0.0)
            cs = pool.tile([P, 239], F32, tag="cs")
            nsq = pool.tile([P, S1], F32, tag="nsq")
            rnsq = pool.tile([P, S1], F32, tag="rnsq")
            yres16 = pool.tile([P, 80], F16, tag="yres16")
            w2 = pool.tile([P, S1 * 80], F16, tag="w2")
            w4 = pool.tile([P, S1 * 40], F16, tag="w4")
            w5 = pool.tile([P, S1 * 20], F16, tag="w5")
            w6 = pool.tile([P, S1 * 10], F16, tag="w6")
            dot16 = pool.tile([P, S1], F16, tag="dot16")
            adot = pool.tile([P, S1], F16, tag="adot")
            gsel = pool.tile([P, S1], F32, tag="gsel")
            mx8 = pool.tile([P, 8], F32, tag="mx8")
            mi8 = pool.tile([P, 8], U32, tag="mi8")
            thf = pool.tile([P, 1], F32, tag="thf")
            th2 = pool.tile([P, 1], F32, tag="th2")
            sf = pool.tile([P, 1], F32, tag="sf")
            df = pool.tile([P, 1], F32, tag="df")
            ixf = pool.tile([P, 256], F32, tag="ixf")
            ix1 = pool.tile([P, 256], I16, tag="ix1")
            ix2 = pool.tile([P, 256], I16, tag="ix2")
            ix3 = pool.tile([P, 256], I16, tag="ix3")
            ix4 = pool.tile([P, 256], I16, tag="ix4")
            yal = pool.tile([P, 256], F16, tag="yal")
            xele = pool.tile([P, 256], F16, tag="xele")
            yhat = pool.tile([P, 256], F16, tag="yhat")
            yele = pool.tile([P, 160], F16, tag="yele")
            zt = pool.tile([P, 80], F32, tag="zt")
            et = pool.tile([P, 80], F32, tag="et")
            ssum = pool.tile([P, 1], F32, tag="ssum")
            rsum = pool.tile([P, 1], F32, tag="rsum")
            nzm = pool.tile([P, 1], F32, tag="nzm")
            zero1 = pool.tile([P, 1], F32, tag="zero1")
            nc.vector.memset(zero1[:], 0.0)
            zf16 = pool.tile([P, NFEAT], F16, tag="zf16")
            etail = pool.tile([81, 128], F16, tag="etail")
            nc.sync.dma_start(etail[80:81, :], d_on[:])
            yhT0 = pool.tile([128, 128], F16, tag="yhT0")
            yhT1 = pool.tile([32, 128], F16, tag="yhT1")
            hsT = pool.tile([128, 4 * 128], F16, tag="hsT")
            xeT0 = pool.tile([128, 128], F16, tag="xeT0")
            xeT1 = pool.tile([32, 128], F16, tag="xeT1")
            xext16 = pool.tile([P, 160], F16, tag="xext16")
            dtmp = pool.tile([P, 80], F32, tag="dtmp")
            dsq = pool.tile([P, 80], F32, tag="dsq")

            def vap(tile, free0, fdims):
                b = tile[:]
                return bass.AP(b.tensor, b.offset + free0,
                               [list(b.ap[0])] + list(fdims))

            for it in range(THINK_ITER):
                # --- sliding norms ---
                nc.scalar.activation(sqx[:, 1:239], xpad[:], AF.Square)
                nc.vector.tensor_tensor_scan(cs[:], sqx[:],
                                             zero1[:].to_broadcast((P, 239)),
                                             0.0, Op.add, Op.bypass)
                nc.vector.tensor_tensor(nsq[:], cs[:, 80:239], cs[:, 0:159],
                                        Op.subtract)
                nc.vector.tensor_scalar_max(rnsq[:], nsq[:], 1e-30)
                nc.vector.reciprocal_approx_fast(rnsq[:], rnsq[:])
                # --- dot: fp16 broadcast product (2x) + fold-adds (2x)
                #     + small strided reduce ---
                nc.scalar.copy(xpad16[:], xpad[:])
                nc.scalar.copy(yres16[:], yres[:])
                in0 = vap(xpad16, 0, [[1, S1], [1, 80]])
                in1 = yres16[:].unsqueeze(1).to_broadcast((P, S1, 80))
                wv = w2[:].rearrange("p (s c) -> p s c", c=80)
                nc.vector.tensor_tensor(wv, in0, in1, Op.mult)
                warm = ppw.tile([128, 128], F32, tag="warm")
                nc.tensor.matmul(warm[:], id_t[:], w2[:, 0:128],
                                 start=True, stop=True)
                with nc.allow_low_precision("argmax-only dot"):
                    nc.vector.tensor_tensor(
                        w4[:].rearrange("p (s c) -> p s c", c=40),
                        vap(w2, 0, [[80, S1], [1, 40]]),
                        vap(w2, 40, [[80, S1], [1, 40]]), Op.add)
                    nc.tensor.matmul(warm[:], id_t[:], w4[:, 0:128],
                                     start=True, stop=True)
                    nc.vector.tensor_tensor(
                        w5[:].rearrange("p (s c) -> p s c", c=20),
                        vap(w4, 0, [[40, S1], [1, 20]]),
                        vap(w4, 20, [[40, S1], [1, 20]]), Op.add)
                    nc.tensor.matmul(warm[:], id_t[:], w5[:, 0:128],
                                     start=True, stop=True)
                    nc.vector.tensor_tensor(
                        w6[:].rearrange("p (s c) -> p s c", c=10),
                        vap(w5, 0, [[20, S1], [1, 10]]),
                        vap(w5, 10, [[20, S1], [1, 10]]), Op.add)
                    nc.tensor.matmul(warm[:], id_t[:], w6[:, 0:128],
                                     start=True, stop=True)
                    nc.vector.tensor_reduce(dot16[:],
                                            vap(w6, 0, [[10, S1], [1, 10]]),
                                            mybir.AxisListType.X, Op.add)
                # --- theta = argmax dot*|dot|/nsq ---
                nc.scalar.activation(adot[:], dot16[:], AF.Abs)
                nc.vector.tensor_tensor(gsel[:], dot16[:], adot[:], Op.mult)
                nc.vector.tensor_tensor(gsel[:], gsel[:], rnsq[:], Op.mult)
                nc.vector.max(mx8[:], gsel[:])
                nc.vector.max_index(mi8[:], mx8[:], gsel[:])
                nc.vector.tensor_copy(thf[:], mi8[:, 0:1])
                # --- y_align: scatter xpad16[theta+j] -> yal[j] ---
                nc.vector.scalar_tensor_tensor(ixf[:, 0:238], io_t[:, 0:238],
                                               thf[:, 0:1], io_t[:, 0:238],
                                               Op.subtract, Op.bypass)
                nc.vector.tensor_copy(ix1[:, 0:238], ixf[:, 0:238])
                nc.gpsimd.local_scatter(yal[:], xpad16[:], ix1[:, 0:238],
                                        channels=128, num_elems=256,
                                        num_idxs=238)
                # --- softmax attention -> y_att in yap16[:, 80:160] ---
                nc.vector.tensor_tensor(zt[:], yal[:, 0:80], yres[:], Op.mult)
                nc.vector.max(mx8[:], zt[:])
                nc.vector.tensor_scalar_mul(nzm[:], mx8[:, 0:1], -1.0 / TEMPER)
                nc.scalar.activation(et[:], zt[:], AF.Exp, bias=nzm[:, 0:1],
                                     scale=1.0 / TEMPER)
                nc.vector.tensor_reduce(ssum[:], et[:], mybir.AxisListType.X,
                                        Op.add)
                nc.vector.reciprocal_approx_fast(rsum[:], ssum[:])
                nc.vector.tensor_tensor(et[:], et[:], yal[:, 0:80], Op.mult)
                nc.vector.tensor_scalar_mul(yap16[:, 80:160], et[:],
                                            rsum[:, 0:1])
                # --- x_ele: scatter yap16[(159-theta)+j] -> xele[j] ---
                nc.vector.tensor_scalar(th2[:], thf[:], -1.0, 159.0,
                                        Op.mult, Op.add)
                nc.vector.scalar_tensor_tensor(ixf[:, 0:240], io_t[:, 0:240],
                                               th2[:, 0:1], io_t[:, 0:240],
                                               Op.subtract, Op.bypass)
                nc.vector.tensor_copy(ix2[:, 0:240], ixf[:, 0:240])
                nc.gpsimd.local_scatter(xele[:], yap16[:], ix2[:, 0:240],
                                        channels=128, num_elems=256,
                                        num_idxs=240)
                nc.vector.tensor_tensor(xpad[:, 79:159], xpad[:, 79:159],
                                        xele[:, 0:80], Op.subtract)
                # --- z features (fp16, packed 4096) ---
                foff = 0
                yb = yap16[:, 80:240]
                for d0, d1, im in ZBLOCKS:
                    nblk = (d1 - d0) * im
                    ov = bass.AP(zf16[:].tensor, zf16[:].offset + foff,
                                 [list(zf16[:].ap[0]), [im, d1 - d0], [1, im]])
                    b0 = bass.AP(yb.tensor, yb.offset,
                                 [list(yb.ap[0]), [0, d1 - d0], [1, im]])
                    b1 = bass.AP(yb.tensor, yb.offset + d0,
                                 [list(yb.ap[0]), [1, d1 - d0], [1, im]])
                    nc.vector.tensor_tensor(ov, b0, b1, Op.mult)
                    foff += nblk
                # --- E accumulation: transpose chunk (matmul w/ id) ->
                #     copy to SBUF -> accumulate token-major in PSUM ---
                Eps = ppe.tile([128, 81], F32, tag="Eps")
                zsb = [None] * NCHUNK
                for k in range(NCHUNK + 2):
                    if k < NCHUNK:
                        zTp = pp.tile([128, 128], F32, tag="zTp")
                        nc.tensor.matmul(zTp[:],
                                         zf16[:, k * 128:(k + 1) * 128],
                                         id_t[:], start=True, stop=True)
                        zsb_k = zpool.tile([128, 128], F16, tag="zT")
                        zsb[k] = zsb_k
                        nc.scalar.copy(zsb[k][:], zTp[:])
                    j = k - 2
                    if 0 <= j < NCHUNK:
                        nc.tensor.matmul(Eps[:], zsb[j][:],
                                         A_t[:, j * 81:(j + 1) * 81],
                                         start=(j == 0), stop=False)
                # tail: feats [yaT(80); ones]
                yaTp = pp.tile([128, 128], F32, tag="zTp")
                nc.tensor.matmul(yaTp[0:80, :], yap16[:, 80:160], id_t[:],
                                 start=True, stop=True)
                nc.scalar.copy(etail[0:80, :], yaTp[0:80, :])
                nc.tensor.matmul(Eps[:], etail[:], At_t[:], start=False,
                                 stop=True)
                # --- s* argmax directly on PSUM, d* = 80 - s* ---
                nc.vector.max(mx8[:], Eps[:])
                nc.vector.max_index(mi8[:], mx8[:], Eps[:])
                nc.vector.tensor_copy(sf[:], mi8[:, 0:1])
                nc.vector.tensor_scalar(df[:], sf[:], -1.0, 80.0,
                                        Op.mult, Op.add)
                # --- yhat: scatter yap16[s*+j] -> yhat[j] (160 wide) ---
                nc.vector.scalar_tensor_tensor(ixf[:, 0:240], io_t[:, 0:240],
                                               sf[:, 0:1], io_t[:, 0:240],
                                               Op.subtract, Op.bypass)
                nc.vector.tensor_copy(ix3[:, 0:240], ixf[:, 0:240])
                nc.gpsimd.local_scatter(yhat[:], yap16[:], ix3[:, 0:240],
                                        channels=128, num_elems=256,
                                        num_idxs=240)
                # --- h_selT = W_enc @ yhat^T (+ b_enc) ---
                yhTp = pp.tile([128, 128], F32, tag="zTp")
                nc.tensor.matmul(yhTp[:], yhat[:, 0:128], id_t[:],
                                 start=True, stop=True)
                nc.scalar.copy(yhT0[:], yhTp[:])
                yhTp2 = pp.tile([128, 128], F32, tag="zTp")
                nc.tensor.matmul(yhTp2[0:32, :], yhat[:, 128:160], id_t[:],
                                 start=True, stop=True)
                nc.scalar.copy(yhT1[:], yhTp2[0:32, :])
                for hc in range(4):
                    Hp = pph.tile([128, 128], F32, tag="Hp")
                    nc.tensor.matmul(Hp[:], We_t[:, hc * 128:(hc + 1) * 128],
                                     yhT0[:], start=True, stop=False)
                    nc.tensor.matmul(Hp[:],
                                     We_t[0:32, 512 + hc * 128:512 + (hc + 1) * 128],
                                     yhT1[:], start=False, stop=True)
                    nc.scalar.activation(hsT[:, hc * 128:(hc + 1) * 128],
                                         Hp[:], AF.Identity,
                                         bias=be_t[:, hc:hc + 1])
                # --- x_extT = W_src @ h_selT (+ b_src) ---
                for oc in range(2):
                    ow = 128 if oc == 0 else 32
                    Xp = pph.tile([128, 128], F32, tag="Hp")
                    for hc in range(4):
                        nc.tensor.matmul(
                            Xp[0:ow, :],
                            Ws_t[:, hc * 160 + oc * 128: hc * 160 + oc * 128 + ow],
                            hsT[:, hc * 128:(hc + 1) * 128],
                            start=(hc == 0), stop=(hc == 3))
                    dst = xeT0 if oc == 0 else xeT1
                    nc.scalar.activation(dst[:], Xp[0:ow, :], AF.Identity,
                                         bias=bs_t[0:ow, oc:oc + 1])
                Xtp = pph.tile([128, 128], F32, tag="Hp")
                nc.tensor.matmul(Xtp[:], xeT0[:], id_t[:], start=True,
                                 stop=True)
                nc.scalar.copy(xext16[:, 0:128], Xtp[:])
                Xtp2 = pph.tile([128, 128], F32, tag="Hp")
                nc.tensor.matmul(Xtp2[:, 0:32], xeT1[:], id_t[0:32, 0:32],
                                 start=True, stop=True)
                nc.scalar.copy(xext16[:, 128:160], Xtp2[:, 0:32])
                # --- y_ele: scatter xext16[d*+j] -> yele[j] ---
                nc.vector.scalar_tensor_tensor(ixf[:, 0:160], io_t[:, 0:160],
                                               df[:, 0:1], io_t[:, 0:160],
                                               Op.subtract, Op.bypass)
                nc.vector.tensor_copy(ix4[:, 0:160], ixf[:, 0:160])
                nc.gpsimd.local_scatter(yele[:], xext16[:], ix4[:, 0:160],
                                        channels=128, num_elems=160,
                                        num_idxs=160)
                # --- loss partial + state updates ---
                nc.vector.tensor_tensor(dtmp[:], yele[:, 0:80], yres[:],
                                        Op.subtract)
                nc.vector.tensor_tensor(dtmp[:], dtmp[:], keep[:], Op.mult)
                nc.scalar.activation(dsq[:], dtmp[:], AF.Square,
                                     accum_out=lossp[:, it:it + 1])
                nc.vector.tensor_tensor(yres[:], yres[:], yele[:, 0:80],
                                        Op.subtract)

            nc.sync.dma_start(d_out[:], lossp[:])
    return nc


def kernel(x, y, W_enc, b_enc, W_src, b_src):
    import sys
    if '/opt/trn_rl_repo' not in sys.path:
        sys.path.insert(0, '/opt/trn_rl_repo')
    x = np.asarray(x, np.float32)
    y = np.asarray(y, np.float32)
    consts = _build_consts(W_enc, b_enc, W_src, b_src)

    if "nc" not in _cache:
        _cache["nc"] = _build_nc()
        _cache["nc"].finalize()
    nc = _cache["nc"]

    xt = x.reshape(NTOK, IDIM)
    yt = y.reshape(NTOK, ODIM)
    in_maps = []
    for c in range(NCORES):
        m = dict(consts)
        m["x"] = np.ascontiguousarray(xt[c * P:(c + 1) * P])
        m["y"] = np.ascontiguousarray(yt[c * P:(c + 1) * P])
        in_maps.append(m)

    from concourse.bass_utils import run_bass_kernel_spmd
    res = run_bass_kernel_spmd(nc, in_maps, list(range(NCORES)))
    parts = np.stack([r["losspart"] for r in res.results])
    keep_cnt = max(int((y != 0.0).sum()), 1)
    nums = parts[:, :, :THINK_ITER].sum(axis=(0, 1), dtype=np.float64)
    losses = (nums / keep_cnt).astype(np.float32)
    return np.float32(np.mean(losses))


# revision 11
# speedup vs baseline: 6.6115x; 1.2195x over previous
"""Trainium2 Bass kernel for nn_Net_17532056502451.

5 "think" iterations: shift-window cosine selector (159 shifts) + softmax
attention + scatter-back + conv-style encoder/decoder with energy argmax
(81 shifts), masked-MSE losses averaged.  Data-parallel: 1024 tokens over
8 cores, 128 tokens/core (one per SBUF partition), token-major.

v4 design notes:
- dot correlation: fp16 broadcast-product (DVE 2x mode) + 2x fold-adds
  (80->40->20->10) + one small strided reduce.
- energy Gram features packed 6400 -> 4096 = 32 chunks of 128; whole PE
  path fp16 (1-pass matmuls, FWL); transposes via matmul against fp16
  identity; E-matmul operands swapped so E accumulates TOKEN-major in
  PSUM (argmax reads PSUM directly).
- E-pipeline: chunk PAIRS share one [128,256] PSUM tile; PSUM->SBUF
  copies are 256 wide and alternate Vector/Scalar engines.
- per-token dynamic window gathers via gpsimd local_scatter with
  per-partition indices (idx[p,j] = j - start_p, negatives ignored);
  gathers read only the 80 nonzero source columns.
- all constants pre-swizzled on host into one fp16 blob + one fp32 blob
  (3 input DMAs total instead of ~46).
"""
import numpy as np

IDIM = 80
ODIM = 80
HDIM = 512
THINK_ITER = 5
TEMPER = 0.7
B, T = 4, 256
NTOK = B * T
P = 128
NCORES = 8
S1 = 159
S2 = 81
ZBLOCKS = [(0, 32, 80), (32, 48, 48), (48, 64, 32), (64, 80, 16)]
NFEAT = sum((d1 - d0) * im for d0, d1, im in ZBLOCKS)   # 4096
NCHUNK = NFEAT // 128   # 32
NPAIR = NCHUNK // 2
# fp16 const blob column offsets
OF_A = 0
OF_AT = OF_A + NCHUNK * 81          # 2592
OF_WE = OF_AT + 81                  # 2673
OF_WS = OF_WE + 1024                # 3697
OF_ID = OF_WS + 640                 # 4337
OF_ON = OF_ID + 128                 # 4465
W16 = OF_ON + 128                   # 4593
# fp32 const blob: benc(4) bsrc(2) iota(256)
OF_BE = 0
OF_BS = 4
OF_IO = 6
W32 = 262

_cache = {}


def _feat_list():
    feats = []
    for d0, d1, im in ZBLOCKS:
        for d in range(d0, d1):
            for i in range(im):
                feats.append((d, i))
    return feats


def _build_consts(W_enc, b_enc, W_src, b_src):
    W_enc = np.asarray(W_enc, np.float32)
    b_enc = np.asarray(b_enc, np.float32)
    W_src = np.asarray(W_src, np.float32)
    b_src = np.asarray(b_src, np.float32)
    C = (W_enc.T @ W_enc).astype(np.float32)
    q = (W_enc.T @ b_enc).astype(np.float32)
    bb = np.float32(b_enc @ b_enc)
    feats = _feat_list()
    Az = np.zeros((S2, NFEAT), np.float32)
    Al = np.zeros((S2, 81), np.float32)
    for s in range(S2):
        dd = 80 - s
        for f, (d, i) in enumerate(feats):
            if i < 80 - d:
                Az[s, f] = (2.0 if d > 0 else 1.0) * C[dd + i, dd + i + d]
        Al[s, :80] = 2.0 * q[dd:dd + 80]
        Al[s, 80] = bb
    c16 = np.zeros((P, W16), np.float16)
    # A: chunk k at cols OF_A + k*81, partition p holds Az.T[k*128+p, :]
    AzT = np.ascontiguousarray(Az.T).astype(np.float16)          # (4096, 81)
    c16[:, OF_A:OF_AT] = AzT.reshape(NCHUNK, 128, 81).transpose(1, 0, 2) \
                            .reshape(128, NCHUNK * 81)
    c16[0:81, OF_AT:OF_AT + 81] = np.ascontiguousarray(Al.T).astype(np.float16)
    WeT = np.ascontiguousarray(W_enc.T).astype(np.float16)       # (160, 512)
    c16[:, OF_WE:OF_WE + 512] = WeT[0:128]
    c16[0:32, OF_WE + 512:OF_WE + 1024] = WeT[128:160]
    WsT = np.ascontiguousarray(W_src.T).astype(np.float16)       # (512, 160)
    c16[:, OF_WS:OF_WS + 640] = WsT.reshape(4, 128, 160).transpose(1, 0, 2) \
                                   .reshape(128, 640)
    c16[:, OF_ID:OF_ID + 128] = np.eye(128, dtype=np.float16)
    c16[:, OF_ON:OF_ON + 128] = 1.0
    c32 = np.zeros((P, W32), np.float32)
    c32[:, OF_BE:OF_BE + 4] = b_enc.reshape(4, 128).T
    c32[:, OF_BS] = b_src[0:128]
    c32[0:32, OF_BS + 1] = b_src[128:160]
    c32[:, OF_IO:OF_IO + 256] = np.arange(256, dtype=np.float32)
    return dict(c16=c16, c32=c32)


def _make_in_maps(x, y, consts):
    xt = x.reshape(NTOK, IDIM)
    yt = y.reshape(NTOK, ODIM)
    in_maps = []
    for c in range(NCORES):
        m = dict(consts)
        m["xy"] = np.ascontiguousarray(
            np.concatenate([xt[c * P:(c + 1) * P], yt[c * P:(c + 1) * P]],
                           axis=1))
        in_maps.append(m)
    return in_maps


def _build_nc():
    import concourse.bass as bass
    import concourse.bacc as bacc
    import concourse.mybir as mybir
    from concourse.tile import TileContext

    F32 = mybir.dt.float32
    F16 = mybir.dt.float16
    I16 = mybir.dt.int16
    U32 = mybir.dt.uint32
    Op = mybir.AluOpType
    AF = mybir.ActivationFunctionType

    nc = bacc.Bacc()
    d_xy = nc.declare_dram_parameter("xy", [P, 160], F32, isOutput=False)
    d_c16 = nc.declare_dram_parameter("c16", [P, W16], F16, isOutput=False)
    d_c32 = nc.declare_dram_parameter("c32", [P, W32], F32, isOutput=False)
    d_out = nc.declare_dram_parameter("losspart", [P, 8], F32, isOutput=True)

    with TileContext(nc) as tc:
        with (
            tc.tile_pool(name="const", bufs=1) as cpool,
            tc.tile_pool(name="work", bufs=1) as pool,
            tc.tile_pool(name="zrot", bufs=6) as zpool,
            tc.tile_pool(name="ps_rot", bufs=3, space="PSUM") as pp,
            tc.tile_pool(name="ps_h", bufs=3, space="PSUM") as pph,
            tc.tile_pool(name="ps_acc", bufs=1, space="PSUM") as ppe,
        ):
            # ---- inputs + constants (3 DMAs) ----
            xy_t = pool.tile([P, 160], F32, tag="xy")
            nc.sync.dma_start(xy_t[:], d_xy[:])
            c16 = cpool.tile([P, W16], F16, tag="c16")
            nc.sync.dma_start(c16[:], d_c16[:])
            c32 = cpool.tile([P, W32], F32, tag="c32")
            nc.sync.dma_start(c32[:], d_c32[:])

            def Achunk(k):
                return c16[:, OF_A + k * 81:OF_A + (k + 1) * 81]
            At_t = c16[0:81, OF_AT:OF_AT + 81]
            We_t = c16[:, OF_WE:OF_WE + 1024]
            Ws_t = c16[:, OF_WS:OF_WS + 640]
            id_t = c16[:, OF_ID:OF_ID + 128]
            be_t = c32[:, OF_BE:OF_BE + 4]
            bs_t = c32[:, OF_BS:OF_BS + 2]
            io_t = c32[:, OF_IO:OF_IO + 256]

            # ---- state ----
            xpad = pool.tile([P, 238], F32, tag="xpad")
            xpad16 = pool.tile([P, 238], F16, tag="xpad16")
            yres = pool.tile([P, 80], F32, tag="yres")
            keep = pool.tile([P, 80], F32, tag="keep")
            yap16 = pool.tile([P, 240], F16, tag="yap16")
            lossp = pool.tile([P, 8], F32, tag="lossp")
            nc.vector.memset(xpad[:], 0.0)
            nc.vector.memset(yap16[:], 0.0)
            nc.vector.memset(lossp[:], 0.0)
            nc.scalar.copy(xpad[:, 79:159], xy_t[:, 0:80])
            nc.vector.tensor_copy(yres[:], xy_t[:, 80:160])
            nc.vector.tensor_scalar(keep[:], yres[:], 0.0, None, Op.not_equal)

            sqx = pool.tile([P, 239], F32, tag="sqx")
            nc.vector.memset(sqx[:, 0:1], 0.0)
            cs = pool.tile([P, 239], F32, tag="cs")
            nsq = pool.tile([P, S1], F32, tag="nsq")
            rnsq = pool.tile([P, S1], F32, tag="rnsq")
            yres16 = pool.tile([P, 80], F16, tag="yres16")
            w2 = pool.tile([P, S1 * 80], F16, tag="w2")
            w4 = pool.tile([P, S1 * 40], F16, tag="w4")
            w5 = pool.tile([P, S1 * 20], F16, tag="w5")
            w6 = pool.tile([P, S1 * 10], F16, tag="w6")
            dot16 = pool.tile([P, S1], F16, tag="dot16")
            adot = pool.tile([P, S1], F16, tag="adot")
            gsel = pool.tile([P, S1], F32, tag="gsel")
            mx8 = pool.tile([P, 8], F32, tag="mx8")
            mi8 = pool.tile([P, 8], U32, tag="mi8")
            thf = pool.tile([P, 1], F32, tag="thf")
            th2 = pool.tile([P, 1], F32, tag="th2")
            sf = pool.tile([P, 1], F32, tag="sf")
            df = pool.tile([P, 1], F32, tag="df")
            ixf = pool.tile([P, 160], F32, tag="ixf")
            ix1 = pool.tile([P, 80], I16, tag="ix1")
            ix2 = pool.tile([P, 80], I16, tag="ix2")
            ix3 = pool.tile([P, 80], I16, tag="ix3")
            ix4 = pool.tile([P, 160], I16, tag="ix4")
            yal = pool.tile([P, 256], F16, tag="yal")
            xele = pool.tile([P, 256], F16, tag="xele")
            yhat = pool.tile([P, 256], F16, tag="yhat")
            yele = pool.tile([P, 160], F16, tag="yele")
            zt = pool.tile([P, 80], F32, tag="zt")
            et = pool.tile([P, 80], F32, tag="et")
            ssum = pool.tile([P, 1], F32, tag="ssum")
            rsum = pool.tile([P, 1], F32, tag="rsum")
            nzm = pool.tile([P, 1], F32, tag="nzm")
            zero1 = pool.tile([P, 1], F32, tag="zero1")
            nc.vector.memset(zero1[:], 0.0)
            zf16 = pool.tile([P, NFEAT], F16, tag="zf16")
            etail = pool.tile([81, 128], F16, tag="etail")
            nc.sync.dma_start(etail[80:81, :], d_c16[80:81, OF_ON:OF_ON + 128])
            yhT0 = pool.tile([128, 128], F16, tag="yhT0")
            yhT1 = pool.tile([32, 128], F16, tag="yhT1")
            hsT = pool.tile([128, 4 * 128], F16, tag="hsT")
            xeT0 = pool.tile([128, 128], F16, tag="xeT0")
            xeT1 = pool.tile([32, 128], F16, tag="xeT1")
            xext16 = pool.tile([P, 160], F16, tag="xext16")
            dtmp = pool.tile([P, 80], F32, tag="dtmp")
            dsq = pool.tile([P, 80], F32, tag="dsq")

            def vap(tile_ap, free0, fdims):
                b = tile_ap
                return bass.AP(b.tensor, b.offset + free0,
                               [list(b.ap[0])] + list(fdims))

            for it in range(THINK_ITER):
                # --- sliding norms ---
                nc.scalar.activation(sqx[:, 1:239], xpad[:], AF.Square)
                nc.vector.tensor_tensor_scan(cs[:], sqx[:],
                                             zero1[:].to_broadcast((P, 239)),
                                             0.0, Op.add, Op.bypass)
                nc.vector.tensor_tensor(nsq[:], cs[:, 80:239], cs[:, 0:159],
                                        Op.subtract)
                nc.vector.tensor_scalar_max(rnsq[:], nsq[:], 1e-30)
                nc.vector.reciprocal_approx_fast(rnsq[:], rnsq[:])
                # --- dot: fp16 product (2x) + fold-adds + small reduce ---
                nc.scalar.copy(xpad16[:], xpad[:])
                nc.scalar.copy(yres16[:], yres[:])
                in0 = vap(xpad16[:], 0, [[1, S1], [1, 80]])
                in1 = yres16[:].unsqueeze(1).to_broadcast((P, S1, 80))
                wv = w2[:].rearrange("p (s c) -> p s c", c=80)
                nc.vector.tensor_tensor(wv, in0, in1, Op.mult)
                with nc.allow_low_precision("argmax-only dot"):
                    nc.vector.tensor_tensor(
                        w4[:].rearrange("p (s c) -> p s c", c=40),
                        vap(w2[:], 0, [[80, S1], [1, 40]]),
                        vap(w2[:], 40, [[80, S1], [1, 40]]), Op.add)
                    nc.vector.tensor_tensor(
                        w5[:].rearrange("p (s c) -> p s c", c=20),
                        vap(w4[:], 0, [[40, S1], [1, 20]]),
                        vap(w4[:], 20, [[40, S1], [1, 20]]), Op.add)
                    nc.vector.tensor_tensor(
                        w6[:].rearrange("p (s c) -> p s c", c=10),
                        vap(w5[:], 0, [[20, S1], [1, 10]]),
                        vap(w5[:], 10, [[20, S1], [1, 10]]), Op.add)
                    nc.vector.tensor_reduce(dot16[:],
                                            vap(w6[:], 0, [[10, S1], [1, 10]]),
                                            mybir.AxisListType.X, Op.add)
                # --- theta = argmax dot*|dot|/nsq ---
                nc.scalar.activation(adot[:], dot16[:], AF.Abs)
                nc.vector.tensor_tensor(gsel[:], dot16[:], adot[:], Op.mult)
                nc.vector.tensor_tensor(gsel[:], gsel[:], rnsq[:], Op.mult)
                nc.vector.max(mx8[:], gsel[:])
                nc.vector.max_index(mi8[:], mx8[:], gsel[:])
                nc.vector.tensor_copy(thf[:], mi8[:, 0:1])
                # --- y_align: scatter xpad16[79+j] -> yal[79+j-theta] ---
                nc.vector.scalar_tensor_tensor(ixf[:, 0:80], io_t[:, 79:159],
                                               thf[:, 0:1], io_t[:, 79:159],
                                               Op.subtract, Op.bypass)
                nc.vector.tensor_copy(ix1[:], ixf[:, 0:80])
                nc.gpsimd.local_scatter(yal[:], xpad16[:, 79:159], ix1[:],
                                        channels=128, num_elems=256,
                                        num_idxs=80)
                # --- softmax attention -> y_att in yap16[:, 80:160] ---
                nc.vector.tensor_tensor(zt[:], yal[:, 0:80], yres[:], Op.mult)
                nc.vector.max(mx8[:], zt[:])
                nc.vector.tensor_scalar_mul(nzm[:], mx8[:, 0:1], -1.0 / TEMPER)
                nc.scalar.activation(et[:], zt[:], AF.Exp, bias=nzm[:, 0:1],
                                     scale=1.0 / TEMPER)
                nc.vector.tensor_reduce(ssum[:], et[:], mybir.AxisListType.X,
                                        Op.add)
                nc.vector.reciprocal_approx_fast(rsum[:], ssum[:])
                nc.vector.tensor_tensor(et[:], et[:], yal[:, 0:80], Op.mult)
                nc.vector.tensor_scalar_mul(yap16[:, 80:160], et[:],
                                            rsum[:, 0:1])
                # --- x_ele: scatter yap16[80+j] -> xele[j+theta-79] ---
                nc.vector.tensor_scalar(th2[:], thf[:], -1.0, 159.0,
                                        Op.mult, Op.add)
                nc.vector.scalar_tensor_tensor(ixf[:, 0:80], io_t[:, 80:160],
                                               th2[:, 0:1], io_t[:, 80:160],
                                               Op.subtract, Op.bypass)
                nc.vector.tensor_copy(ix2[:], ixf[:, 0:80])
                nc.gpsimd.local_scatter(xele[:], yap16[:, 80:160], ix2[:],
                                        channels=128, num_elems=256,
                                        num_idxs=80)
                nc.vector.tensor_tensor(xpad[:, 79:159], xpad[:, 79:159],
                                        xele[:, 0:80], Op.subtract)
                # --- z features (fp16, packed 4096) ---
                foff = 0
                yb = yap16[:, 80:240]
                for d0, d1, im in ZBLOCKS:
                    nblk = (d1 - d0) * im
                    ov = bass.AP(zf16[:].tensor, zf16[:].offset + foff,
                                 [list(zf16[:].ap[0]), [im, d1 - d0], [1, im]])
                    b0 = bass.AP(yb.tensor, yb.offset,
                                 [list(yb.ap[0]), [0, d1 - d0], [1, im]])
                    b1 = bass.AP(yb.tensor, yb.offset + d0,
                                 [list(yb.ap[0]), [1, d1 - d0], [1, im]])
                    nc.vector.tensor_tensor(ov, b0, b1, Op.mult)
                    foff += nblk
                # --- E: transpose chunk-pairs -> one 256-wide copy (V/ACT
                #     alternating) -> token-major accumulate in PSUM ---
                Eps = ppe.tile([128, 81], F32, tag="Eps")
                zsb = [None] * NPAIR
                for k2 in range(NPAIR + 1):
                    if k2 < NPAIR:
                        zTp = pp.tile([128, 256], F32, tag="zTp")
                        nc.tensor.matmul(zTp[:, 0:128],
                                         zf16[:, (2 * k2) * 128:(2 * k2 + 1) * 128],
                                         id_t, start=True, stop=True)
                        nc.tensor.matmul(zTp[:, 128:256],
                                         zf16[:, (2 * k2 + 1) * 128:(2 * k2 + 2) * 128],
                                         id_t, start=True, stop=True)
                        zsb_k = zpool.tile([128, 256], F16, tag="zT")
                        zsb[k2] = zsb_k
                        if k2 % 2 == 0:
                            nc.scalar.copy(zsb[k2][:], zTp[:])
                        else:
                            nc.vector.tensor_copy(zsb[k2][:], zTp[:])
                    j2 = k2 - 1
                    if 0 <= j2 < NPAIR:
                        nc.tensor.matmul(Eps[:], zsb[j2][:, 0:128],
                                         Achunk(2 * j2),
                                         start=(j2 == 0), stop=False)
                        nc.tensor.matmul(Eps[:], zsb[j2][:, 128:256],
                                         Achunk(2 * j2 + 1),
                                         start=False, stop=False)
                # tail: feats [yaT(80); ones]
                yaTp = pp.tile([128, 256], F32, tag="zTp")
                nc.tensor.matmul(yaTp[0:80, 0:128], yap16[:, 80:160], id_t,
                                 start=True, stop=True)
                nc.scalar.copy(etail[0:80, :], yaTp[0:80, 0:128])
                nc.tensor.matmul(Eps[:], etail[:], At_t, start=False,
                                 stop=True)
                # --- s* argmax directly on PSUM, d* = 80 - s* ---
                nc.vector.max(mx8[:], Eps[:])
                nc.vector.max_index(mi8[:], mx8[:], Eps[:])
                nc.vector.tensor_copy(sf[:], mi8[:, 0:1])
                nc.vector.tensor_scalar(df[:], sf[:], -1.0, 80.0,
                                        Op.mult, Op.add)
                # --- yhat: scatter yap16[80+j] -> yhat[80+j-s*] ---
                nc.vector.scalar_tensor_tensor(ixf[:, 0:80], io_t[:, 80:160],
                                               sf[:, 0:1], io_t[:, 80:160],
                                               Op.subtract, Op.bypass)
                nc.vector.tensor_copy(ix3[:], ixf[:, 0:80])
                nc.gpsimd.local_scatter(yhat[:], yap16[:, 80:160], ix3[:],
                                        channels=128, num_elems=256,
                                        num_idxs=80)
                # --- h_selT = W_enc @ yhat^T (+ b_enc) ---
                yhTp = pph.tile([128, 128], F32, tag="Hp")
                nc.tensor.matmul(yhTp[:], yhat[:, 0:128], id_t,
                                 start=True, stop=True)
                nc.scalar.copy(yhT0[:], yhTp[:])
                yhTp2 = pph.tile([128, 128], F32, tag="Hp")
                nc.tensor.matmul(yhTp2[0:32, :], yhat[:, 128:160], id_t,
                                 start=True, stop=True)
                nc.vector.tensor_copy(yhT1[:], yhTp2[0:32, :])
                for hc in range(4):
                    Hp = pph.tile([128, 128], F32, tag="Hp")
                    nc.tensor.matmul(Hp[:], We_t[:, hc * 128:(hc + 1) * 128],
                                     yhT0[:], start=True, stop=False)
                    nc.tensor.matmul(Hp[:],
                                     We_t[0:32, 512 + hc * 128:512 + (hc + 1) * 128],
                                     yhT1[:], start=False, stop=True)
                    if hc % 2 == 0:
                        nc.scalar.activation(hsT[:, hc * 128:(hc + 1) * 128],
                                             Hp[:], AF.Identity,
                                             bias=be_t[:, hc:hc + 1])
                    else:
                        nc.vector.tensor_scalar(hsT[:, hc * 128:(hc + 1) * 128],
                                                Hp[:], be_t[:, hc:hc + 1],
                                                None, Op.add)
                # --- x_extT = W_src @ h_selT (+ b_src) ---
                for oc in range(2):
                    ow = 128 if oc == 0 else 32
                    Xp = pph.tile([128, 128], F32, tag="Hp")
                    for hc in range(4):
                        nc.tensor.matmul(
                            Xp[0:ow, :],
                            Ws_t[:, hc * 160 + oc * 128: hc * 160 + oc * 128 + ow],
                            hsT[:, hc * 128:(hc + 1) * 128],
                            start=(hc == 0), stop=(hc == 3))
                    dst = xeT0 if oc == 0 else xeT1
                    if oc == 0:
                        nc.scalar.activation(dst[:], Xp[0:ow, :], AF.Identity,
                                             bias=bs_t[0:ow, 0:1])
                    else:
                        nc.vector.tensor_scalar(dst[:], Xp[0:ow, :],
                                                bs_t[0:ow, 1:2], None, Op.add)
                Xtp = pph.tile([128, 128], F32, tag="Hp")
                nc.tensor.matmul(Xtp[:], xeT0[:], id_t, start=True, stop=True)
                nc.scalar.copy(xext16[:, 0:128], Xtp[:])
                Xtp2 = pph.tile([128, 128], F32, tag="Hp")
                nc.tensor.matmul(Xtp2[:, 0:32], xeT1[:], c16[0:32, OF_ID:OF_ID + 32],
                                 start=True, stop=True)
                nc.vector.tensor_copy(xext16[:, 128:160], Xtp2[:, 0:32])
                # --- y_ele: scatter xext16[j] -> yele[j-d*] ---
                nc.vector.scalar_tensor_tensor(ixf[:, 0:160], io_t[:, 0:160],
                                               df[:, 0:1], io_t[:, 0:160],
                                               Op.subtract, Op.bypass)
                nc.vector.tensor_copy(ix4[:], ixf[:, 0:160])
                nc.gpsimd.local_scatter(yele[:], xext16[:], ix4[:],
                                        channels=128, num_elems=160,
                                        num_idxs=160)
                # --- loss partial + state updates ---
                nc.vector.tensor_tensor(dtmp[:], yele[:, 0:80], yres[:],
                                        Op.subtract)
                nc.vector.tensor_tensor(dtmp[:], dtmp[:], keep[:], Op.mult)
                nc.scalar.activation(dsq[:], dtmp[:], AF.Square,
                                     accum_out=lossp[:, it:it + 1])
                nc.vector.tensor_tensor(yres[:], yres[:], yele[:, 0:80],
                                        Op.subtract)

            nc.sync.dma_start(d_out[:], lossp[:])
    return nc


def kernel(x, y, W_enc, b_enc, W_src, b_src):
    import sys
    if '/opt/trn_rl_repo' not in sys.path:
        sys.path.insert(0, '/opt/trn_rl_repo')
    x = np.asarray(x, np.float32)
    y = np.asarray(y, np.float32)
    consts = _build_consts(W_enc, b_enc, W_src, b_src)

    if "nc" not in _cache:
        _cache["nc"] = _build_nc()
        _cache["nc"].finalize()
    nc = _cache["nc"]

    in_maps = _make_in_maps(x, y, consts)
    from concourse.bass_utils import run_bass_kernel_spmd
    res = run_bass_kernel_spmd(nc, in_maps, list(range(NCORES)))
    parts = np.stack([r["losspart"] for r in res.results])
    keep_cnt = max(int((y != 0.0).sum()), 1)
    nums = parts[:, :, :THINK_ITER].sum(axis=(0, 1), dtype=np.float64)
    losses = (nums / keep_cnt).astype(np.float32)
    return np.float32(np.mean(losses))


# revision 13
# speedup vs baseline: 6.6250x; 1.0020x over previous
"""Trainium2 Bass kernel for nn_Net_17532056502451.

5 "think" iterations: shift-window cosine selector (159 shifts) + softmax
attention + scatter-back + conv-style encoder/decoder with energy argmax
(81 shifts), masked-MSE losses averaged.  Data-parallel: 1024 tokens over
8 cores, 128 tokens/core (one per SBUF partition), token-major.

v4 design notes:
- dot correlation: fp16 broadcast-product (DVE 2x mode) + 2x fold-adds
  (80->40->20->10) + one small strided reduce.
- energy Gram features packed 6400 -> 4096 = 32 chunks of 128; whole PE
  path fp16 (1-pass matmuls, FWL); transposes via matmul against fp16
  identity; E-matmul operands swapped so E accumulates TOKEN-major in
  PSUM (argmax reads PSUM directly).
- E-pipeline: chunk PAIRS share one [128,256] PSUM tile; PSUM->SBUF
  copies are 256 wide and alternate Vector/Scalar engines.
- per-token dynamic window gathers via gpsimd local_scatter with
  per-partition indices (idx[p,j] = j - start_p, negatives ignored);
  gathers read only the 80 nonzero source columns.
- all constants pre-swizzled on host into one fp16 blob + one fp32 blob
  (3 input DMAs total instead of ~46).
"""
import numpy as np

IDIM = 80
ODIM = 80
HDIM = 512
THINK_ITER = 5
TEMPER = 0.7
B, T = 4, 256
NTOK = B * T
P = 128
NCORES = 8
S1 = 159
S2 = 81
ZBLOCKS = [(0, 32, 80), (32, 48, 48), (48, 64, 32), (64, 80, 16)]
NFEAT = sum((d1 - d0) * im for d0, d1, im in ZBLOCKS)   # 4096
NCHUNK = NFEAT // 128   # 32
NPAIR = NCHUNK // 2
# fp16 const blob column offsets
OF_A = 0
OF_AT = OF_A + NCHUNK * 81          # 2592
OF_WE = OF_AT + 81                  # 2673
OF_WS = OF_WE + 1024                # 3697
OF_ID = OF_WS + 640                 # 4337
OF_ON = OF_ID + 128                 # 4465
W16 = OF_ON + 128                   # 4593
# fp32 const blob: benc(4) bsrc(2) iota(256)
OF_BE = 0
OF_BS = 4
OF_IO = 6
W32 = 262

_cache = {}


def _feat_list():
    feats = []
    for d0, d1, im in ZBLOCKS:
        for d in range(d0, d1):
            for i in range(im):
                feats.append((d, i))
    return feats


def _build_consts(W_enc, b_enc, W_src, b_src):
    W_enc = np.asarray(W_enc, np.float32)
    b_enc = np.asarray(b_enc, np.float32)
    W_src = np.asarray(W_src, np.float32)
    b_src = np.asarray(b_src, np.float32)
    C = (W_enc.T @ W_enc).astype(np.float32)
    q = (W_enc.T @ b_enc).astype(np.float32)
    bb = np.float32(b_enc @ b_enc)
    feats = _feat_list()
    Az = np.zeros((S2, NFEAT), np.float32)
    Al = np.zeros((S2, 81), np.float32)
    for s in range(S2):
        dd = 80 - s
        for f, (d, i) in enumerate(feats):
            if i < 80 - d:
                Az[s, f] = (2.0 if d > 0 else 1.0) * C[dd + i, dd + i + d]
        Al[s, :80] = 2.0 * q[dd:dd + 80]
        Al[s, 80] = bb
    c16 = np.zeros((P, W16), np.float16)
    # A: chunk k at cols OF_A + k*81, partition p holds Az.T[k*128+p, :]
    AzT = np.ascontiguousarray(Az.T).astype(np.float16)          # (4096, 81)
    c16[:, OF_A:OF_AT] = AzT.reshape(NCHUNK, 128, 81).transpose(1, 0, 2) \
                            .reshape(128, NCHUNK * 81)
    c16[0:81, OF_AT:OF_AT + 81] = np.ascontiguousarray(Al.T).astype(np.float16)
    WeT = np.ascontiguousarray(W_enc.T).astype(np.float16)       # (160, 512)
    c16[:, OF_WE:OF_WE + 512] = WeT[0:128]
    c16[0:32, OF_WE + 512:OF_WE + 1024] = WeT[128:160]
    WsT = np.ascontiguousarray(W_src.T).astype(np.float16)       # (512, 160)
    c16[:, OF_WS:OF_WS + 640] = WsT.reshape(4, 128, 160).transpose(1, 0, 2) \
                                   .reshape(128, 640)
    c16[:, OF_ID:OF_ID + 128] = np.eye(128, dtype=np.float16)
    c16[:, OF_ON:OF_ON + 128] = 1.0
    c32 = np.zeros((P, W32), np.float32)
    c32[:, OF_BE:OF_BE + 4] = b_enc.reshape(4, 128).T
    c32[:, OF_BS] = b_src[0:128]
    c32[0:32, OF_BS + 1] = b_src[128:160]
    c32[:, OF_IO:OF_IO + 256] = np.arange(256, dtype=np.float32)
    return dict(c16=c16, c32=c32)


def _make_in_maps(x, y, consts):
    xt = x.reshape(NTOK, IDIM)
    yt = y.reshape(NTOK, ODIM)
    in_maps = []
    for c in range(NCORES):
        m = dict(consts)
        m["xy"] = np.ascontiguousarray(
            np.concatenate([xt[c * P:(c + 1) * P], yt[c * P:(c + 1) * P]],
                           axis=1))
        in_maps.append(m)
    return in_maps


def _build_nc():
    import concourse.bass as bass
    import concourse.bacc as bacc
    import concourse.mybir as mybir
    from concourse.tile import TileContext

    F32 = mybir.dt.float32
    F16 = mybir.dt.float16
    I16 = mybir.dt.int16
    U32 = mybir.dt.uint32
    Op = mybir.AluOpType
    AF = mybir.ActivationFunctionType

    nc = bacc.Bacc()
    d_xy = nc.declare_dram_parameter("xy", [P, 160], F32, isOutput=False)
    d_c16 = nc.declare_dram_parameter("c16", [P, W16], F16, isOutput=False)
    d_c32 = nc.declare_dram_parameter("c32", [P, W32], F32, isOutput=False)
    d_out = nc.declare_dram_parameter("losspart", [P, 8], F32, isOutput=True)

    with TileContext(nc) as tc:
        with (
            tc.tile_pool(name="const", bufs=1) as cpool,
            tc.tile_pool(name="work", bufs=1) as pool,
            tc.tile_pool(name="zrot", bufs=6) as zpool,
            tc.tile_pool(name="ps_rot", bufs=3, space="PSUM") as pp,
            tc.tile_pool(name="ps_h", bufs=3, space="PSUM") as pph,
            tc.tile_pool(name="ps_acc", bufs=1, space="PSUM") as ppe,
        ):
            # ---- inputs + constants (3 DMAs) ----
            xy_t = pool.tile([P, 160], F32, tag="xy")
            nc.sync.dma_start(xy_t[:], d_xy[:])
            c16 = cpool.tile([P, W16], F16, tag="c16")
            nc.sync.dma_start(c16[:], d_c16[:])
            c32 = cpool.tile([P, W32], F32, tag="c32")
            nc.sync.dma_start(c32[:], d_c32[:])

            def Achunk(k):
                return c16[:, OF_A + k * 81:OF_A + (k + 1) * 81]
            At_t = c16[0:81, OF_AT:OF_AT + 81]
            We_t = c16[:, OF_WE:OF_WE + 1024]
            Ws_t = c16[:, OF_WS:OF_WS + 640]
            id_t = c16[:, OF_ID:OF_ID + 128]
            be_t = c32[:, OF_BE:OF_BE + 4]
            bs_t = c32[:, OF_BS:OF_BS + 2]
            io_t = c32[:, OF_IO:OF_IO + 256]

            # ---- state ----
            xpad = pool.tile([P, 238], F32, tag="xpad")
            xpad16 = pool.tile([P, 238], F16, tag="xpad16")
            yres = pool.tile([P, 80], F32, tag="yres")
            keep = pool.tile([P, 80], F32, tag="keep")
            yap16 = pool.tile([P, 240], F16, tag="yap16")
            lossp = pool.tile([P, 8], F32, tag="lossp")
            nc.vector.memset(xpad[:], 0.0)
            nc.vector.memset(yap16[:], 0.0)
            nc.vector.memset(lossp[:], 0.0)
            nc.scalar.copy(xpad[:, 79:159], xy_t[:, 0:80])
            nc.vector.tensor_copy(yres[:], xy_t[:, 80:160])
            nc.vector.tensor_scalar(keep[:], yres[:], 0.0, None, Op.not_equal)

            sqx = pool.tile([P, 239], F32, tag="sqx")
            nc.vector.memset(sqx[:, 0:1], 0.0)
            cs = pool.tile([P, 239], F32, tag="cs")
            nsq = pool.tile([P, S1], F32, tag="nsq")
            rnsq = pool.tile([P, S1], F32, tag="rnsq")
            yres16 = pool.tile([P, 80], F16, tag="yres16")
            w2 = pool.tile([P, S1 * 80], F16, tag="w2")
            w4 = pool.tile([P, S1 * 40], F16, tag="w4")
            w5 = pool.tile([P, S1 * 20], F16, tag="w5")
            w6 = pool.tile([P, S1 * 10], F16, tag="w6")
            dot16 = pool.tile([P, S1], F16, tag="dot16")
            adot = pool.tile([P, S1], F16, tag="adot")
            gsel = pool.tile([P, S1], F32, tag="gsel")
            mx8 = pool.tile([P, 8], F32, tag="mx8")
            mi8 = pool.tile([P, 8], U32, tag="mi8")
            thf = pool.tile([P, 1], F32, tag="thf")
            th2 = pool.tile([P, 1], F32, tag="th2")
            sf = pool.tile([P, 1], F32, tag="sf")
            df = pool.tile([P, 1], F32, tag="df")
            ixf = pool.tile([P, 160], F32, tag="ixf")
            ix1 = pool.tile([P, 80], I16, tag="ix1")
            ix2 = pool.tile([P, 80], I16, tag="ix2")
            ix3 = pool.tile([P, 80], I16, tag="ix3")
            ix4 = pool.tile([P, 160], I16, tag="ix4")
            yal = pool.tile([P, 256], F16, tag="yal")
            xele = pool.tile([P, 256], F16, tag="xele")
            yhat = pool.tile([P, 256], F16, tag="yhat")
            yele = pool.tile([P, 160], F16, tag="yele")
            zt = pool.tile([P, 80], F32, tag="zt")
            et = pool.tile([P, 80], F32, tag="et")
            ssum = pool.tile([P, 1], F32, tag="ssum")
            rsum = pool.tile([P, 1], F32, tag="rsum")
            nzm = pool.tile([P, 1], F32, tag="nzm")
            zero1 = pool.tile([P, 1], F32, tag="zero1")
            nc.vector.memset(zero1[:], 0.0)
            zf16 = pool.tile([P, NFEAT], F16, tag="zf16")
            etail = pool.tile([81, 128], F16, tag="etail")
            nc.sync.dma_start(etail[80:81, :], d_c16[80:81, OF_ON:OF_ON + 128])
            yhT0 = pool.tile([128, 128], F16, tag="yhT0")
            yhT1 = pool.tile([32, 128], F16, tag="yhT1")
            hsT = pool.tile([128, 4 * 128], F16, tag="hsT")
            xeT0 = pool.tile([128, 128], F16, tag="xeT0")
            xeT1 = pool.tile([32, 128], F16, tag="xeT1")
            xext16 = pool.tile([P, 160], F16, tag="xext16")
            dtmp = pool.tile([P, 80], F32, tag="dtmp")
            dsq = pool.tile([P, 80], F32, tag="dsq")

            def vap(tile_ap, free0, fdims):
                b = tile_ap
                return bass.AP(b.tensor, b.offset + free0,
                               [list(b.ap[0])] + list(fdims))

            for it in range(THINK_ITER):
                # --- sliding norms ---
                nc.scalar.activation(sqx[:, 1:239], xpad[:], AF.Square)
                nc.vector.tensor_tensor_scan(cs[:], sqx[:],
                                             zero1[:].to_broadcast((P, 239)),
                                             0.0, Op.add, Op.bypass)
                nc.vector.tensor_tensor(nsq[:], cs[:, 80:239], cs[:, 0:159],
                                        Op.subtract)
                nc.vector.tensor_scalar_max(rnsq[:], nsq[:], 1e-30)
                nc.vector.reciprocal_approx_fast(rnsq[:], rnsq[:])
                # --- dot: fp16 product (2x) + fold-adds + small reduce ---
                nc.scalar.copy(xpad16[:], xpad[:])
                nc.scalar.copy(yres16[:], yres[:])
                in0 = vap(xpad16[:], 0, [[1, S1], [1, 80]])
                in1 = yres16[:].unsqueeze(1).to_broadcast((P, S1, 80))
                wv = w2[:].rearrange("p (s c) -> p s c", c=80)
                nc.vector.tensor_tensor(wv, in0, in1, Op.mult)
                with nc.allow_low_precision("argmax-only dot"):
                    nc.vector.tensor_tensor(
                        w4[:].rearrange("p (s c) -> p s c", c=40),
                        vap(w2[:], 0, [[80, S1], [1, 40]]),
                        vap(w2[:], 40, [[80, S1], [1, 40]]), Op.add)
                    nc.vector.tensor_tensor(
                        w5[:].rearrange("p (s c) -> p s c", c=20),
                        vap(w4[:], 0, [[40, S1], [1, 20]]),
                        vap(w4[:], 20, [[40, S1], [1, 20]]), Op.add)
                    nc.vector.tensor_tensor(
                        w6[:].rearrange("p (s c) -> p s c", c=10),
                        vap(w5[:], 0, [[20, S1], [1, 10]]),
                        vap(w5[:], 10, [[20, S1], [1, 10]]), Op.add)
                    nc.vector.tensor_reduce(dot16[:],
                                            vap(w6[:], 0, [[10, S1], [1, 10]]),
                                            mybir.AxisListType.X, Op.add)
                # --- theta = argmax dot*|dot|/nsq ---
                nc.scalar.activation(adot[:], dot16[:], AF.Abs)
                nc.vector.tensor_tensor(gsel[:], dot16[:], adot[:], Op.mult)
                nc.vector.tensor_tensor(gsel[:], gsel[:], rnsq[:], Op.mult)
                nc.vector.max(mx8[:], gsel[:])
                nc.vector.max_index(mi8[:], mx8[:], gsel[:])
                nc.vector.tensor_copy(thf[:], mi8[:, 0:1])
                # --- y_align: scatter xpad16[79+j] -> yal[79+j-theta] ---
                nc.vector.scalar_tensor_tensor(ixf[:, 0:80], io_t[:, 79:159],
                                               thf[:, 0:1], io_t[:, 79:159],
                                               Op.subtract, Op.bypass)
                nc.vector.tensor_copy(ix1[:], ixf[:, 0:80])
                nc.gpsimd.local_scatter(yal[:], xpad16[:, 79:159], ix1[:],
                                        channels=128, num_elems=256,
                                        num_idxs=80)
                # --- softmax attention -> y_att in yap16[:, 80:160] ---
                nc.vector.tensor_tensor(zt[:], yal[:, 0:80], yres[:], Op.mult)
                nc.vector.max(mx8[:], zt[:])
                nc.vector.tensor_scalar_mul(nzm[:], mx8[:, 0:1], -1.0 / TEMPER)
                nc.scalar.activation(et[:], zt[:], AF.Exp, bias=nzm[:, 0:1],
                                     scale=1.0 / TEMPER)
                nc.vector.tensor_reduce(ssum[:], et[:], mybir.AxisListType.X,
                                        Op.add)
                nc.vector.reciprocal_approx_fast(rsum[:], ssum[:])
                nc.vector.scalar_tensor_tensor(yap16[:, 80:160], et[:],
                                                rsum[:, 0:1], yal[:, 0:80],
                                                Op.mult, Op.mult)
                # --- x_ele: scatter yap16[80+j] -> xele[j+theta-79] ---
                nc.vector.tensor_scalar(th2[:], thf[:], -1.0, 159.0,
                                        Op.mult, Op.add)
                nc.vector.scalar_tensor_tensor(ixf[:, 0:80], io_t[:, 80:160],
                                               th2[:, 0:1], io_t[:, 80:160],
                                               Op.subtract, Op.bypass)
                nc.vector.tensor_copy(ix2[:], ixf[:, 0:80])
                nc.gpsimd.local_scatter(xele[:], yap16[:, 80:160], ix2[:],
                                        channels=128, num_elems=256,
                                        num_idxs=80)
                nc.vector.tensor_tensor(xpad[:, 79:159], xpad[:, 79:159],
                                        xele[:, 0:80], Op.subtract)
                # --- z features (fp16, packed 4096) ---
                foff = 0
                yb = yap16[:, 80:240]
                for d0, d1, im in ZBLOCKS:
                    nblk = (d1 - d0) * im
                    ov = bass.AP(zf16[:].tensor, zf16[:].offset + foff,
                                 [list(zf16[:].ap[0]), [im, d1 - d0], [1, im]])
                    b0 = bass.AP(yb.tensor, yb.offset,
                                 [list(yb.ap[0]), [0, d1 - d0], [1, im]])
                    b1 = bass.AP(yb.tensor, yb.offset + d0,
                                 [list(yb.ap[0]), [1, d1 - d0], [1, im]])
                    nc.vector.tensor_tensor(ov, b0, b1, Op.mult)
                    foff += nblk
                # --- E: transpose chunk-pairs -> one 256-wide copy (V/ACT
                #     alternating) -> token-major accumulate in PSUM ---
                Eps = ppe.tile([128, 81], F32, tag="Eps")
                zsb = [None] * NPAIR
                for k2 in range(NPAIR + 1):
                    if k2 < NPAIR:
                        zTp = pp.tile([128, 256], F32, tag="zTp")
                        nc.tensor.matmul(zTp[:, 0:128],
                                         zf16[:, (2 * k2) * 128:(2 * k2 + 1) * 128],
                                         id_t, start=True, stop=True)
                        nc.tensor.matmul(zTp[:, 128:256],
                                         zf16[:, (2 * k2 + 1) * 128:(2 * k2 + 2) * 128],
                                         id_t, start=True, stop=True)
                        zsb_k = zpool.tile([128, 256], F16, tag="zT")
                        zsb[k2] = zsb_k
                        if k2 % 4 == 3:
                            nc.vector.tensor_copy(zsb[k2][:], zTp[:])
                        else:
                            nc.scalar.copy(zsb[k2][:], zTp[:])
                    j2 = k2 - 1
                    if 0 <= j2 < NPAIR:
                        nc.tensor.matmul(Eps[:], zsb[j2][:, 0:128],
                                         Achunk(2 * j2),
                                         start=(j2 == 0), stop=False)
                        nc.tensor.matmul(Eps[:], zsb[j2][:, 128:256],
                                         Achunk(2 * j2 + 1),
                                         start=False, stop=False)
                # tail: feats [yaT(80); ones]
                yaTp = pp.tile([128, 256], F32, tag="zTp")
                nc.tensor.matmul(yaTp[0:80, 0:128], yap16[:, 80:160], id_t,
                                 start=True, stop=True)
                nc.scalar.copy(etail[0:80, :], yaTp[0:80, 0:128])
                nc.tensor.matmul(Eps[:], etail[:], At_t, start=False,
                                 stop=True)
                # --- s* argmax directly on PSUM, d* = 80 - s* ---
                nc.vector.max(mx8[:], Eps[:])
                nc.vector.max_index(mi8[:], mx8[:], Eps[:])
                nc.vector.tensor_copy(sf[:], mi8[:, 0:1])
                nc.vector.tensor_scalar(df[:], sf[:], -1.0, 80.0,
                                        Op.mult, Op.add)
                # --- yhat: scatter yap16[80+j] -> yhat[80+j-s*] ---
                nc.vector.scalar_tensor_tensor(ixf[:, 0:80], io_t[:, 80:160],
                                               sf[:, 0:1], io_t[:, 80:160],
                                               Op.subtract, Op.bypass)
                nc.vector.tensor_copy(ix3[:], ixf[:, 0:80])
                nc.gpsimd.local_scatter(yhat[:], yap16[:, 80:160], ix3[:],
                                        channels=128, num_elems=256,
                                        num_idxs=80)
                # --- h_selT = W_enc @ yhat^T (+ b_enc) ---
                yhTp = pph.tile([128, 128], F32, tag="Hp")
                nc.tensor.matmul(yhTp[:], yhat[:, 0:128], id_t,
                                 start=True, stop=True)
                nc.scalar.copy(yhT0[:], yhTp[:])
                yhTp2 = pph.tile([128, 128], F32, tag="Hp")
                nc.tensor.matmul(yhTp2[0:32, :], yhat[:, 128:160], id_t,
                                 start=True, stop=True)
                nc.scalar.copy(yhT1[:], yhTp2[0:32, :])
                for hc in range(4):
                    Hp = pph.tile([128, 128], F32, tag="Hp")
                    nc.tensor.matmul(Hp[:], We_t[:, hc * 128:(hc + 1) * 128],
                                     yhT0[:], start=True, stop=False)
                    nc.tensor.matmul(Hp[:],
                                     We_t[0:32, 512 + hc * 128:512 + (hc + 1) * 128],
                                     yhT1[:], start=False, stop=True)
                    nc.scalar.activation(hsT[:, hc * 128:(hc + 1) * 128],
                                         Hp[:], AF.Identity,
                                         bias=be_t[:, hc:hc + 1])
                # --- x_extT = W_src @ h_selT (+ b_src) ---
                for oc in range(2):
                    ow = 128 if oc == 0 else 32
                    Xp = pph.tile([128, 128], F32, tag="Hp")
                    for hc in range(4):
                        nc.tensor.matmul(
                            Xp[0:ow, :],
                            Ws_t[:, hc * 160 + oc * 128: hc * 160 + oc * 128 + ow],
                            hsT[:, hc * 128:(hc + 1) * 128],
                            start=(hc == 0), stop=(hc == 3))
                    dst = xeT0 if oc == 0 else xeT1
                    nc.scalar.activation(dst[:], Xp[0:ow, :], AF.Identity,
                                         bias=bs_t[0:ow, oc:oc + 1])
                Xtp = pph.tile([128, 128], F32, tag="Hp")
                nc.tensor.matmul(Xtp[:], xeT0[:], id_t, start=True, stop=True)
                nc.scalar.copy(xext16[:, 0:128], Xtp[:])
                Xtp2 = pph.tile([128, 128], F32, tag="Hp")
                nc.tensor.matmul(Xtp2[:, 0:32], xeT1[:], c16[0:32, OF_ID:OF_ID + 32],
                                 start=True, stop=True)
                nc.scalar.copy(xext16[:, 128:160], Xtp2[:, 0:32])
                # --- y_ele: scatter xext16[j] -> yele[j-d*] ---
                nc.vector.scalar_tensor_tensor(ixf[:, 0:160], io_t[:, 0:160],
                                               df[:, 0:1], io_t[:, 0:160],
                                               Op.subtract, Op.bypass)
                nc.vector.tensor_copy(ix4[:], ixf[:, 0:160])
                nc.gpsimd.local_scatter(yele[:], xext16[:], ix4[:],
                                        channels=128, num_elems=160,
                                        num_idxs=160)
                # --- loss partial + state updates ---
                nc.vector.tensor_tensor(dtmp[:], yele[:, 0:80], yres[:],
                                        Op.subtract)
                nc.vector.tensor_tensor(dtmp[:], dtmp[:], keep[:], Op.mult)
                nc.scalar.activation(dsq[:], dtmp[:], AF.Square,
                                     accum_out=lossp[:, it:it + 1])
                nc.vector.tensor_tensor(yres[:], yres[:], yele[:, 0:80],
                                        Op.subtract)

            nc.sync.dma_start(d_out[:], lossp[:])
    return nc


def kernel(x, y, W_enc, b_enc, W_src, b_src):
    import sys
    if '/opt/trn_rl_repo' not in sys.path:
        sys.path.insert(0, '/opt/trn_rl_repo')
    x = np.asarray(x, np.float32)
    y = np.asarray(y, np.float32)
    consts = _build_consts(W_enc, b_enc, W_src, b_src)

    if "nc" not in _cache:
        _cache["nc"] = _build_nc()
        _cache["nc"].finalize()
    nc = _cache["nc"]

    in_maps = _make_in_maps(x, y, consts)
    from concourse.bass_utils import run_bass_kernel_spmd
    res = run_bass_kernel_spmd(nc, in_maps, list(range(NCORES)))
    parts = np.stack([r["losspart"] for r in res.results])
    keep_cnt = max(int((y != 0.0).sum()), 1)
    nums = parts[:, :, :THINK_ITER].sum(axis=(0, 1), dtype=np.float64)
    losses = (nums / keep_cnt).astype(np.float32)
    return np.float32(np.mean(losses))


# revision 15
# speedup vs baseline: 6.6424x; 1.0026x over previous
"""Trainium2 Bass kernel for nn_Net_17532056502451.

5 "think" iterations: shift-window cosine selector (159 shifts) + softmax
attention + scatter-back + conv-style encoder/decoder with energy argmax
(81 shifts), masked-MSE losses averaged.  Data-parallel: 1024 tokens over
8 cores, 128 tokens/core (one per SBUF partition), token-major.

v4 design notes:
- dot correlation: fp16 broadcast-product (DVE 2x mode) + 2x fold-adds
  (80->40->20->10) + one small strided reduce.
- energy Gram features packed 6400 -> 4096 = 32 chunks of 128; whole PE
  path fp16 (1-pass matmuls, FWL); transposes via matmul against fp16
  identity; E-matmul operands swapped so E accumulates TOKEN-major in
  PSUM (argmax reads PSUM directly).
- E-pipeline: chunk PAIRS share one [128,256] PSUM tile; PSUM->SBUF
  copies are 256 wide and alternate Vector/Scalar engines.
- per-token dynamic window gathers via gpsimd local_scatter with
  per-partition indices (idx[p,j] = j - start_p, negatives ignored);
  gathers read only the 80 nonzero source columns.
- all constants pre-swizzled on host into one fp16 blob + one fp32 blob
  (3 input DMAs total instead of ~46).
"""
import numpy as np

IDIM = 80
ODIM = 80
HDIM = 512
THINK_ITER = 5
TEMPER = 0.7
B, T = 4, 256
NTOK = B * T
P = 128
NCORES = 8
S1 = 159
S2 = 81
ZBLOCKS = [(0, 32, 80), (32, 48, 48), (48, 64, 32), (64, 80, 16)]
NFEAT = sum((d1 - d0) * im for d0, d1, im in ZBLOCKS)   # 4096
NCHUNK = NFEAT // 128   # 32
NPAIR = NCHUNK // 2
# fp16 const blob column offsets
OF_A = 0
OF_AT = OF_A + NCHUNK * 81          # 2592
OF_WE = OF_AT + 81                  # 2673
OF_WS = OF_WE + 1024                # 3697
OF_ID = OF_WS + 640                 # 4337
OF_ON = OF_ID + 128                 # 4465
W16 = OF_ON + 128                   # 4593
# fp32 const blob: benc(4) bsrc(2) iota(256)
OF_BE = 0
OF_BS = 4
OF_IO = 6
W32 = 262

_cache = {}


def _feat_list():
    feats = []
    for d0, d1, im in ZBLOCKS:
        for d in range(d0, d1):
            for i in range(im):
                feats.append((d, i))
    return feats


def _build_consts(W_enc, b_enc, W_src, b_src):
    W_enc = np.asarray(W_enc, np.float32)
    b_enc = np.asarray(b_enc, np.float32)
    W_src = np.asarray(W_src, np.float32)
    b_src = np.asarray(b_src, np.float32)
    C = (W_enc.T @ W_enc).astype(np.float32)
    q = (W_enc.T @ b_enc).astype(np.float32)
    bb = np.float32(b_enc @ b_enc)
    feats = _feat_list()
    Az = np.zeros((S2, NFEAT), np.float32)
    Al = np.zeros((S2, 81), np.float32)
    for s in range(S2):
        dd = 80 - s
        for f, (d, i) in enumerate(feats):
            if i < 80 - d:
                Az[s, f] = (2.0 if d > 0 else 1.0) * C[dd + i, dd + i + d]
        Al[s, :80] = 2.0 * q[dd:dd + 80]
        Al[s, 80] = bb
    c16 = np.zeros((P, W16), np.float16)
    # A: chunk k at cols OF_A + k*81, partition p holds Az.T[k*128+p, :]
    AzT = np.ascontiguousarray(Az.T).astype(np.float16)          # (4096, 81)
    c16[:, OF_A:OF_AT] = AzT.reshape(NCHUNK, 128, 81).transpose(1, 0, 2) \
                            .reshape(128, NCHUNK * 81)
    c16[0:81, OF_AT:OF_AT + 81] = np.ascontiguousarray(Al.T).astype(np.float16)
    WeT = np.ascontiguousarray(W_enc.T).astype(np.float16)       # (160, 512)
    c16[:, OF_WE:OF_WE + 512] = WeT[0:128]
    c16[0:32, OF_WE + 512:OF_WE + 1024] = WeT[128:160]
    WsT = np.ascontiguousarray(W_src.T).astype(np.float16)       # (512, 160)
    c16[:, OF_WS:OF_WS + 640] = WsT.reshape(4, 128, 160).transpose(1, 0, 2) \
                                   .reshape(128, 640)
    c16[:, OF_ID:OF_ID + 128] = np.eye(128, dtype=np.float16)
    c16[:, OF_ON:OF_ON + 128] = 1.0
    c32 = np.zeros((P, W32), np.float32)
    c32[:, OF_BE:OF_BE + 4] = b_enc.reshape(4, 128).T
    c32[:, OF_BS] = b_src[0:128]
    c32[0:32, OF_BS + 1] = b_src[128:160]
    c32[:, OF_IO:OF_IO + 256] = np.arange(256, dtype=np.float32)
    return dict(c16=c16, c32=c32)


def _make_in_maps(x, y, consts):
    xt = x.reshape(NTOK, IDIM)
    yt = y.reshape(NTOK, ODIM)
    in_maps = []
    for c in range(NCORES):
        m = dict(consts)
        m["xy"] = np.ascontiguousarray(
            np.concatenate([xt[c * P:(c + 1) * P], yt[c * P:(c + 1) * P]],
                           axis=1))
        in_maps.append(m)
    return in_maps


def _build_nc():
    import concourse.bass as bass
    import concourse.bacc as bacc
    import concourse.mybir as mybir
    from concourse.tile import TileContext

    F32 = mybir.dt.float32
    F16 = mybir.dt.float16
    I16 = mybir.dt.int16
    U32 = mybir.dt.uint32
    Op = mybir.AluOpType
    AF = mybir.ActivationFunctionType

    nc = bacc.Bacc()
    d_xy = nc.declare_dram_parameter("xy", [P, 160], F32, isOutput=False)
    d_c16 = nc.declare_dram_parameter("c16", [P, W16], F16, isOutput=False)
    d_c32 = nc.declare_dram_parameter("c32", [P, W32], F32, isOutput=False)
    d_out = nc.declare_dram_parameter("losspart", [P, 8], F32, isOutput=True)

    with TileContext(nc) as tc:
        with (
            tc.tile_pool(name="const", bufs=1) as cpool,
            tc.tile_pool(name="work", bufs=1) as pool,
            tc.tile_pool(name="zrot", bufs=6) as zpool,
            tc.tile_pool(name="ps_rot", bufs=3, space="PSUM") as pp,
            tc.tile_pool(name="ps_h", bufs=3, space="PSUM") as pph,
            tc.tile_pool(name="ps_acc", bufs=1, space="PSUM") as ppe,
        ):
            # ---- inputs + constants (3 DMAs) ----
            xy_t = pool.tile([P, 160], F32, tag="xy")
            nc.sync.dma_start(xy_t[:], d_xy[:])
            c16 = cpool.tile([P, W16], F16, tag="c16")
            HH = W16 // 2
            nc.gpsimd.dma_start(c16[:, 0:HH], d_c16[:, 0:HH])
            nc.scalar.dma_start(c16[:, HH:W16], d_c16[:, HH:W16])
            c32 = cpool.tile([P, W32], F32, tag="c32")
            nc.sync.dma_start(c32[:], d_c32[:])

            def Achunk(k):
                return c16[:, OF_A + k * 81:OF_A + (k + 1) * 81]
            At_t = c16[0:81, OF_AT:OF_AT + 81]
            We_t = c16[:, OF_WE:OF_WE + 1024]
            Ws_t = c16[:, OF_WS:OF_WS + 640]
            id_t = c16[:, OF_ID:OF_ID + 128]
            be_t = c32[:, OF_BE:OF_BE + 4]
            bs_t = c32[:, OF_BS:OF_BS + 2]
            io_t = c32[:, OF_IO:OF_IO + 256]

            # ---- state ----
            xpad = pool.tile([P, 238], F32, tag="xpad")
            xpad16 = pool.tile([P, 238], F16, tag="xpad16")
            yres = pool.tile([P, 80], F32, tag="yres")
            keep = pool.tile([P, 80], F32, tag="keep")
            yap16 = pool.tile([P, 240], F16, tag="yap16")
            lossp = pool.tile([P, 8], F32, tag="lossp")
            nc.vector.memset(xpad[:], 0.0)
            nc.vector.memset(yap16[:], 0.0)
            nc.vector.memset(lossp[:], 0.0)
            nc.scalar.copy(xpad[:, 79:159], xy_t[:, 0:80])
            nc.vector.tensor_copy(yres[:], xy_t[:, 80:160])
            nc.vector.tensor_scalar(keep[:], yres[:], 0.0, None, Op.not_equal)

            sqx = pool.tile([P, 239], F32, tag="sqx")
            nc.vector.memset(sqx[:, 0:1], 0.0)
            cs = pool.tile([P, 239], F32, tag="cs")
            nsq = pool.tile([P, S1], F32, tag="nsq")
            rnsq = pool.tile([P, S1], F32, tag="rnsq")
            yres16 = pool.tile([P, 80], F16, tag="yres16")
            w2 = pool.tile([P, S1 * 80], F16, tag="w2")
            nc.vector.memset(w2[:], 0.0)
            w4 = pool.tile([P, S1 * 40], F16, tag="w4")
            w5 = pool.tile([P, S1 * 20], F16, tag="w5")
            w6 = pool.tile([P, S1 * 10], F16, tag="w6")
            dot16 = pool.tile([P, S1], F16, tag="dot16")
            adot = pool.tile([P, S1], F16, tag="adot")
            gsel = pool.tile([P, S1], F32, tag="gsel")
            mx8 = pool.tile([P, 8], F32, tag="mx8")
            mi8 = pool.tile([P, 8], U32, tag="mi8")
            thf = pool.tile([P, 1], F32, tag="thf")
            th2 = pool.tile([P, 1], F32, tag="th2")
            sf = pool.tile([P, 1], F32, tag="sf")
            df = pool.tile([P, 1], F32, tag="df")
            ix1 = pool.tile([P, 80], I16, tag="ix1")
            ix2 = pool.tile([P, 80], I16, tag="ix2")
            ix3 = pool.tile([P, 80], I16, tag="ix3")
            ix4 = pool.tile([P, 160], I16, tag="ix4")
            yal = pool.tile([P, 256], F16, tag="yal")
            xele = pool.tile([P, 256], F16, tag="xele")
            yhat = pool.tile([P, 256], F16, tag="yhat")
            yele = pool.tile([P, 160], F16, tag="yele")
            zt = pool.tile([P, 80], F32, tag="zt")
            et = pool.tile([P, 80], F32, tag="et")
            ssum = pool.tile([P, 1], F32, tag="ssum")
            rsum = pool.tile([P, 1], F32, tag="rsum")
            nzm = pool.tile([P, 1], F32, tag="nzm")
            zero1 = pool.tile([P, 1], F32, tag="zero1")
            nc.vector.memset(zero1[:], 0.0)
            zf16 = pool.tile([P, NFEAT], F16, tag="zf16")
            etail = pool.tile([81, 128], F16, tag="etail")
            nc.sync.dma_start(etail[80:81, :], d_c16[80:81, OF_ON:OF_ON + 128])
            yhT0 = pool.tile([128, 128], F16, tag="yhT0")
            yhT1 = pool.tile([32, 128], F16, tag="yhT1")
            hsT = pool.tile([128, 4 * 128], F16, tag="hsT")
            xeT0 = pool.tile([128, 128], F16, tag="xeT0")
            xeT1 = pool.tile([32, 128], F16, tag="xeT1")
            xext16 = pool.tile([P, 160], F16, tag="xext16")
            dtmp = pool.tile([P, 80], F32, tag="dtmp")
            dsq = pool.tile([P, 80], F32, tag="dsq")

            def vap(tile_ap, free0, fdims):
                b = tile_ap
                return bass.AP(b.tensor, b.offset + free0,
                               [list(b.ap[0])] + list(fdims))

            for it in range(THINK_ITER):
                # --- sliding norms ---
                nc.scalar.activation(sqx[:, 1:239], xpad[:], AF.Square)
                nc.vector.tensor_tensor_scan(cs[:], sqx[:],
                                             zero1[:].to_broadcast((P, 239)),
                                             0.0, Op.add, Op.bypass)
                nc.vector.tensor_tensor(nsq[:], cs[:, 80:239], cs[:, 0:159],
                                        Op.subtract)
                nc.vector.tensor_scalar_max(rnsq[:], nsq[:], 1e-30)
                nc.vector.reciprocal_approx_fast(rnsq[:], rnsq[:])
                # --- dot: fp16 product (2x) + fold-adds + small reduce ---
                nc.scalar.copy(xpad16[:], xpad[:])
                nc.scalar.copy(yres16[:], yres[:])
                # band E1: s in [0,40), c in [40,80)
                nc.vector.tensor_tensor(
                    vap(w2[:], 40, [[80, 40], [1, 40]]),
                    vap(xpad16[:], 40, [[1, 40], [1, 40]]),
                    vap(yres16[:], 40, [[0, 40], [1, 40]]), Op.mult)
                # band C: s in [40,119), full c
                nc.vector.tensor_tensor(
                    vap(w2[:], 40 * 80, [[80, 79], [1, 80]]),
                    vap(xpad16[:], 40, [[1, 79], [1, 80]]),
                    vap(yres16[:], 0, [[0, 79], [1, 80]]), Op.mult)
                # band E2: s in [119,159), c in [0,40)
                nc.vector.tensor_tensor(
                    vap(w2[:], 119 * 80, [[80, 40], [1, 40]]),
                    vap(xpad16[:], 119, [[1, 40], [1, 40]]),
                    vap(yres16[:], 0, [[0, 40], [1, 40]]), Op.mult)
                with nc.allow_low_precision("argmax-only dot"):
                    nc.vector.tensor_tensor(
                        w4[:].rearrange("p (s c) -> p s c", c=40),
                        vap(w2[:], 0, [[80, S1], [1, 40]]),
                        vap(w2[:], 40, [[80, S1], [1, 40]]), Op.add)
                    nc.vector.tensor_tensor(
                        w5[:].rearrange("p (s c) -> p s c", c=20),
                        vap(w4[:], 0, [[40, S1], [1, 20]]),
                        vap(w4[:], 20, [[40, S1], [1, 20]]), Op.add)
                    nc.vector.tensor_tensor(
                        w6[:].rearrange("p (s c) -> p s c", c=10),
                        vap(w5[:], 0, [[20, S1], [1, 10]]),
                        vap(w5[:], 10, [[20, S1], [1, 10]]), Op.add)
                    nc.vector.tensor_reduce(dot16[:],
                                            vap(w6[:], 0, [[10, S1], [1, 10]]),
                                            mybir.AxisListType.X, Op.add)
                # --- theta = argmax dot*|dot|/nsq ---
                nc.scalar.activation(adot[:], dot16[:], AF.Abs)
                nc.vector.tensor_tensor(gsel[:], dot16[:], adot[:], Op.mult)
                nc.vector.tensor_tensor(gsel[:], gsel[:], rnsq[:], Op.mult)
                nc.vector.max(mx8[:], gsel[:])
                nc.vector.max_index(mi8[:], mx8[:], gsel[:])
                nc.vector.tensor_copy(thf[:], mi8[:, 0:1])
                # --- y_align: scatter xpad16[79+j] -> yal[79+j-theta] ---
                nc.vector.scalar_tensor_tensor(ix1[:], io_t[:, 79:159],
                                               thf[:, 0:1], io_t[:, 79:159],
                                               Op.subtract, Op.bypass)
                nc.gpsimd.local_scatter(yal[:], xpad16[:, 79:159], ix1[:],
                                        channels=128, num_elems=256,
                                        num_idxs=80)
                # --- softmax attention -> y_att in yap16[:, 80:160] ---
                nc.vector.tensor_tensor(zt[:], yal[:, 0:80], yres[:], Op.mult)
                nc.vector.max(mx8[:], zt[:])
                nc.vector.tensor_scalar_mul(nzm[:], mx8[:, 0:1], -1.0 / TEMPER)
                nc.scalar.activation(et[:], zt[:], AF.Exp, bias=nzm[:, 0:1],
                                     scale=1.0 / TEMPER)
                nc.vector.tensor_reduce(ssum[:], et[:], mybir.AxisListType.X,
                                        Op.add)
                nc.vector.reciprocal_approx_fast(rsum[:], ssum[:])
                nc.vector.scalar_tensor_tensor(yap16[:, 80:160], et[:],
                                                rsum[:, 0:1], yal[:, 0:80],
                                                Op.mult, Op.mult)
                # --- x_ele: scatter yap16[80+j] -> xele[j+theta-79] ---
                nc.vector.tensor_scalar(th2[:], thf[:], -1.0, 159.0,
                                        Op.mult, Op.add)
                nc.vector.scalar_tensor_tensor(ix2[:], io_t[:, 80:160],
                                               th2[:, 0:1], io_t[:, 80:160],
                                               Op.subtract, Op.bypass)
                nc.gpsimd.local_scatter(xele[:], yap16[:, 80:160], ix2[:],
                                        channels=128, num_elems=256,
                                        num_idxs=80)
                nc.vector.tensor_tensor(xpad[:, 79:159], xpad[:, 79:159],
                                        xele[:, 0:80], Op.subtract)
                # --- z features (fp16, packed 4096) ---
                foff = 0
                yb = yap16[:, 80:240]
                for d0, d1, im in ZBLOCKS:
                    nblk = (d1 - d0) * im
                    ov = bass.AP(zf16[:].tensor, zf16[:].offset + foff,
                                 [list(zf16[:].ap[0]), [im, d1 - d0], [1, im]])
                    b0 = bass.AP(yb.tensor, yb.offset,
                                 [list(yb.ap[0]), [0, d1 - d0], [1, im]])
                    b1 = bass.AP(yb.tensor, yb.offset + d0,
                                 [list(yb.ap[0]), [1, d1 - d0], [1, im]])
                    nc.vector.tensor_tensor(ov, b0, b1, Op.mult)
                    foff += nblk
                # --- E: transpose chunk-pairs -> one 256-wide copy (V/ACT
                #     alternating) -> token-major accumulate in PSUM ---
                Eps = ppe.tile([128, 81], F32, tag="Eps")
                zsb = [None] * NPAIR
                for k2 in range(NPAIR + 1):
                    if k2 < NPAIR:
                        zTp = pp.tile([128, 256], F32, tag="zTp")
                        nc.tensor.matmul(zTp[:, 0:128],
                                         zf16[:, (2 * k2) * 128:(2 * k2 + 1) * 128],
                                         id_t, start=True, stop=True)
                        nc.tensor.matmul(zTp[:, 128:256],
                                         zf16[:, (2 * k2 + 1) * 128:(2 * k2 + 2) * 128],
                                         id_t, start=True, stop=True)
                        zsb_k = zpool.tile([128, 256], F16, tag="zT")
                        zsb[k2] = zsb_k
                        if k2 % 4 == 3:
                            nc.vector.tensor_copy(zsb[k2][:], zTp[:])
                        else:
                            nc.scalar.copy(zsb[k2][:], zTp[:])
                    j2 = k2 - 1
                    if 0 <= j2 < NPAIR:
                        nc.tensor.matmul(Eps[:], zsb[j2][:, 0:128],
                                         Achunk(2 * j2),
                                         start=(j2 == 0), stop=False)
                        nc.tensor.matmul(Eps[:], zsb[j2][:, 128:256],
                                         Achunk(2 * j2 + 1),
                                         start=False, stop=False)
                # tail: feats [yaT(80); ones]
                yaTp = pp.tile([128, 256], F32, tag="zTp")
                nc.tensor.matmul(yaTp[0:80, 0:128], yap16[:, 80:160], id_t,
                                 start=True, stop=True)
                nc.scalar.copy(etail[0:80, :], yaTp[0:80, 0:128])
                nc.tensor.matmul(Eps[:], etail[:], At_t, start=False,
                                 stop=True)
                # --- s* argmax directly on PSUM, d* = 80 - s* ---
                nc.vector.max(mx8[:], Eps[:])
                nc.vector.max_index(mi8[:], mx8[:], Eps[:])
                nc.vector.tensor_copy(sf[:], mi8[:, 0:1])
                nc.vector.tensor_scalar(df[:], sf[:], -1.0, 80.0,
                                        Op.mult, Op.add)
                # --- yhat: scatter yap16[80+j] -> yhat[80+j-s*] ---
                nc.vector.scalar_tensor_tensor(ix3[:], io_t[:, 80:160],
                                               sf[:, 0:1], io_t[:, 80:160],
                                               Op.subtract, Op.bypass)
                nc.gpsimd.local_scatter(yhat[:], yap16[:, 80:160], ix3[:],
                                        channels=128, num_elems=256,
                                        num_idxs=80)
                # --- h_selT = W_enc @ yhat^T (+ b_enc) ---
                yhTp = pph.tile([128, 128], F32, tag="Hp")
                nc.tensor.matmul(yhTp[:], yhat[:, 0:128], id_t,
                                 start=True, stop=True)
                nc.scalar.copy(yhT0[:], yhTp[:])
                yhTp2 = pph.tile([128, 128], F32, tag="Hp")
                nc.tensor.matmul(yhTp2[0:32, :], yhat[:, 128:160], id_t,
                                 start=True, stop=True)
                nc.scalar.copy(yhT1[:], yhTp2[0:32, :])
                for hc in range(4):
                    Hp = pph.tile([128, 128], F32, tag="Hp")
                    nc.tensor.matmul(Hp[:], We_t[:, hc * 128:(hc + 1) * 128],
                                     yhT0[:], start=True, stop=False)
                    nc.tensor.matmul(Hp[:],
                                     We_t[0:32, 512 + hc * 128:512 + (hc + 1) * 128],
                                     yhT1[:], start=False, stop=True)
                    nc.scalar.activation(hsT[:, hc * 128:(hc + 1) * 128],
                                         Hp[:], AF.Identity,
                                         bias=be_t[:, hc:hc + 1])
                # --- x_extT = W_src @ h_selT (+ b_src) ---
                for oc in range(2):
                    ow = 128 if oc == 0 else 32
                    Xp = pph.tile([128, 128], F32, tag="Hp")
                    for hc in range(4):
                        nc.tensor.matmul(
                            Xp[0:ow, :],
                            Ws_t[:, hc * 160 + oc * 128: hc * 160 + oc * 128 + ow],
                            hsT[:, hc * 128:(hc + 1) * 128],
                            start=(hc == 0), stop=(hc == 3))
                    dst = xeT0 if oc == 0 else xeT1
                    nc.scalar.activation(dst[:], Xp[0:ow, :], AF.Identity,
                                         bias=bs_t[0:ow, oc:oc + 1])
                Xtp = pph.tile([128, 128], F32, tag="Hp")
                nc.tensor.matmul(Xtp[:], xeT0[:], id_t, start=True, stop=True)
                nc.scalar.copy(xext16[:, 0:128], Xtp[:])
                Xtp2 = pph.tile([128, 128], F32, tag="Hp")
                nc.tensor.matmul(Xtp2[:, 0:32], xeT1[:], c16[0:32, OF_ID:OF_ID + 32],
                                 start=True, stop=True)
                nc.scalar.copy(xext16[:, 128:160], Xtp2[:, 0:32])
                # --- y_ele: scatter xext16[j] -> yele[j-d*] ---
                nc.vector.scalar_tensor_tensor(ix4[:], io_t[:, 0:160],
                                               df[:, 0:1], io_t[:, 0:160],
                                               Op.subtract, Op.bypass)
                nc.gpsimd.local_scatter(yele[:], xext16[:], ix4[:],
                                        channels=128, num_elems=160,
                                        num_idxs=160)
                # --- loss partial + state updates ---
                nc.vector.tensor_tensor(dtmp[:], yele[:, 0:80], yres[:],
                                        Op.subtract)
                nc.vector.tensor_tensor(dtmp[:], dtmp[:], keep[:], Op.mult)
                nc.scalar.activation(dsq[:], dtmp[:], AF.Square,
                                     accum_out=lossp[:, it:it + 1])
                nc.vector.tensor_tensor(yres[:], yres[:], yele[:, 0:80],
                                        Op.subtract)

            nc.sync.dma_start(d_out[:], lossp[:])
    return nc


def kernel(x, y, W_enc, b_enc, W_src, b_src):
    import sys
    if '/opt/trn_rl_repo' not in sys.path:
        sys.path.insert(0, '/opt/trn_rl_repo')
    x = np.asarray(x, np.float32)
    y = np.asarray(y, np.float32)
    consts = _build_consts(W_enc, b_enc, W_src, b_src)

    if "nc" not in _cache:
        _cache["nc"] = _build_nc()
        _cache["nc"].finalize()
    nc = _cache["nc"]

    in_maps = _make_in_maps(x, y, consts)
    from concourse.bass_utils import run_bass_kernel_spmd
    res = run_bass_kernel_spmd(nc, in_maps, list(range(NCORES)))
    parts = np.stack([r["losspart"] for r in res.results])
    keep_cnt = max(int((y != 0.0).sum()), 1)
    nums = parts[:, :, :THINK_ITER].sum(axis=(0, 1), dtype=np.float64)
    losses = (nums / keep_cnt).astype(np.float32)
    return np.float32(np.mean(losses))


# revision 17
# speedup vs baseline: 6.7038x; 1.0092x over previous
"""Trainium2 Bass kernel for nn_Net_17532056502451.

5 "think" iterations: shift-window cosine selector (159 shifts) + softmax
attention + scatter-back + conv-style encoder/decoder with energy argmax
(81 shifts), masked-MSE losses averaged.  Data-parallel: 1024 tokens over
8 cores, 128 tokens/core (one per SBUF partition), token-major.

v4 design notes:
- dot correlation: fp16 broadcast-product (DVE 2x mode) + 2x fold-adds
  (80->40->20->10) + one small strided reduce.
- energy Gram features packed 6400 -> 4096 = 32 chunks of 128; whole PE
  path fp16 (1-pass matmuls, FWL); transposes via matmul against fp16
  identity; E-matmul operands swapped so E accumulates TOKEN-major in
  PSUM (argmax reads PSUM directly).
- E-pipeline: chunk PAIRS share one [128,256] PSUM tile; PSUM->SBUF
  copies are 256 wide and alternate Vector/Scalar engines.
- per-token dynamic window gathers via gpsimd local_scatter with
  per-partition indices (idx[p,j] = j - start_p, negatives ignored);
  gathers read only the 80 nonzero source columns.
- all constants pre-swizzled on host into one fp16 blob + one fp32 blob
  (3 input DMAs total instead of ~46).
"""
import numpy as np

IDIM = 80
ODIM = 80
HDIM = 512
THINK_ITER = 5
TEMPER = 0.7
B, T = 4, 256
NTOK = B * T
P = 128
NCORES = 8
S1 = 159
S2 = 81
ZBLOCKS = [(0, 32, 80), (32, 48, 48), (48, 64, 32), (64, 80, 16)]
NFEAT = sum((d1 - d0) * im for d0, d1, im in ZBLOCKS)   # 4096
NCHUNK = NFEAT // 128   # 32
NPAIR = NCHUNK // 2
# fp16 const blob column offsets
OF_A = 0
OF_AT = OF_A + NCHUNK * 81          # 2592
OF_WE = OF_AT + 81                  # 2673
OF_WS = OF_WE + 1024                # 3697
OF_ID = OF_WS + 640                 # 4337
OF_ON = OF_ID + 128                 # 4465
OF_IX = OF_ON + 128                 # 4593
W16 = OF_IX + 2                     # 4595
# fp32 const blob: benc(4) bsrc(2) iota(256)
OF_BE = 0
OF_BS = 4
OF_IO = 6
W32 = 262

_cache = {}


def _feat_list():
    feats = []
    for d0, d1, im in ZBLOCKS:
        for d in range(d0, d1):
            for i in range(im):
                feats.append((d, i))
    return feats


def _build_consts(W_enc, b_enc, W_src, b_src):
    W_enc = np.asarray(W_enc, np.float32)
    b_enc = np.asarray(b_enc, np.float32)
    W_src = np.asarray(W_src, np.float32)
    b_src = np.asarray(b_src, np.float32)
    C = (W_enc.T @ W_enc).astype(np.float32)
    q = (W_enc.T @ b_enc).astype(np.float32)
    bb = np.float32(b_enc @ b_enc)
    feats = _feat_list()
    Az = np.zeros((S2, NFEAT), np.float32)
    Al = np.zeros((S2, 81), np.float32)
    for s in range(S2):
        dd = 80 - s
        for f, (d, i) in enumerate(feats):
            if i < 80 - d:
                Az[s, f] = (2.0 if d > 0 else 1.0) * C[dd + i, dd + i + d]
        Al[s, :80] = 2.0 * q[dd:dd + 80]
        Al[s, 80] = bb
    c16 = np.zeros((P, W16), np.float16)
    # A: chunk k at cols OF_A + k*81, partition p holds Az.T[k*128+p, :]
    AzT = np.ascontiguousarray(Az.T).astype(np.float16)          # (4096, 81)
    c16[:, OF_A:OF_AT] = AzT.reshape(NCHUNK, 128, 81).transpose(1, 0, 2) \
                            .reshape(128, NCHUNK * 81)
    c16[0:81, OF_AT:OF_AT + 81] = np.ascontiguousarray(Al.T).astype(np.float16)
    WeT = np.ascontiguousarray(W_enc.T).astype(np.float16)       # (160, 512)
    c16[:, OF_WE:OF_WE + 512] = WeT[0:128]
    c16[0:32, OF_WE + 512:OF_WE + 1024] = WeT[128:160]
    WsT = np.ascontiguousarray(W_src.T).astype(np.float16)       # (512, 160)
    c16[:, OF_WS:OF_WS + 640] = WsT.reshape(4, 128, 160).transpose(1, 0, 2) \
                                   .reshape(128, 640)
    c16[:, OF_ID:OF_ID + 128] = np.eye(128, dtype=np.float16)
    c16[:, OF_ON:OF_ON + 128] = 1.0
    c16[:, OF_IX:OF_IX + 2] = np.broadcast_to(
        np.array([0, 1], np.int16).view(np.float16), (P, 2))
    c32 = np.zeros((P, W32), np.float32)
    c32[:, OF_BE:OF_BE + 4] = b_enc.reshape(4, 128).T
    c32[:, OF_BS] = b_src[0:128]
    c32[0:32, OF_BS + 1] = b_src[128:160]
    c32[:, OF_IO:OF_IO + 256] = np.arange(256, dtype=np.float32)
    return dict(c16=c16, c32=c32)


def _make_in_maps(x, y, consts):
    xt = x.reshape(NTOK, IDIM)
    yt = y.reshape(NTOK, ODIM)
    in_maps = []
    for c in range(NCORES):
        m = dict(consts)
        m["xy"] = np.ascontiguousarray(
            np.concatenate([xt[c * P:(c + 1) * P], yt[c * P:(c + 1) * P]],
                           axis=1))
        in_maps.append(m)
    return in_maps


def _build_nc():
    import concourse.bass as bass
    import concourse.bacc as bacc
    import concourse.mybir as mybir
    from concourse.tile import TileContext

    F32 = mybir.dt.float32
    F16 = mybir.dt.float16
    I16 = mybir.dt.int16
    U32 = mybir.dt.uint32
    Op = mybir.AluOpType
    AF = mybir.ActivationFunctionType

    nc = bacc.Bacc()
    d_xy = nc.declare_dram_parameter("xy", [P, 160], F32, isOutput=False)
    d_c16 = nc.declare_dram_parameter("c16", [P, W16], F16, isOutput=False)
    d_c32 = nc.declare_dram_parameter("c32", [P, W32], F32, isOutput=False)
    d_out = nc.declare_dram_parameter("losspart", [P, 8], F32, isOutput=True)

    with TileContext(nc) as tc:
        with (
            tc.tile_pool(name="const", bufs=1) as cpool,
            tc.tile_pool(name="work", bufs=1) as pool,
            tc.tile_pool(name="zrot", bufs=6) as zpool,
            tc.tile_pool(name="ps_rot", bufs=3, space="PSUM") as pp,
            tc.tile_pool(name="ps_h", bufs=3, space="PSUM") as pph,
            tc.tile_pool(name="ps_acc", bufs=1, space="PSUM") as ppe,
        ):
            # ---- inputs + constants (3 DMAs) ----
            xy_t = pool.tile([P, 160], F32, tag="xy")
            nc.sync.dma_start(xy_t[:], d_xy[:])
            c16 = cpool.tile([P, W16], F16, tag="c16")
            HH = W16 // 2
            nc.gpsimd.dma_start(c16[:, 0:HH], d_c16[:, 0:HH])
            nc.scalar.dma_start(c16[:, HH:W16], d_c16[:, HH:W16])
            c32 = cpool.tile([P, W32], F32, tag="c32")
            nc.sync.dma_start(c32[:], d_c32[:])

            def Achunk(k):
                return c16[:, OF_A + k * 81:OF_A + (k + 1) * 81]
            At_t = c16[0:81, OF_AT:OF_AT + 81]
            We_t = c16[:, OF_WE:OF_WE + 1024]
            Ws_t = c16[:, OF_WS:OF_WS + 640]
            id_t = c16[:, OF_ID:OF_ID + 128]
            be_t = c32[:, OF_BE:OF_BE + 4]
            bs_t = c32[:, OF_BS:OF_BS + 2]
            io_t = c32[:, OF_IO:OF_IO + 256]

            # ---- state ----
            xpad = pool.tile([P, 238], F32, tag="xpad")
            xpad16 = pool.tile([P, 238], F16, tag="xpad16")
            yres = pool.tile([P, 80], F32, tag="yres")
            keep = pool.tile([P, 80], F32, tag="keep")
            yap16 = pool.tile([P, 240], F16, tag="yap16")
            lossp = pool.tile([P, 8], F32, tag="lossp")
            nc.vector.memset(xpad[:], 0.0)
            nc.vector.memset(yap16[:], 0.0)
            nc.vector.memset(lossp[:], 0.0)
            nc.scalar.copy(xpad[:, 79:159], xy_t[:, 0:80])
            nc.vector.tensor_copy(yres[:], xy_t[:, 80:160])
            nc.vector.tensor_scalar(keep[:], yres[:], 0.0, None, Op.not_equal)

            sqx = pool.tile([P, 239], F32, tag="sqx")
            nc.vector.memset(sqx[:, 0:1], 0.0)
            cs = pool.tile([P, 239], F32, tag="cs")
            nsq = pool.tile([P, S1], F32, tag="nsq")
            rnsq = pool.tile([P, S1], F32, tag="rnsq")
            yres16 = pool.tile([P, 80], F16, tag="yres16")
            w2 = pool.tile([P, S1 * 80], F16, tag="w2")
            nc.vector.memset(w2[:], 0.0)
            w4 = pool.tile([P, S1 * 40], F16, tag="w4")
            w5 = pool.tile([P, S1 * 20], F16, tag="w5")
            w6 = pool.tile([P, S1 * 10], F16, tag="w6")
            dot16 = pool.tile([P, S1], F16, tag="dot16")
            adot = pool.tile([P, S1], F16, tag="adot")
            gsel = pool.tile([P, S1], F32, tag="gsel")
            mx8 = pool.tile([P, 8], F32, tag="mx8")
            mi8 = pool.tile([P, 8], U32, tag="mi8")
            thf = pool.tile([P, 1], F32, tag="thf")
            th2 = pool.tile([P, 1], F32, tag="th2")
            sf = pool.tile([P, 1], F32, tag="sf")
            df = pool.tile([P, 1], F32, tag="df")
            ix1 = pool.tile([P, 80], I16, tag="ix1")
            ix2 = pool.tile([P, 80], I16, tag="ix2")
            ix3 = pool.tile([P, 80], I16, tag="ix3")
            ix4 = pool.tile([P, 160], I16, tag="ix4")
            yal = pool.tile([P, 256], F16, tag="yal")
            xele = pool.tile([P, 256], F16, tag="xele")
            yhat = pool.tile([P, 256], F16, tag="yhat")
            yele = pool.tile([P, 160], F16, tag="yele")
            zt = pool.tile([P, 80], F32, tag="zt")
            et = pool.tile([P, 80], F32, tag="et")
            ssum = pool.tile([P, 1], F32, tag="ssum")
            rsum = pool.tile([P, 1], F32, tag="rsum")
            nzm = pool.tile([P, 1], F32, tag="nzm")
            zero1 = pool.tile([P, 1], F32, tag="zero1")
            nc.vector.memset(zero1[:], 0.0)
            zf16 = pool.tile([P, NFEAT], F16, tag="zf16")
            etail = pool.tile([81, 128], F16, tag="etail")
            nc.sync.dma_start(etail[80:81, :], d_c16[80:81, OF_ON:OF_ON + 128])
            yhT0 = pool.tile([128, 128], F16, tag="yhT0")
            yhT1 = pool.tile([32, 128], F16, tag="yhT1")
            hsT = pool.tile([128, 4 * 128], F16, tag="hsT")
            xeT0 = pool.tile([128, 128], F16, tag="xeT0")
            xeT1 = pool.tile([32, 128], F16, tag="xeT1")
            xext16 = pool.tile([P, 160], F16, tag="xext16")
            dtmp = pool.tile([P, 80], F32, tag="dtmp")
            dsq = pool.tile([P, 80], F32, tag="dsq")
            gdum = pool.tile([P, 2], F16, tag="gdum")
            ixdum = c16[:, OF_IX:OF_IX + 2].bitcast(I16)

            def vap(tile_ap, free0, fdims):
                b = tile_ap
                return bass.AP(b.tensor, b.offset + free0,
                               [list(b.ap[0])] + list(fdims))

            for it in range(THINK_ITER):
                # gpsimd library warm-up: absorbs the MODIFY_POOL_CONFIG +
                # IRAM load off the critical path while Vector runs the dot
                nc.gpsimd.local_scatter(gdum[:], c16[:, OF_IX:OF_IX + 2],
                                        ixdum, channels=128, num_elems=2,
                                        num_idxs=2)
                # --- sliding norms ---
                nc.scalar.activation(sqx[:, 1:239], xpad[:], AF.Square)
                nc.vector.tensor_tensor_scan(cs[:], sqx[:],
                                             zero1[:].to_broadcast((P, 239)),
                                             0.0, Op.add, Op.bypass)
                nc.vector.tensor_tensor(nsq[:], cs[:, 80:239], cs[:, 0:159],
                                        Op.subtract)
                nc.vector.tensor_scalar_max(rnsq[:], nsq[:], 1e-30)
                nc.vector.reciprocal_approx_fast(rnsq[:], rnsq[:])
                # --- dot: fp16 product (2x) + fold-adds + small reduce ---
                nc.scalar.copy(xpad16[:], xpad[:])
                nc.scalar.copy(yres16[:], yres[:])
                # band E1: s in [0,40), c in [40,80)
                nc.vector.tensor_tensor(
                    vap(w2[:], 40, [[80, 40], [1, 40]]),
                    vap(xpad16[:], 40, [[1, 40], [1, 40]]),
                    vap(yres16[:], 40, [[0, 40], [1, 40]]), Op.mult)
                # band C: s in [40,119), full c
                nc.vector.tensor_tensor(
                    vap(w2[:], 40 * 80, [[80, 79], [1, 80]]),
                    vap(xpad16[:], 40, [[1, 79], [1, 80]]),
                    vap(yres16[:], 0, [[0, 79], [1, 80]]), Op.mult)
                # band E2: s in [119,159), c in [0,40)
                nc.vector.tensor_tensor(
                    vap(w2[:], 119 * 80, [[80, 40], [1, 40]]),
                    vap(xpad16[:], 119, [[1, 40], [1, 40]]),
                    vap(yres16[:], 0, [[0, 40], [1, 40]]), Op.mult)
                with nc.allow_low_precision("argmax-only dot"):
                    nc.vector.tensor_tensor(
                        w4[:].rearrange("p (s c) -> p s c", c=40),
                        vap(w2[:], 0, [[80, S1], [1, 40]]),
                        vap(w2[:], 40, [[80, S1], [1, 40]]), Op.add)
                    nc.vector.tensor_tensor(
                        w5[:].rearrange("p (s c) -> p s c", c=20),
                        vap(w4[:], 0, [[40, S1], [1, 20]]),
                        vap(w4[:], 20, [[40, S1], [1, 20]]), Op.add)
                    nc.vector.tensor_tensor(
                        w6[:].rearrange("p (s c) -> p s c", c=10),
                        vap(w5[:], 0, [[20, S1], [1, 10]]),
                        vap(w5[:], 10, [[20, S1], [1, 10]]), Op.add)
                    nc.vector.tensor_reduce(dot16[:],
                                            vap(w6[:], 0, [[10, S1], [1, 10]]),
                                            mybir.AxisListType.X, Op.add)
                # --- theta = argmax dot*|dot|/nsq ---
                nc.scalar.activation(adot[:], dot16[:], AF.Abs)
                nc.vector.tensor_tensor(gsel[:], dot16[:], adot[:], Op.mult)
                nc.vector.tensor_tensor(gsel[:], gsel[:], rnsq[:], Op.mult)
                nc.vector.max(mx8[:], gsel[:])
                nc.vector.max_index(mi8[:], mx8[:], gsel[:])
                nc.vector.tensor_copy(thf[:], mi8[:, 0:1])
                # --- y_align: scatter xpad16[79+j] -> yal[79+j-theta] ---
                nc.vector.scalar_tensor_tensor(ix1[:], io_t[:, 79:159],
                                               thf[:, 0:1], io_t[:, 79:159],
                                               Op.subtract, Op.bypass)
                nc.gpsimd.local_scatter(yal[:], xpad16[:, 79:159], ix1[:],
                                        channels=128, num_elems=256,
                                        num_idxs=80)
                # --- softmax attention -> y_att in yap16[:, 80:160] ---
                nc.vector.tensor_tensor(zt[:], yal[:, 0:80], yres[:], Op.mult)
                nc.vector.max(mx8[:], zt[:])
                nc.vector.tensor_scalar_mul(nzm[:], mx8[:, 0:1], -1.0 / TEMPER)
                nc.scalar.activation(et[:], zt[:], AF.Exp, bias=nzm[:, 0:1],
                                     scale=1.0 / TEMPER)
                nc.vector.tensor_reduce(ssum[:], et[:], mybir.AxisListType.X,
                                        Op.add)
                nc.vector.reciprocal_approx_fast(rsum[:], ssum[:])
                nc.vector.scalar_tensor_tensor(yap16[:, 80:160], et[:],
                                                rsum[:, 0:1], yal[:, 0:80],
                                                Op.mult, Op.mult)
                # --- x_ele: scatter yap16[80+j] -> xele[j+theta-79] ---
                nc.vector.tensor_scalar(th2[:], thf[:], -1.0, 159.0,
                                        Op.mult, Op.add)
                nc.vector.scalar_tensor_tensor(ix2[:], io_t[:, 80:160],
                                               th2[:, 0:1], io_t[:, 80:160],
                                               Op.subtract, Op.bypass)
                nc.gpsimd.local_scatter(xele[:], yap16[:, 80:160], ix2[:],
                                        channels=128, num_elems=256,
                                        num_idxs=80)
                nc.vector.tensor_tensor(xpad[:, 79:159], xpad[:, 79:159],
                                        xele[:, 0:80], Op.subtract)
                # --- z features (fp16, packed 4096) ---
                foff = 0
                yb = yap16[:, 80:240]
                for d0, d1, im in ZBLOCKS:
                    nblk = (d1 - d0) * im
                    ov = bass.AP(zf16[:].tensor, zf16[:].offset + foff,
                                 [list(zf16[:].ap[0]), [im, d1 - d0], [1, im]])
                    b0 = bass.AP(yb.tensor, yb.offset,
                                 [list(yb.ap[0]), [0, d1 - d0], [1, im]])
                    b1 = bass.AP(yb.tensor, yb.offset + d0,
                                 [list(yb.ap[0]), [1, d1 - d0], [1, im]])
                    nc.vector.tensor_tensor(ov, b0, b1, Op.mult)
                    foff += nblk
                # --- E: transpose chunk-pairs -> one 256-wide copy (V/ACT
                #     alternating) -> token-major accumulate in PSUM ---
                Eps = ppe.tile([128, 81], F32, tag="Eps")
                zsb = [None] * NPAIR
                for k2 in range(NPAIR + 1):
                    if k2 < NPAIR:
                        zTp = pp.tile([128, 256], F32, tag="zTp")
                        nc.tensor.matmul(zTp[:, 0:128],
                                         zf16[:, (2 * k2) * 128:(2 * k2 + 1) * 128],
                                         id_t, start=True, stop=True)
                        nc.tensor.matmul(zTp[:, 128:256],
                                         zf16[:, (2 * k2 + 1) * 128:(2 * k2 + 2) * 128],
                                         id_t, start=True, stop=True)
                        zsb_k = zpool.tile([128, 256], F16, tag="zT")
                        zsb[k2] = zsb_k
                        if k2 % 4 == 3:
                            nc.vector.tensor_copy(zsb[k2][:], zTp[:])
                        else:
                            nc.scalar.copy(zsb[k2][:], zTp[:])
                    j2 = k2 - 1
                    if 0 <= j2 < NPAIR:
                        nc.tensor.matmul(Eps[:], zsb[j2][:, 0:128],
                                         Achunk(2 * j2),
                                         start=(j2 == 0), stop=False)
                        nc.tensor.matmul(Eps[:], zsb[j2][:, 128:256],
                                         Achunk(2 * j2 + 1),
                                         start=False, stop=False)
                # tail: feats [yaT(80); ones]
                yaTp = pp.tile([128, 256], F32, tag="zTp")
                nc.tensor.matmul(yaTp[0:80, 0:128], yap16[:, 80:160], id_t,
                                 start=True, stop=True)
                nc.scalar.copy(etail[0:80, :], yaTp[0:80, 0:128])
                nc.tensor.matmul(Eps[:], etail[:], At_t, start=False,
                                 stop=True)
                # --- s* argmax directly on PSUM, d* = 80 - s* ---
                nc.vector.max(mx8[:], Eps[:])
                nc.vector.max_index(mi8[:], mx8[:], Eps[:])
                nc.vector.tensor_copy(sf[:], mi8[:, 0:1])
                nc.vector.tensor_scalar(df[:], sf[:], -1.0, 80.0,
                                        Op.mult, Op.add)
                # --- yhat: scatter yap16[80+j] -> yhat[80+j-s*] ---
                nc.vector.scalar_tensor_tensor(ix3[:], io_t[:, 80:160],
                                               sf[:, 0:1], io_t[:, 80:160],
                                               Op.subtract, Op.bypass)
                nc.gpsimd.local_scatter(yhat[:], yap16[:, 80:160], ix3[:],
                                        channels=128, num_elems=256,
                                        num_idxs=80)
                # --- h_selT = W_enc @ yhat^T (+ b_enc) ---
                yhTp = pph.tile([128, 128], F32, tag="Hp")
                nc.tensor.matmul(yhTp[:], yhat[:, 0:128], id_t,
                                 start=True, stop=True)
                nc.scalar.copy(yhT0[:], yhTp[:])
                yhTp2 = pph.tile([128, 128], F32, tag="Hp")
                nc.tensor.matmul(yhTp2[0:32, :], yhat[:, 128:160], id_t,
                                 start=True, stop=True)
                nc.scalar.copy(yhT1[:], yhTp2[0:32, :])
                for hc in range(4):
                    Hp = pph.tile([128, 128], F32, tag="Hp")
                    nc.tensor.matmul(Hp[:], We_t[:, hc * 128:(hc + 1) * 128],
                                     yhT0[:], start=True, stop=False)
                    nc.tensor.matmul(Hp[:],
                                     We_t[0:32, 512 + hc * 128:512 + (hc + 1) * 128],
                                     yhT1[:], start=False, stop=True)
                    nc.scalar.activation(hsT[:, hc * 128:(hc + 1) * 128],
                                         Hp[:], AF.Identity,
                                         bias=be_t[:, hc:hc + 1])
                # --- x_extT = W_src @ h_selT (+ b_src) ---
                for oc in range(2):
                    ow = 128 if oc == 0 else 32
                    Xp = pph.tile([128, 128], F32, tag="Hp")
                    for hc in range(4):
                        nc.tensor.matmul(
                            Xp[0:ow, :],
                            Ws_t[:, hc * 160 + oc * 128: hc * 160 + oc * 128 + ow],
                            hsT[:, hc * 128:(hc + 1) * 128],
                            start=(hc == 0), stop=(hc == 3))
                    dst = xeT0 if oc == 0 else xeT1
                    nc.scalar.activation(dst[:], Xp[0:ow, :], AF.Identity,
                                         bias=bs_t[0:ow, oc:oc + 1])
                Xtp = pph.tile([128, 128], F32, tag="Hp")
                nc.tensor.matmul(Xtp[:], xeT0[:], id_t, start=True, stop=True)
                nc.scalar.copy(xext16[:, 0:128], Xtp[:])
                Xtp2 = pph.tile([128, 128], F32, tag="Hp")
                nc.tensor.matmul(Xtp2[:, 0:32], xeT1[:], c16[0:32, OF_ID:OF_ID + 32],
                                 start=True, stop=True)
                nc.scalar.copy(xext16[:, 128:160], Xtp2[:, 0:32])
                # --- y_ele: scatter xext16[j] -> yele[j-d*] ---
                nc.vector.scalar_tensor_tensor(ix4[:], io_t[:, 0:160],
                                               df[:, 0:1], io_t[:, 0:160],
                                               Op.subtract, Op.bypass)
                nc.gpsimd.local_scatter(yele[:], xext16[:], ix4[:],
                                        channels=128, num_elems=160,
                                        num_idxs=160)
                # --- loss partial + state updates ---
                nc.vector.tensor_tensor(dtmp[:], yele[:, 0:80], yres[:],
                                        Op.subtract)
                nc.vector.tensor_tensor(dtmp[:], dtmp[:], keep[:], Op.mult)
                nc.scalar.activation(dsq[:], dtmp[:], AF.Square,
                                     accum_out=lossp[:, it:it + 1])
                nc.vector.tensor_tensor(yres[:], yres[:], yele[:, 0:80],
                                        Op.subtract)

            nc.sync.dma_start(d_out[:], lossp[:])
    return nc


def kernel(x, y, W_enc, b_enc, W_src, b_src):
    import sys
    if '/opt/trn_rl_repo' not in sys.path:
        sys.path.insert(0, '/opt/trn_rl_repo')
    x = np.asarray(x, np.float32)
    y = np.asarray(y, np.float32)
    consts = _build_consts(W_enc, b_enc, W_src, b_src)

    if "nc" not in _cache:
        _cache["nc"] = _build_nc()
        _cache["nc"].finalize()
    nc = _cache["nc"]

    in_maps = _make_in_maps(x, y, consts)
    from concourse.bass_utils import run_bass_kernel_spmd
    res = run_bass_kernel_spmd(nc, in_maps, list(range(NCORES)))
    parts = np.stack([r["losspart"] for r in res.results])
    keep_cnt = max(int((y != 0.0).sum()), 1)
    nums = parts[:, :, :THINK_ITER].sum(axis=(0, 1), dtype=np.float64)
    losses = (nums / keep_cnt).astype(np.float32)
    return np.float32(np.mean(losses))


# revision 18
# speedup vs baseline: 6.9918x; 1.0430x over previous
"""Trainium2 Bass kernel for nn_Net_17532056502451.

5 "think" iterations: shift-window cosine selector (159 shifts) + softmax
attention + scatter-back + conv-style encoder/decoder with energy argmax
(81 shifts), masked-MSE losses averaged.  Data-parallel: 1024 tokens over
8 cores, 128 tokens/core (one per SBUF partition), token-major.

v4 design notes:
- dot correlation: fp16 broadcast-product (DVE 2x mode) + 2x fold-adds
  (80->40->20->10) + one small strided reduce.
- energy Gram features packed 6400 -> 4096 = 32 chunks of 128; whole PE
  path fp16 (1-pass matmuls, FWL); transposes via matmul against fp16
  identity; E-matmul operands swapped so E accumulates TOKEN-major in
  PSUM (argmax reads PSUM directly).
- E-pipeline: chunk PAIRS share one [128,256] PSUM tile; PSUM->SBUF
  copies are 256 wide and alternate Vector/Scalar engines.
- per-token dynamic window gathers via gpsimd local_scatter with
  per-partition indices (idx[p,j] = j - start_p, negatives ignored);
  gathers read only the 80 nonzero source columns.
- all constants pre-swizzled on host into one fp16 blob + one fp32 blob
  (3 input DMAs total instead of ~46).
"""
import numpy as np

IDIM = 80
ODIM = 80
HDIM = 512
THINK_ITER = 5
TEMPER = 0.7
B, T = 4, 256
NTOK = B * T
P = 128
NCORES = 8
S1 = 159
S2 = 81
ZBLOCKS = [(0, 32, 80), (32, 48, 48), (48, 64, 32), (64, 80, 16)]
NFEAT = sum((d1 - d0) * im for d0, d1, im in ZBLOCKS)   # 4096
NCHUNK = NFEAT // 128   # 32
NPAIR = NCHUNK // 2
# fp16 const blob column offsets
OF_A = 0
OF_AT = OF_A + NCHUNK * 81          # 2592
OF_WE = OF_AT + 81                  # 2673
OF_WS = OF_WE + 1024                # 3697
OF_ID = OF_WS + 640                 # 4337
OF_ON = OF_ID + 128                 # 4465
OF_IX = OF_ON + 128                 # 4593
W16 = OF_IX + 2                     # 4595
# fp32 const blob: benc(4) bsrc(2) iota(256)
OF_BE = 0
OF_BS = 4
OF_IO = 6
W32 = 262

_cache = {}


def _feat_list():
    feats = []
    for d0, d1, im in ZBLOCKS:
        for d in range(d0, d1):
            for i in range(im):
                feats.append((d, i))
    return feats


def _build_consts(W_enc, b_enc, W_src, b_src):
    W_enc = np.asarray(W_enc, np.float32)
    b_enc = np.asarray(b_enc, np.float32)
    W_src = np.asarray(W_src, np.float32)
    b_src = np.asarray(b_src, np.float32)
    C = (W_enc.T @ W_enc).astype(np.float32)
    q = (W_enc.T @ b_enc).astype(np.float32)
    bb = np.float32(b_enc @ b_enc)
    feats = _feat_list()
    Az = np.zeros((S2, NFEAT), np.float32)
    Al = np.zeros((S2, 81), np.float32)
    for s in range(S2):
        dd = 80 - s
        for f, (d, i) in enumerate(feats):
            if i < 80 - d:
                Az[s, f] = (2.0 if d > 0 else 1.0) * C[dd + i, dd + i + d]
        Al[s, :80] = 2.0 * q[dd:dd + 80]
        Al[s, 80] = bb
    c16 = np.zeros((P, W16), np.float16)
    # A: chunk k at cols OF_A + k*81, partition p holds Az.T[k*128+p, :]
    AzT = np.ascontiguousarray(Az.T).astype(np.float16)          # (4096, 81)
    c16[:, OF_A:OF_AT] = AzT.reshape(NCHUNK, 128, 81).transpose(1, 0, 2) \
                            .reshape(128, NCHUNK * 81)
    c16[0:81, OF_AT:OF_AT + 81] = np.ascontiguousarray(Al.T).astype(np.float16)
    WeT = np.ascontiguousarray(W_enc.T).astype(np.float16)       # (160, 512)
    c16[:, OF_WE:OF_WE + 512] = WeT[0:128]
    c16[0:32, OF_WE + 512:OF_WE + 1024] = WeT[128:160]
    WsT = np.ascontiguousarray(W_src.T).astype(np.float16)       # (512, 160)
    c16[:, OF_WS:OF_WS + 640] = WsT.reshape(4, 128, 160).transpose(1, 0, 2) \
                                   .reshape(128, 640)
    c16[:, OF_ID:OF_ID + 128] = np.eye(128, dtype=np.float16)
    c16[:, OF_ON:OF_ON + 128] = 1.0
    c16[:, OF_IX:OF_IX + 2] = np.broadcast_to(
        np.array([0, 1], np.int16).view(np.float16), (P, 2))
    c32 = np.zeros((P, W32), np.float32)
    c32[:, OF_BE:OF_BE + 4] = b_enc.reshape(4, 128).T
    c32[:, OF_BS] = b_src[0:128]
    c32[0:32, OF_BS + 1] = b_src[128:160]
    c32[:, OF_IO:OF_IO + 256] = np.arange(256, dtype=np.float32)
    return dict(c16=c16, c32=c32)


def _make_in_maps(x, y, consts):
    xt = x.reshape(NTOK, IDIM)
    yt = y.reshape(NTOK, ODIM)
    in_maps = []
    for c in range(NCORES):
        m = dict(consts)
        m["xy"] = np.ascontiguousarray(
            np.concatenate([xt[c * P:(c + 1) * P], yt[c * P:(c + 1) * P]],
                           axis=1))
        in_maps.append(m)
    return in_maps


def _build_nc():
    import concourse.bass as bass
    import concourse.bacc as bacc
    import concourse.mybir as mybir
    from concourse.tile import TileContext

    F32 = mybir.dt.float32
    F16 = mybir.dt.float16
    I16 = mybir.dt.int16
    U32 = mybir.dt.uint32
    Op = mybir.AluOpType
    AF = mybir.ActivationFunctionType

    nc = bacc.Bacc()
    d_xy = nc.declare_dram_parameter("xy", [P, 160], F32, isOutput=False)
    d_c16 = nc.declare_dram_parameter("c16", [P, W16], F16, isOutput=False)
    d_c32 = nc.declare_dram_parameter("c32", [P, W32], F32, isOutput=False)
    d_out = nc.declare_dram_parameter("losspart", [P, 8], F32, isOutput=True)

    with TileContext(nc) as tc:
        with (
            tc.tile_pool(name="const", bufs=1) as cpool,
            tc.tile_pool(name="work", bufs=1) as pool,
            tc.tile_pool(name="zrot", bufs=6) as zpool,
            tc.tile_pool(name="ps_rot", bufs=3, space="PSUM") as pp,
            tc.tile_pool(name="ps_h", bufs=3, space="PSUM") as pph,
            tc.tile_pool(name="ps_acc", bufs=1, space="PSUM") as ppe,
        ):
            # ---- inputs + constants (3 DMAs) ----
            xy_t = pool.tile([P, 160], F32, tag="xy")
            nc.sync.dma_start(xy_t[:], d_xy[:])
            c16 = cpool.tile([P, W16], F16, tag="c16")
            HH = W16 // 2
            nc.gpsimd.dma_start(c16[:, 0:HH], d_c16[:, 0:HH])
            nc.scalar.dma_start(c16[:, HH:W16], d_c16[:, HH:W16])
            c32 = cpool.tile([P, W32], F32, tag="c32")
            nc.sync.dma_start(c32[:], d_c32[:])

            def Achunk(k):
                return c16[:, OF_A + k * 81:OF_A + (k + 1) * 81]
            At_t = c16[0:81, OF_AT:OF_AT + 81]
            We_t = c16[:, OF_WE:OF_WE + 1024]
            Ws_t = c16[:, OF_WS:OF_WS + 640]
            id_t = c16[:, OF_ID:OF_ID + 128]
            be_t = c32[:, OF_BE:OF_BE + 4]
            bs_t = c32[:, OF_BS:OF_BS + 2]
            io_t = c32[:, OF_IO:OF_IO + 256]

            # ---- state ----
            xpad = pool.tile([P, 238], F32, tag="xpad")
            xpad16 = pool.tile([P, 238], F16, tag="xpad16")
            yres = pool.tile([P, 80], F32, tag="yres")
            keep = pool.tile([P, 80], F32, tag="keep")
            yap16 = pool.tile([P, 240], F16, tag="yap16")
            lossp = pool.tile([P, 8], F32, tag="lossp")
            nc.vector.memset(xpad[:], 0.0)
            nc.vector.memset(yap16[:], 0.0)
            nc.vector.memset(lossp[:], 0.0)
            nc.scalar.copy(xpad[:, 79:159], xy_t[:, 0:80])
            nc.vector.tensor_copy(yres[:], xy_t[:, 80:160])
            nc.vector.tensor_scalar(keep[:], yres[:], 0.0, None, Op.not_equal)

            sqx = pool.tile([P, 239], F32, tag="sqx")
            nc.vector.memset(sqx[:, 0:1], 0.0)
            cs = pool.tile([P, 239], F32, tag="cs")
            nsq = pool.tile([P, S1], F32, tag="nsq")
            rnsq = pool.tile([P, S1], F32, tag="rnsq")
            yres16 = pool.tile([P, 80], F16, tag="yres16")
            w2 = pool.tile([P, S1 * 80], F16, tag="w2")
            nc.vector.memset(w2[:], 0.0)
            w4 = pool.tile([P, S1 * 40], F16, tag="w4")
            w5 = pool.tile([P, S1 * 20], F16, tag="w5")
            w6 = pool.tile([P, S1 * 10], F16, tag="w6")
            dot16 = pool.tile([P, S1], F16, tag="dot16")
            adot = pool.tile([P, S1], F16, tag="adot")
            gsel = pool.tile([P, S1], F32, tag="gsel")
            mx8 = pool.tile([P, 8], F32, tag="mx8")
            mi8 = pool.tile([P, 8], U32, tag="mi8")
            thf = pool.tile([P, 1], F32, tag="thf")
            th2 = pool.tile([P, 1], F32, tag="th2")
            sf = pool.tile([P, 1], F32, tag="sf")
            df = pool.tile([P, 1], F32, tag="df")
            ix1 = pool.tile([P, 80], I16, tag="ix1")
            ix2 = pool.tile([P, 80], I16, tag="ix2")
            ix3 = pool.tile([P, 80], I16, tag="ix3")
            ix4 = pool.tile([P, 160], I16, tag="ix4")
            yal = pool.tile([P, 256], F16, tag="yal")
            xele = pool.tile([P, 256], F16, tag="xele")
            yhat = pool.tile([P, 256], F16, tag="yhat")
            yele = pool.tile([P, 160], F16, tag="yele")
            zt = pool.tile([P, 80], F32, tag="zt")
            et = pool.tile([P, 80], F32, tag="et")
            ssum = pool.tile([P, 1], F32, tag="ssum")
            rsum = pool.tile([P, 1], F32, tag="rsum")
            nzm = pool.tile([P, 1], F32, tag="nzm")
            zero1 = pool.tile([P, 1], F32, tag="zero1")
            nc.vector.memset(zero1[:], 0.0)
            zf16 = pool.tile([P, NFEAT], F16, tag="zf16")
            etail = pool.tile([81, 128], F16, tag="etail")
            nc.sync.dma_start(etail[80:81, :], d_c16[80:81, OF_ON:OF_ON + 128])
            yhT0 = pool.tile([128, 128], F16, tag="yhT0")
            yhT1 = pool.tile([32, 128], F16, tag="yhT1")
            hsT = pool.tile([128, 4 * 128], F16, tag="hsT")
            xeT0 = pool.tile([128, 128], F16, tag="xeT0")
            xeT1 = pool.tile([32, 128], F16, tag="xeT1")
            xext16 = pool.tile([P, 160], F16, tag="xext16")
            dtmp = pool.tile([P, 80], F32, tag="dtmp")
            dsq = pool.tile([P, 80], F32, tag="dsq")
            gdum = pool.tile([P, 2], F16, tag="gdum")
            ixdum = c16[:, OF_IX:OF_IX + 2].bitcast(I16)

            def vap(tile_ap, free0, fdims):
                b = tile_ap
                return bass.AP(b.tensor, b.offset + free0,
                               [list(b.ap[0])] + list(fdims))

            for it in range(THINK_ITER):
                # gpsimd library warm-up: absorbs the MODIFY_POOL_CONFIG +
                # IRAM load off the critical path while Vector runs the dot
                nc.gpsimd.local_scatter(gdum[:], c16[:, OF_IX:OF_IX + 2],
                                        ixdum, channels=128, num_elems=2,
                                        num_idxs=2)
                # --- sliding norms ---
                nc.scalar.activation(sqx[:, 1:239], xpad[:], AF.Square)
                nc.vector.tensor_tensor_scan(cs[:], sqx[:],
                                             zero1[:].to_broadcast((P, 239)),
                                             0.0, Op.add, Op.bypass)
                nc.vector.tensor_tensor(nsq[:], cs[:, 80:239], cs[:, 0:159],
                                        Op.subtract)
                nc.vector.tensor_scalar_max(rnsq[:], nsq[:], 1e-30)
                nc.vector.reciprocal_approx_fast(rnsq[:], rnsq[:])
                # --- dot: fp16 product (2x) + fold-adds + small reduce ---
                nc.scalar.copy(xpad16[:], xpad[:])
                nc.vector.tensor_copy(yres16[:], yres[:])
                # band E1: s in [0,40), c in [40,80)
                nc.vector.tensor_tensor(
                    vap(w2[:], 40, [[80, 40], [1, 40]]),
                    vap(xpad16[:], 40, [[1, 40], [1, 40]]),
                    vap(yres16[:], 40, [[0, 40], [1, 40]]), Op.mult)
                # band C: s in [40,119), full c
                nc.vector.tensor_tensor(
                    vap(w2[:], 40 * 80, [[80, 79], [1, 80]]),
                    vap(xpad16[:], 40, [[1, 79], [1, 80]]),
                    vap(yres16[:], 0, [[0, 79], [1, 80]]), Op.mult)
                # band E2: s in [119,159), c in [0,40)
                nc.vector.tensor_tensor(
                    vap(w2[:], 119 * 80, [[80, 40], [1, 40]]),
                    vap(xpad16[:], 119, [[1, 40], [1, 40]]),
                    vap(yres16[:], 0, [[0, 40], [1, 40]]), Op.mult)
                with nc.allow_low_precision("argmax-only dot"):
                    nc.vector.tensor_copy(
                        vap(w4[:], 0, [[40, 40], [1, 40]]),
                        vap(w2[:], 40, [[80, 40], [1, 40]]))
                    nc.vector.tensor_tensor(
                        vap(w4[:], 40 * 40, [[40, 79], [1, 40]]),
                        vap(w2[:], 40 * 80, [[80, 79], [1, 40]]),
                        vap(w2[:], 40 * 80 + 40, [[80, 79], [1, 40]]), Op.add)
                    nc.vector.tensor_copy(
                        vap(w4[:], 119 * 40, [[40, 40], [1, 40]]),
                        vap(w2[:], 119 * 80, [[80, 40], [1, 40]]))
                    nc.vector.tensor_tensor(
                        w5[:].rearrange("p (s c) -> p s c", c=20),
                        vap(w4[:], 0, [[40, S1], [1, 20]]),
                        vap(w4[:], 20, [[40, S1], [1, 20]]), Op.add)
                    nc.vector.tensor_tensor(
                        w6[:].rearrange("p (s c) -> p s c", c=10),
                        vap(w5[:], 0, [[20, S1], [1, 10]]),
                        vap(w5[:], 10, [[20, S1], [1, 10]]), Op.add)
                    nc.vector.tensor_reduce(dot16[:],
                                            vap(w6[:], 0, [[10, S1], [1, 10]]),
                                            mybir.AxisListType.X, Op.add)
                # --- theta = argmax dot*|dot|/nsq ---
                nc.scalar.activation(adot[:], dot16[:], AF.Abs)
                nc.vector.tensor_tensor(gsel[:], dot16[:], adot[:], Op.mult)
                nc.vector.tensor_tensor(gsel[:], gsel[:], rnsq[:], Op.mult)
                nc.vector.max(mx8[:], gsel[:])
                nc.vector.max_index(mi8[:], mx8[:], gsel[:])
                nc.vector.tensor_copy(thf[:], mi8[:, 0:1])
                # --- y_align: scatter xpad16[79+j] -> yal[79+j-theta] ---
                nc.vector.scalar_tensor_tensor(ix1[:], io_t[:, 79:159],
                                               thf[:, 0:1], io_t[:, 79:159],
                                               Op.subtract, Op.bypass)
                nc.gpsimd.local_scatter(yal[:], xpad16[:, 79:159], ix1[:],
                                        channels=128, num_elems=256,
                                        num_idxs=80)
                # --- softmax attention -> y_att in yap16[:, 80:160] ---
                nc.vector.tensor_tensor(zt[:], yal[:, 0:80], yres[:], Op.mult)
                nc.scalar.activation(et[:], zt[:], AF.Exp,
                                     scale=1.0 / TEMPER,
                                     accum_out=ssum[:])
                nc.vector.reciprocal_approx_fast(rsum[:], ssum[:])
                nc.vector.scalar_tensor_tensor(yap16[:, 80:160], et[:],
                                                rsum[:, 0:1], yal[:, 0:80],
                                                Op.mult, Op.mult)
                # --- x_ele: scatter yap16[80+j] -> xele[j+theta-79] ---
                nc.vector.tensor_scalar(th2[:], thf[:], -1.0, 159.0,
                                        Op.mult, Op.add)
                nc.vector.scalar_tensor_tensor(ix2[:], io_t[:, 80:160],
                                               th2[:, 0:1], io_t[:, 80:160],
                                               Op.subtract, Op.bypass)
                nc.gpsimd.local_scatter(xele[:], yap16[:, 80:160], ix2[:],
                                        channels=128, num_elems=256,
                                        num_idxs=80)
                nc.vector.tensor_tensor(xpad[:, 79:159], xpad[:, 79:159],
                                        xele[:, 0:80], Op.subtract)
                # --- z features (fp16, packed 4096) ---
                foff = 0
                yb = yap16[:, 80:240]
                for d0, d1, im in ZBLOCKS:
                    nblk = (d1 - d0) * im
                    ov = bass.AP(zf16[:].tensor, zf16[:].offset + foff,
                                 [list(zf16[:].ap[0]), [im, d1 - d0], [1, im]])
                    b0 = bass.AP(yb.tensor, yb.offset,
                                 [list(yb.ap[0]), [0, d1 - d0], [1, im]])
                    b1 = bass.AP(yb.tensor, yb.offset + d0,
                                 [list(yb.ap[0]), [1, d1 - d0], [1, im]])
                    nc.vector.tensor_tensor(ov, b0, b1, Op.mult)
                    foff += nblk
                # --- E: transpose chunk-pairs -> one 256-wide copy (V/ACT
                #     alternating) -> token-major accumulate in PSUM ---
                Eps = ppe.tile([128, 81], F32, tag="Eps")
                zsb = [None] * NPAIR
                for k2 in range(NPAIR + 1):
                    if k2 < NPAIR:
                        zTp = pp.tile([128, 256], F32, tag="zTp")
                        nc.tensor.matmul(zTp[:, 0:128],
                                         zf16[:, (2 * k2) * 128:(2 * k2 + 1) * 128],
                                         id_t, start=True, stop=True)
                        nc.tensor.matmul(zTp[:, 128:256],
                                         zf16[:, (2 * k2 + 1) * 128:(2 * k2 + 2) * 128],
                                         id_t, start=True, stop=True)
                        zsb_k = zpool.tile([128, 256], F16, tag="zT")
                        zsb[k2] = zsb_k
                        if k2 % 4 == 3:
                            nc.vector.tensor_copy(zsb[k2][:], zTp[:])
                        else:
                            nc.scalar.copy(zsb[k2][:], zTp[:])
                    j2 = k2 - 1
                    if 0 <= j2 < NPAIR:
                        nc.tensor.matmul(Eps[:], zsb[j2][:, 0:128],
                                         Achunk(2 * j2),
                                         start=(j2 == 0), stop=False)
                        nc.tensor.matmul(Eps[:], zsb[j2][:, 128:256],
                                         Achunk(2 * j2 + 1),
                                         start=False, stop=False)
                # tail: feats [yaT(80); ones]
                yaTp = pp.tile([128, 256], F32, tag="zTp")
                nc.tensor.matmul(yaTp[0:80, 0:128], yap16[:, 80:160], id_t,
                                 start=True, stop=True)
                nc.scalar.copy(etail[0:80, :], yaTp[0:80, 0:128])
                nc.tensor.matmul(Eps[:], etail[:], At_t, start=False,
                                 stop=True)
                # --- s* argmax directly on PSUM, d* = 80 - s* ---
                nc.vector.max(mx8[:], Eps[:])
                nc.vector.max_index(mi8[:], mx8[:], Eps[:])
                nc.vector.tensor_copy(sf[:], mi8[:, 0:1])
                nc.vector.tensor_scalar(df[:], sf[:], -1.0, 80.0,
                                        Op.mult, Op.add)
                # --- yhat: scatter yap16[80+j] -> yhat[80+j-s*] ---
                nc.vector.scalar_tensor_tensor(ix3[:], io_t[:, 80:160],
                                               sf[:, 0:1], io_t[:, 80:160],
                                               Op.subtract, Op.bypass)
                nc.gpsimd.local_scatter(yhat[:], yap16[:, 80:160], ix3[:],
                                        channels=128, num_elems=256,
                                        num_idxs=80)
                # --- h_selT = W_enc @ yhat^T (+ b_enc) ---
                yhTp = pph.tile([128, 128], F32, tag="Hp")
                nc.tensor.matmul(yhTp[:], yhat[:, 0:128], id_t,
                                 start=True, stop=True)
                nc.scalar.copy(yhT0[:], yhTp[:])
                yhTp2 = pph.tile([128, 128], F32, tag="Hp")
                nc.tensor.matmul(yhTp2[0:32, :], yhat[:, 128:160], id_t,
                                 start=True, stop=True)
                nc.scalar.copy(yhT1[:], yhTp2[0:32, :])
                for hc in range(4):
                    Hp = pph.tile([128, 128], F32, tag="Hp")
                    nc.tensor.matmul(Hp[:], We_t[:, hc * 128:(hc + 1) * 128],
                                     yhT0[:], start=True, stop=False)
                    nc.tensor.matmul(Hp[:],
                                     We_t[0:32, 512 + hc * 128:512 + (hc + 1) * 128],
                                     yhT1[:], start=False, stop=True)
                    nc.scalar.activation(hsT[:, hc * 128:(hc + 1) * 128],
                                         Hp[:], AF.Identity,
                                         bias=be_t[:, hc:hc + 1])
                # --- x_extT = W_src @ h_selT (+ b_src) ---
                for oc in range(2):
                    ow = 128 if oc == 0 else 32
                    Xp = pph.tile([128, 128], F32, tag="Hp")
                    for hc in range(4):
                        nc.tensor.matmul(
                            Xp[0:ow, :],
                            Ws_t[:, hc * 160 + oc * 128: hc * 160 + oc * 128 + ow],
                            hsT[:, hc * 128:(hc + 1) * 128],
                            start=(hc == 0), stop=(hc == 3))
                    dst = xeT0 if oc == 0 else xeT1
                    nc.scalar.activation(dst[:], Xp[0:ow, :], AF.Identity,
                                         bias=bs_t[0:ow, oc:oc + 1])
                Xtp = pph.tile([128, 128], F32, tag="Hp")
                nc.tensor.matmul(Xtp[:], xeT0[:], id_t, start=True, stop=True)
                nc.scalar.copy(xext16[:, 0:128], Xtp[:])
                Xtp2 = pph.tile([128, 128], F32, tag="Hp")
                nc.tensor.matmul(Xtp2[:, 0:32], xeT1[:], c16[0:32, OF_ID:OF_ID + 32],
                                 start=True, stop=True)
                nc.scalar.copy(xext16[:, 128:160], Xtp2[:, 0:32])
                # --- y_ele: scatter xext16[j] -> yele[j-d*] ---
                nc.vector.scalar_tensor_tensor(ix4[:], io_t[:, 0:160],
                                               df[:, 0:1], io_t[:, 0:160],
                                               Op.subtract, Op.bypass)
                nc.gpsimd.local_scatter(yele[:], xext16[:], ix4[:],
                                        channels=128, num_elems=160,
                                        num_idxs=160)
                # --- loss partial + state updates ---
                nc.vector.tensor_tensor(dtmp[:], yele[:, 0:80], yres[:],
                                        Op.subtract)
                nc.vector.tensor_tensor(dtmp[:], dtmp[:], keep[:], Op.mult)
                nc.scalar.activation(dsq[:], dtmp[:], AF.Square,
                                     accum_out=lossp[:, it:it + 1])
                nc.vector.tensor_tensor(yres[:], yres[:], yele[:, 0:80],
                                        Op.subtract)

            nc.sync.dma_start(d_out[:], lossp[:])
    return nc


def kernel(x, y, W_enc, b_enc, W_src, b_src):
    import sys
    if '/opt/trn_rl_repo' not in sys.path:
        sys.path.insert(0, '/opt/trn_rl_repo')
    x = np.asarray(x, np.float32)
    y = np.asarray(y, np.float32)
    consts = _build_consts(W_enc, b_enc, W_src, b_src)

    if "nc" not in _cache:
        _cache["nc"] = _build_nc()
        _cache["nc"].finalize()
    nc = _cache["nc"]

    in_maps = _make_in_maps(x, y, consts)
    from concourse.bass_utils import run_bass_kernel_spmd
    res = run_bass_kernel_spmd(nc, in_maps, list(range(NCORES)))
    parts = np.stack([r["losspart"] for r in res.results])
    keep_cnt = max(int((y != 0.0).sum()), 1)
    nums = parts[:, :, :THINK_ITER].sum(axis=(0, 1), dtype=np.float64)
    losses = (nums / keep_cnt).astype(np.float32)
    return np.float32(np.mean(losses))


# revision 19
# speedup vs baseline: 7.3508x; 1.0514x over previous
"""Trainium2 Bass kernel for nn_Net_17532056502451.

5 "think" iterations: shift-window cosine selector (159 shifts) + softmax
attention + scatter-back + conv-style encoder/decoder with energy argmax
(81 shifts), masked-MSE losses averaged.  Data-parallel: 1024 tokens over
8 cores, 128 tokens/core (one per SBUF partition), token-major.

v4 design notes:
- dot correlation: fp16 broadcast-product (DVE 2x mode) + 2x fold-adds
  (80->40->20->10) + one small strided reduce.
- energy Gram features packed 6400 -> 4096 = 32 chunks of 128; whole PE
  path fp16 (1-pass matmuls, FWL); transposes via matmul against fp16
  identity; E-matmul operands swapped so E accumulates TOKEN-major in
  PSUM (argmax reads PSUM directly).
- E-pipeline: chunk PAIRS share one [128,256] PSUM tile; PSUM->SBUF
  copies are 256 wide and alternate Vector/Scalar engines.
- per-token dynamic window gathers via gpsimd local_scatter with
  per-partition indices (idx[p,j] = j - start_p, negatives ignored);
  gathers read only the 80 nonzero source columns.
- all constants pre-swizzled on host into one fp16 blob + one fp32 blob
  (3 input DMAs total instead of ~46).
"""
import numpy as np

IDIM = 80
ODIM = 80
HDIM = 512
THINK_ITER = 5
TEMPER = 0.7
B, T = 4, 256
NTOK = B * T
P = 128
NCORES = 8
S1 = 159
S2 = 81
ZBLOCKS = [(0, 32, 80), (32, 48, 48), (48, 64, 32), (64, 80, 16)]
NFEAT = sum((d1 - d0) * im for d0, d1, im in ZBLOCKS)   # 4096
NCHUNK = NFEAT // 128   # 32
NPAIR = NCHUNK // 2
# fp16 const blob column offsets
OF_A = 0
OF_AT = OF_A + NCHUNK * 81          # 2592
OF_WE = OF_AT + 81                  # 2673
OF_WS = OF_WE + 1024                # 3697
OF_ID = OF_WS + 640                 # 4337
OF_ON = OF_ID + 128                 # 4465
OF_IX = OF_ON + 128                 # 4593
W16 = OF_IX + 2                     # 4595
# fp32 const blob: benc(4) bsrc(2) iota(256)
OF_BE = 0
OF_BS = 4
OF_IO = 6
W32 = 262

_cache = {}


def _feat_list():
    feats = []
    for d0, d1, im in ZBLOCKS:
        for d in range(d0, d1):
            for i in range(im):
                feats.append((d, i))
    return feats


def _build_consts(W_enc, b_enc, W_src, b_src):
    W_enc = np.asarray(W_enc, np.float32)
    b_enc = np.asarray(b_enc, np.float32)
    W_src = np.asarray(W_src, np.float32)
    b_src = np.asarray(b_src, np.float32)
    C = (W_enc.T @ W_enc).astype(np.float32)
    q = (W_enc.T @ b_enc).astype(np.float32)
    bb = np.float32(b_enc @ b_enc)
    feats = _feat_list()
    Az = np.zeros((S2, NFEAT), np.float32)
    Al = np.zeros((S2, 81), np.float32)
    for s in range(S2):
        dd = 80 - s
        for f, (d, i) in enumerate(feats):
            if i < 80 - d:
                Az[s, f] = (2.0 if d > 0 else 1.0) * C[dd + i, dd + i + d]
        Al[s, :80] = 2.0 * q[dd:dd + 80]
        Al[s, 80] = bb
    c16 = np.zeros((P, W16), np.float16)
    # A: chunk k at cols OF_A + k*81, partition p holds Az.T[k*128+p, :]
    AzT = np.ascontiguousarray(Az.T).astype(np.float16)          # (4096, 81)
    c16[:, OF_A:OF_AT] = AzT.reshape(NCHUNK, 128, 81).transpose(1, 0, 2) \
                            .reshape(128, NCHUNK * 81)
    c16[0:81, OF_AT:OF_AT + 81] = np.ascontiguousarray(Al.T).astype(np.float16)
    WeT = np.ascontiguousarray(W_enc.T).astype(np.float16)       # (160, 512)
    c16[:, OF_WE:OF_WE + 512] = WeT[0:128]
    c16[0:32, OF_WE + 512:OF_WE + 1024] = WeT[128:160]
    WsT = np.ascontiguousarray(W_src.T).astype(np.float16)       # (512, 160)
    c16[:, OF_WS:OF_WS + 640] = WsT.reshape(4, 128, 160).transpose(1, 0, 2) \
                                   .reshape(128, 640)
    c16[:, OF_ID:OF_ID + 128] = np.eye(128, dtype=np.float16)
    c16[:, OF_ON:OF_ON + 128] = 1.0
    c16[:, OF_IX:OF_IX + 2] = np.broadcast_to(
        np.array([0, 1], np.int16).view(np.float16), (P, 2))
    c32 = np.zeros((P, W32), np.float32)
    c32[:, OF_BE:OF_BE + 4] = b_enc.reshape(4, 128).T
    c32[:, OF_BS] = b_src[0:128]
    c32[0:32, OF_BS + 1] = b_src[128:160]
    c32[:, OF_IO:OF_IO + 256] = np.arange(256, dtype=np.float32)
    return dict(c16=c16, c32=c32)


def _make_in_maps(x, y, consts):
    xt = x.reshape(NTOK, IDIM)
    yt = y.reshape(NTOK, ODIM)
    in_maps = []
    for c in range(NCORES):
        m = dict(consts)
        m["xy"] = np.ascontiguousarray(
            np.concatenate([xt[c * P:(c + 1) * P], yt[c * P:(c + 1) * P]],
                           axis=1))
        in_maps.append(m)
    return in_maps


def _build_nc():
    import concourse.bass as bass
    import concourse.bacc as bacc
    import concourse.mybir as mybir
    from concourse.tile import TileContext

    F32 = mybir.dt.float32
    F16 = mybir.dt.float16
    I16 = mybir.dt.int16
    U32 = mybir.dt.uint32
    Op = mybir.AluOpType
    AF = mybir.ActivationFunctionType

    nc = bacc.Bacc()
    d_xy = nc.declare_dram_parameter("xy", [P, 160], F32, isOutput=False)
    d_c16 = nc.declare_dram_parameter("c16", [P, W16], F16, isOutput=False)
    d_c32 = nc.declare_dram_parameter("c32", [P, W32], F32, isOutput=False)
    d_out = nc.declare_dram_parameter("losspart", [P, 8], F32, isOutput=True)

    with TileContext(nc) as tc:
        with (
            tc.tile_pool(name="const", bufs=1) as cpool,
            tc.tile_pool(name="work", bufs=1) as pool,
            tc.tile_pool(name="zrot", bufs=6) as zpool,
            tc.tile_pool(name="ps_rot", bufs=3, space="PSUM") as pp,
            tc.tile_pool(name="ps_h", bufs=3, space="PSUM") as pph,
            tc.tile_pool(name="ps_acc", bufs=1, space="PSUM") as ppe,
        ):
            # ---- inputs + constants (3 DMAs) ----
            xy_t = pool.tile([P, 160], F32, tag="xy")
            nc.sync.dma_start(xy_t[:], d_xy[:])
            c16 = cpool.tile([P, W16], F16, tag="c16")
            HH = W16 // 2
            nc.gpsimd.dma_start(c16[:, 0:HH], d_c16[:, 0:HH])
            nc.scalar.dma_start(c16[:, HH:W16], d_c16[:, HH:W16])
            c32 = cpool.tile([P, W32], F32, tag="c32")
            nc.sync.dma_start(c32[:], d_c32[:])

            def Achunk(k):
                return c16[:, OF_A + k * 81:OF_A + (k + 1) * 81]
            At_t = c16[0:81, OF_AT:OF_AT + 81]
            We_t = c16[:, OF_WE:OF_WE + 1024]
            Ws_t = c16[:, OF_WS:OF_WS + 640]
            id_t = c16[:, OF_ID:OF_ID + 128]
            be_t = c32[:, OF_BE:OF_BE + 4]
            bs_t = c32[:, OF_BS:OF_BS + 2]
            io_t = c32[:, OF_IO:OF_IO + 256]

            # ---- state ----
            xpad = pool.tile([P, 238], F32, tag="xpad")
            xpad16 = pool.tile([P, 238], F16, tag="xpad16")
            yres = pool.tile([P, 80], F32, tag="yres")
            keep = pool.tile([P, 80], F32, tag="keep")
            yap16 = pool.tile([P, 240], F16, tag="yap16")
            lossp = pool.tile([P, 8], F32, tag="lossp")
            nc.vector.memset(xpad[:], 0.0)
            nc.vector.memset(yap16[:], 0.0)
            nc.vector.memset(lossp[:], 0.0)
            nc.scalar.copy(xpad[:, 79:159], xy_t[:, 0:80])
            nc.vector.tensor_copy(yres[:], xy_t[:, 80:160])
            nc.vector.tensor_scalar(keep[:], yres[:], 0.0, None, Op.not_equal)

            sqx = pool.tile([P, 239], F32, tag="sqx")
            nc.vector.memset(sqx[:, 0:1], 0.0)
            cs = pool.tile([P, 239], F32, tag="cs")
            nsq = pool.tile([P, S1], F32, tag="nsq")
            rnsq = pool.tile([P, S1], F32, tag="rnsq")
            yres16 = pool.tile([P, 80], F16, tag="yres16")
            w2 = pool.tile([P, S1 * 80], F16, tag="w2")
            w4 = pool.tile([P, S1 * 40], F16, tag="w4")
            w5 = pool.tile([P, S1 * 20], F16, tag="w5")
            w6 = pool.tile([P, S1 * 10], F16, tag="w6")
            dot16 = pool.tile([P, S1], F16, tag="dot16")
            adot = pool.tile([P, S1], F16, tag="adot")
            gsel = pool.tile([P, S1], F32, tag="gsel")
            mx8 = pool.tile([P, 8], F32, tag="mx8")
            mi8 = pool.tile([P, 8], U32, tag="mi8")
            thf = pool.tile([P, 1], F32, tag="thf")
            th2 = pool.tile([P, 1], F32, tag="th2")
            sf = pool.tile([P, 1], F32, tag="sf")
            df = pool.tile([P, 1], F32, tag="df")
            ix1 = pool.tile([P, 80], I16, tag="ix1")
            ix2 = pool.tile([P, 80], I16, tag="ix2")
            ix3 = pool.tile([P, 80], I16, tag="ix3")
            ix4 = pool.tile([P, 160], I16, tag="ix4")
            yal = pool.tile([P, 256], F16, tag="yal")
            xele = pool.tile([P, 256], F16, tag="xele")
            yhat = pool.tile([P, 256], F16, tag="yhat")
            yele = pool.tile([P, 160], F16, tag="yele")
            zt = pool.tile([P, 80], F32, tag="zt")
            et = pool.tile([P, 80], F32, tag="et")
            ssum = pool.tile([P, 1], F32, tag="ssum")
            rsum = pool.tile([P, 1], F32, tag="rsum")
            nzm = pool.tile([P, 1], F32, tag="nzm")
            zero1 = pool.tile([P, 1], F32, tag="zero1")
            nc.vector.memset(zero1[:], 0.0)
            zf16 = pool.tile([P, NFEAT], F16, tag="zf16")
            etail = pool.tile([81, 128], F16, tag="etail")
            nc.sync.dma_start(etail[80:81, :], d_c16[80:81, OF_ON:OF_ON + 128])
            yhT0 = pool.tile([128, 128], F16, tag="yhT0")
            yhT1 = pool.tile([32, 128], F16, tag="yhT1")
            hsT = pool.tile([128, 4 * 128], F16, tag="hsT")
            xeT0 = pool.tile([128, 128], F16, tag="xeT0")
            xeT1 = pool.tile([32, 128], F16, tag="xeT1")
            xext16 = pool.tile([P, 160], F16, tag="xext16")
            dtmp = pool.tile([P, 80], F32, tag="dtmp")
            dsq = pool.tile([P, 80], F32, tag="dsq")
            gdum = pool.tile([P, 2], F16, tag="gdum")
            ixdum = c16[:, OF_IX:OF_IX + 2].bitcast(I16)

            def vap(tile_ap, free0, fdims):
                b = tile_ap
                return bass.AP(b.tensor, b.offset + free0,
                               [list(b.ap[0])] + list(fdims))

            nc.vector.memset(vap(w2[:], 0, [[80, 40], [1, 40]]), 0.0)
            nc.vector.memset(vap(w2[:], 119 * 80 + 40, [[80, 40], [1, 40]]), 0.0)

            for it in range(THINK_ITER):
                # gpsimd library warm-up: absorbs the MODIFY_POOL_CONFIG +
                # IRAM load off the critical path while Vector runs the dot
                nc.gpsimd.local_scatter(gdum[:], c16[:, OF_IX:OF_IX + 2],
                                        ixdum, channels=128, num_elems=2,
                                        num_idxs=2)
                # --- sliding norms ---
                nc.scalar.activation(sqx[:, 1:239], xpad[:], AF.Square)
                nc.vector.tensor_tensor_scan(cs[:], sqx[:],
                                             zero1[:].to_broadcast((P, 239)),
                                             0.0, Op.add, Op.bypass)
                nc.vector.tensor_tensor(nsq[:], cs[:, 80:239], cs[:, 0:159],
                                        Op.subtract)
                nc.vector.tensor_scalar_max(rnsq[:], nsq[:], 1e-30)
                nc.vector.reciprocal_approx_fast(rnsq[:], rnsq[:])
                # --- dot: fp16 product (2x) + fold-adds + small reduce ---
                nc.scalar.copy(xpad16[:], xpad[:])
                nc.vector.tensor_copy(yres16[:], yres[:])
                # band E1: s in [0,40), c in [40,80)
                nc.vector.tensor_tensor(
                    vap(w2[:], 40, [[80, 40], [1, 40]]),
                    vap(xpad16[:], 40, [[1, 40], [1, 40]]),
                    vap(yres16[:], 40, [[0, 40], [1, 40]]), Op.mult)
                # band C: s in [40,119), full c
                nc.vector.tensor_tensor(
                    vap(w2[:], 40 * 80, [[80, 79], [1, 80]]),
                    vap(xpad16[:], 40, [[1, 79], [1, 80]]),
                    vap(yres16[:], 0, [[0, 79], [1, 80]]), Op.mult)
                # band E2: s in [119,159), c in [0,40)
                nc.vector.tensor_tensor(
                    vap(w2[:], 119 * 80, [[80, 40], [1, 40]]),
                    vap(xpad16[:], 119, [[1, 40], [1, 40]]),
                    vap(yres16[:], 0, [[0, 40], [1, 40]]), Op.mult)
                with nc.allow_low_precision("argmax-only dot"):
                    nc.vector.tensor_copy(
                        vap(w4[:], 0, [[40, 40], [1, 40]]),
                        vap(w2[:], 40, [[80, 40], [1, 40]]))
                    nc.vector.tensor_tensor(
                        vap(w4[:], 40 * 40, [[40, 79], [1, 40]]),
                        vap(w2[:], 40 * 80, [[80, 79], [1, 40]]),
                        vap(w2[:], 40 * 80 + 40, [[80, 79], [1, 40]]), Op.add)
                    nc.vector.tensor_copy(
                        vap(w4[:], 119 * 40, [[40, 40], [1, 40]]),
                        vap(w2[:], 119 * 80, [[80, 40], [1, 40]]))
                    nc.vector.tensor_tensor(
                        w5[:].rearrange("p (s c) -> p s c", c=20),
                        vap(w4[:], 0, [[40, S1], [1, 20]]),
                        vap(w4[:], 20, [[40, S1], [1, 20]]), Op.add)
                    nc.vector.tensor_tensor(
                        w6[:].rearrange("p (s c) -> p s c", c=10),
                        vap(w5[:], 0, [[20, S1], [1, 10]]),
                        vap(w5[:], 10, [[20, S1], [1, 10]]), Op.add)
                    nc.vector.tensor_reduce(dot16[:],
                                            vap(w6[:], 0, [[10, S1], [1, 10]]),
                                            mybir.AxisListType.X, Op.add)
                # --- theta = argmax dot*|dot|/nsq ---
                nc.scalar.activation(adot[:], dot16[:], AF.Abs)
                nc.vector.tensor_tensor(gsel[:], dot16[:], adot[:], Op.mult)
                nc.vector.tensor_tensor(gsel[:], gsel[:], rnsq[:], Op.mult)
                nc.vector.max(mx8[:], gsel[:])
                nc.vector.max_index(mi8[:], mx8[:], gsel[:])
                nc.vector.tensor_copy(thf[:], mi8[:, 0:1])
                # --- y_align: scatter xpad16[79+j] -> yal[79+j-theta] ---
                nc.vector.scalar_tensor_tensor(ix1[:], io_t[:, 79:159],
                                               thf[:, 0:1], io_t[:, 79:159],
                                               Op.subtract, Op.bypass)
                nc.gpsimd.local_scatter(yal[:], xpad16[:, 79:159], ix1[:],
                                        channels=128, num_elems=256,
                                        num_idxs=80)
                # --- softmax attention -> y_att in yap16[:, 80:160] ---
                nc.vector.tensor_tensor(zt[:], yal[:, 0:80], yres[:], Op.mult)
                nc.scalar.activation(et[:], zt[:], AF.Exp,
                                     scale=1.0 / TEMPER,
                                     accum_out=ssum[:])
                nc.vector.reciprocal_approx_fast(rsum[:], ssum[:])
                nc.vector.scalar_tensor_tensor(yap16[:, 80:160], et[:],
                                                rsum[:, 0:1], yal[:, 0:80],
                                                Op.mult, Op.mult)
                # --- x_ele: scatter yap16[80+j] -> xele[j+theta-79] ---
                nc.vector.tensor_scalar(th2[:], thf[:], -1.0, 159.0,
                                        Op.mult, Op.add)
                nc.vector.scalar_tensor_tensor(ix2[:], io_t[:, 80:160],
                                               th2[:, 0:1], io_t[:, 80:160],
                                               Op.subtract, Op.bypass)
                nc.gpsimd.local_scatter(xele[:], yap16[:, 80:160], ix2[:],
                                        channels=128, num_elems=256,
                                        num_idxs=80)
                nc.vector.tensor_tensor(xpad[:, 79:159], xpad[:, 79:159],
                                        xele[:, 0:80], Op.subtract)
                # --- z features (fp16, packed 4096) ---
                foff = 0
                yb = yap16[:, 80:240]
                for d0, d1, im in ZBLOCKS:
                    nblk = (d1 - d0) * im
                    ov = bass.AP(zf16[:].tensor, zf16[:].offset + foff,
                                 [list(zf16[:].ap[0]), [im, d1 - d0], [1, im]])
                    b0 = bass.AP(yb.tensor, yb.offset,
                                 [list(yb.ap[0]), [0, d1 - d0], [1, im]])
                    b1 = bass.AP(yb.tensor, yb.offset + d0,
                                 [list(yb.ap[0]), [1, d1 - d0], [1, im]])
                    nc.vector.tensor_tensor(ov, b0, b1, Op.mult)
                    foff += nblk
                # --- E: transpose chunk-pairs -> one 256-wide copy (V/ACT
                #     alternating) -> token-major accumulate in PSUM ---
                Eps = ppe.tile([128, 81], F32, tag="Eps")
                zsb = [None] * NPAIR
                for k2 in range(NPAIR + 1):
                    if k2 < NPAIR:
                        zTp = pp.tile([128, 256], F32, tag="zTp")
                        nc.tensor.matmul(zTp[:, 0:128],
                                         zf16[:, (2 * k2) * 128:(2 * k2 + 1) * 128],
                                         id_t, start=True, stop=True)
                        nc.tensor.matmul(zTp[:, 128:256],
                                         zf16[:, (2 * k2 + 1) * 128:(2 * k2 + 2) * 128],
                                         id_t, start=True, stop=True)
                        zsb_k = zpool.tile([128, 256], F16, tag="zT")
                        zsb[k2] = zsb_k
                        if k2 % 4 == 3:
                            nc.vector.tensor_copy(zsb[k2][:], zTp[:])
                        else:
                            nc.scalar.copy(zsb[k2][:], zTp[:])
                    j2 = k2 - 1
                    if 0 <= j2 < NPAIR:
                        nc.tensor.matmul(Eps[:], zsb[j2][:, 0:128],
                                         Achunk(2 * j2),
                                         start=(j2 == 0), stop=False)
                        nc.tensor.matmul(Eps[:], zsb[j2][:, 128:256],
                                         Achunk(2 * j2 + 1),
                                         start=False, stop=False)
                # tail: feats [yaT(80); ones]
                yaTp = pp.tile([128, 256], F32, tag="zTp")
                nc.tensor.matmul(yaTp[0:80, 0:128], yap16[:, 80:160], id_t,
                                 start=True, stop=True)
                nc.scalar.copy(etail[0:80, :], yaTp[0:80, 0:128])
                nc.tensor.matmul(Eps[:], etail[:], At_t, start=False,
                                 stop=True)
                # --- s* argmax directly on PSUM, d* = 80 - s* ---
                nc.vector.max(mx8[:], Eps[:])
                nc.vector.max_index(mi8[:], mx8[:], Eps[:])
                nc.vector.tensor_copy(sf[:], mi8[:, 0:1])
                nc.vector.tensor_scalar(df[:], sf[:], -1.0, 80.0,
                                        Op.mult, Op.add)
                # --- yhat: scatter yap16[80+j] -> yhat[80+j-s*] ---
                nc.vector.scalar_tensor_tensor(ix3[:], io_t[:, 80:160],
                                               sf[:, 0:1], io_t[:, 80:160],
                                               Op.subtract, Op.bypass)
                nc.gpsimd.local_scatter(yhat[:], yap16[:, 80:160], ix3[:],
                                        channels=128, num_elems=256,
                                        num_idxs=80)
                # --- h_selT = W_enc @ yhat^T (+ b_enc) ---
                yhTp = pph.tile([128, 128], F32, tag="Hp")
                nc.tensor.matmul(yhTp[:], yhat[:, 0:128], id_t,
                                 start=True, stop=True)
                nc.scalar.copy(yhT0[:], yhTp[:])
                yhTp2 = pph.tile([128, 128], F32, tag="Hp")
                nc.tensor.matmul(yhTp2[0:32, :], yhat[:, 128:160], id_t,
                                 start=True, stop=True)
                nc.scalar.copy(yhT1[:], yhTp2[0:32, :])
                for hc in range(4):
                    Hp = pph.tile([128, 128], F32, tag="Hp")
                    nc.tensor.matmul(Hp[:], We_t[:, hc * 128:(hc + 1) * 128],
                                     yhT0[:], start=True, stop=False)
                    nc.tensor.matmul(Hp[:],
                                     We_t[0:32, 512 + hc * 128:512 + (hc + 1) * 128],
                                     yhT1[:], start=False, stop=True)
                    nc.scalar.activation(hsT[:, hc * 128:(hc + 1) * 128],
                                         Hp[:], AF.Identity,
                                         bias=be_t[:, hc:hc + 1])
                # --- x_extT = W_src @ h_selT (+ b_src) ---
                for oc in range(2):
                    ow = 128 if oc == 0 else 32
                    Xp = pph.tile([128, 128], F32, tag="Hp")
                    for hc in range(4):
                        nc.tensor.matmul(
                            Xp[0:ow, :],
                            Ws_t[:, hc * 160 + oc * 128: hc * 160 + oc * 128 + ow],
                            hsT[:, hc * 128:(hc + 1) * 128],
                            start=(hc == 0), stop=(hc == 3))
                    dst = xeT0 if oc == 0 else xeT1
                    nc.scalar.activation(dst[:], Xp[0:ow, :], AF.Identity,
                                         bias=bs_t[0:ow, oc:oc + 1])
                Xtp = pph.tile([128, 128], F32, tag="Hp")
                nc.tensor.matmul(Xtp[:], xeT0[:], id_t, start=True, stop=True)
                nc.scalar.copy(xext16[:, 0:128], Xtp[:])
                Xtp2 = pph.tile([128, 128], F32, tag="Hp")
                nc.tensor.matmul(Xtp2[:, 0:32], xeT1[:], c16[0:32, OF_ID:OF_ID + 32],
                                 start=True, stop=True)
                nc.scalar.copy(xext16[:, 128:160], Xtp2[:, 0:32])
                # --- y_ele: scatter xext16[j] -> yele[j-d*] ---
                nc.vector.scalar_tensor_tensor(ix4[:], io_t[:, 0:160],
                                               df[:, 0:1], io_t[:, 0:160],
                                               Op.subtract, Op.bypass)
                nc.gpsimd.local_scatter(yele[:], xext16[:], ix4[:],
                                        channels=128, num_elems=160,
                                        num_idxs=160)
                # --- loss partial + state updates ---
                nc.vector.tensor_tensor(dtmp[:], yele[:, 0:80], yres[:],
                                        Op.subtract)
                nc.vector.tensor_tensor(dtmp[:], dtmp[:], keep[:], Op.mult)
                nc.scalar.activation(dsq[:], dtmp[:], AF.Square,
                                     accum_out=lossp[:, it:it + 1])
                nc.vector.tensor_tensor(yres[:], yres[:], yele[:, 0:80],
                                        Op.subtract)

            nc.sync.dma_start(d_out[:], lossp[:])
    return nc


def kernel(x, y, W_enc, b_enc, W_src, b_src):
    import sys
    if '/opt/trn_rl_repo' not in sys.path:
        sys.path.insert(0, '/opt/trn_rl_repo')
    x = np.asarray(x, np.float32)
    y = np.asarray(y, np.float32)
    consts = _build_consts(W_enc, b_enc, W_src, b_src)

    if "nc" not in _cache:
        _cache["nc"] = _build_nc()
        _cache["nc"].finalize()
    nc = _cache["nc"]

    in_maps = _make_in_maps(x, y, consts)
    from concourse.bass_utils import run_bass_kernel_spmd
    res = run_bass_kernel_spmd(nc, in_maps, list(range(NCORES)))
    parts = np.stack([r["losspart"] for r in res.results])
    keep_cnt = max(int((y != 0.0).sum()), 1)
    nums = parts[:, :, :THINK_ITER].sum(axis=(0, 1), dtype=np.float64)
    losses = (nums / keep_cnt).astype(np.float32)
    return np.float32(np.mean(losses))
